# revision 1
# baseline (speedup 1.0000x reference)
"""Trainium2 Bass kernel for nn_Attention (channel attention, XCA-style).

Sharding: 8 cores = (batch b=core//2) x (image half = core%2, 64 rows + halo).
Cross-core: AllReduce of tiny gram stats over core pairs [[0,1],[2,3],...].

Per-core pipeline (single NEFF, SPMD):
  pass1: conv1x1 q,k (bf16 matmuls) -> depthwise 3x3 on DVE (tensor_scalar 4x
         products + tensor_tensor 2x adds, bf16) -> DMA-xbar transpose ->
         gram matmuls accumulated in PSUM + sq-norms via ScalarE Square
  AllReduce([192,26]) over pairs; rsqrt-scale + softmax on-chip
  pass2: conv1x1 v -> depthwise via 9 diagonal matmuls on PE (PSUM acc)
         -> block-diag attn@v -> proj -> DMA out
"""

import sys
import numpy as np

sys.path.insert(0, "/opt/trn_rl_repo")

import contextlib  # noqa: E402

import ml_dtypes  # noqa: E402

from concourse import bass, bacc, tile, mybir  # noqa: E402
from concourse import bass_utils  # noqa: E402

F32 = mybir.dt.float32
BF16 = mybir.dt.bfloat16
ALU = mybir.AluOpType
ACTF = mybir.ActivationFunctionType
AX = mybir.AxisListType
BF16NP = ml_dtypes.bfloat16

C = 192
HEADS = 8
CH = 24
W = 128
HOUT = 64
HIN = HOUT + 2
WS = 132                  # padded row stride in t buffers
PXIN = HIN * W            # 8448
PXOUT = HOUT * W          # 8192

RS = 16                   # stripe out-rows
NS = HOUT // RS
RIN = RS + 2
SPXI = RIN * W            # 2304
SPXO = RS * W             # 2048
LT = RIN * WS             # t buffer length (2376)
MMCH = 512

TAPS = [(dy, dx) for dy in (0, 1, 2) for dx in (0, 1, 2)]

_CACHE = {}


def _chunks(total, step):
    out, s = [], 0
    while s < total:
        out.append((s, min(step, total - s)))
        s += step
    return out


def build_program():
    nc = bacc.Bacc("TRN2", target_bir_lowering=False, debug=False,
                   enable_asserts=False, num_devices=8)
    io = {}
    io["xs"] = nc.dram_tensor("xs", [C, PXIN], F32, kind="ExternalInput").ap()
    io["ys"] = nc.dram_tensor("ys", [C, PXIN], F32, kind="ExternalInput").ap()
    for nm in ("wq", "wk", "wv"):
        io[nm] = nc.dram_tensor(nm, [C, C], BF16, kind="ExternalInput").ap()
    io["wp1"] = nc.dram_tensor("wp1", [120, C], BF16, kind="ExternalInput").ap()
    io["wp2"] = nc.dram_tensor("wp2", [72, C], BF16, kind="ExternalInput").ap()
    io["dqk"] = nc.dram_tensor("dqk", [2 * C, 9], F32, kind="ExternalInput").ap()
    io["dqkd"] = nc.dram_tensor("dqkd", [2 * 9 * 128, 128], BF16,
                                kind="ExternalInput").ap()
    io["dvw"] = nc.dram_tensor("dvw", [C, 9], F32, kind="ExternalInput").ap()
    io["tmpq"] = nc.dram_tensor("tmpq", [C, 1], F32, kind="ExternalInput").ap()
    io["em"] = nc.dram_tensor("em", [HEADS, C], BF16, kind="ExternalInput").ap()
    io["eye"] = nc.dram_tensor("eye", [128, 128], F32,
                               kind="ExternalInput").ap()
    io["outp"] = nc.dram_tensor("outp", [C, PXOUT], F32,
                                kind="ExternalOutput").ap()

    with tile.TileContext(nc) as tc, contextlib.ExitStack() as es:
        _emit(nc, tc, io, es)
    nc.compile()
    return nc


def _emit(nc, tc, io, es):
    # ---------------- persistent weights ------------------------------
    wpool = es.enter_context(tc.tile_pool(name="w", bufs=1))
    wq_a = wpool.tile([128, C], BF16, tag="wqa")
    wq_b = wpool.tile([64, C], BF16, tag="wqb")
    wk_a = wpool.tile([128, C], BF16, tag="wka")
    wk_b = wpool.tile([64, C], BF16, tag="wkb")
    wv_a = wpool.tile([128, C], BF16, tag="wva")
    wv_b = wpool.tile([64, C], BF16, tag="wvb")
    wp1_t = wpool.tile([120, C], BF16, tag="wp1")
    wp2_t = wpool.tile([72, C], BF16, tag="wp2")
    nc.sync.dma_start(wq_a[:], io["wq"][0:128, :])
    nc.sync.dma_start(wq_b[:], io["wq"][128:192, :])
    nc.sync.dma_start(wk_a[:], io["wk"][0:128, :])
    nc.sync.dma_start(wk_b[:], io["wk"][128:192, :])
    nc.sync.dma_start(wv_a[:], io["wv"][0:128, :])
    nc.sync.dma_start(wv_b[:], io["wv"][128:192, :])
    nc.sync.dma_start(wp1_t[:], io["wp1"][:])
    nc.sync.dma_start(wp2_t[:], io["wp2"][:])
    dqk_t = wpool.tile([128, 3 * 9], F32, tag="dqk")
    for blk in range(3):
        nc.sync.dma_start(dqk_t[:, blk * 9:(blk + 1) * 9],
                          io["dqk"][blk * 128:(blk + 1) * 128, :])
    dqkd_t = [wpool.tile([128, 9 * 128], BF16, tag=f"dqkd{i}",
                         name=f"dqkd{i}") for i in range(2)]
    for i in range(2):
        for t in range(9):
            nc.sync.dma_start(
                dqkd_t[i][:, t * 128:(t + 1) * 128],
                io["dqkd"][(i * 9 + t) * 128:(i * 9 + t + 1) * 128, :])
    dvw1_t = wpool.tile([120, 9], F32, tag="dvw1")
    dvw2_t = wpool.tile([72, 9], F32, tag="dvw2")
    nc.sync.dma_start(dvw1_t[:], io["dvw"][0:120, :])
    nc.sync.dma_start(dvw2_t[:], io["dvw"][120:192, :])
    tmpq_t = wpool.tile([128, 2], F32, tag="tmpq")
    nc.sync.dma_start(tmpq_t[:, 0:1], io["tmpq"][0:128, :])
    nc.sync.dma_start(tmpq_t[0:64, 1:2], io["tmpq"][128:192, :])
    em_t = wpool.tile([HEADS, C], BF16, tag="em")
    nc.sync.dma_start(em_t[:], io["em"][:])
    eye_t = wpool.tile([128, 128], F32, tag="eye")
    nc.sync.dma_start(eye_t[:], io["eye"][:])

    # ---------------- pools -------------------------------------------
    inp = es.enter_context(tc.tile_pool(name="inp", bufs=1))     # fp32 input stripes
    inb = es.enter_context(tc.tile_pool(name="inb", bufs=2))     # bf16 casted inputs
    tbuf = es.enter_context(tc.tile_pool(name="tbuf", bufs=1))
    dwo = es.enter_context(tc.tile_pool(name="dwo", bufs=1))
    stck = es.enter_context(tc.tile_pool(name="stck", bufs=1))
    small = es.enter_context(tc.tile_pool(name="small", bufs=1))
    outsb = es.enter_context(tc.tile_pool(name="outsb", bufs=2))
    drm = es.enter_context(tc.tile_pool(name="drm", bufs=1, space="DRAM"))

    qn_part = small.tile([128, 3 * NS], F32, tag="qnp")
    

    # pre-zero both slots of each double-buffered padded-t tag (pads must
    # stay zero; taps only ever read them)
    for b in range(3):
        for _sl in range(2):
            tz = tbuf.tile([128, LT], BF16, tag=f"t{b}", name=f"tz{b}_{_sl}",
                           bufs=2)
            nc.vector.memset(tz[:], 0.0)
            if b == 0:
                sz = tbuf.tile([128, LT], BF16, tag=f"ts{b}",
                               name=f"tsz{b}_{_sl}", bufs=1)
                nc.vector.memset(sz[:], 0.0)

    # ================= PASS 1 =========================================
    with tc.tile_pool(name="cps", bufs=1, space="PSUM") as cpsum, \
         tc.tile_pool(name="gps", bufs=1, space="PSUM") as gpsum:
        ga_ps = gpsum.tile([128, C], F32, tag="ga")
        gb_ps = gpsum.tile([128, 256], F32, tag="gb")
        for s in range(NS):
            i0 = s * RS * W
            t_blk = [tbuf.tile([128, LT], BF16, tag=f"t{b}",
                               name=f"t{b}_{s}", bufs=2) for b in range(3)]
            tsh_blk = [tbuf.tile([128, LT], BF16, tag="ts0",
                                 name=f"ts0_{s}", bufs=1)]
            dwq_t = {b: dwo.tile([128, SPXO], BF16, tag=f"dw{b}",
                                 name=f"dw{b}_{s}", bufs=2) for b in (0, 2)}

            xa = inp.tile([128, SPXI], F32, tag="xa")
            xb = inp.tile([64, SPXI], F32, tag="xb")
            ya = inp.tile([128, SPXI], F32, tag="ya")
            yb = inp.tile([64, SPXI], F32, tag="yb")
            nc.gpsimd.dma_start(xa[:], io["xs"][0:128, i0:i0 + SPXI])
            nc.gpsimd.dma_start(xb[:], io["xs"][128:192, i0:i0 + SPXI])
            nc.gpsimd.dma_start(ya[:], io["ys"][0:128, i0:i0 + SPXI])
            nc.gpsimd.dma_start(yb[:], io["ys"][128:192, i0:i0 + SPXI])
            xa16 = inb.tile([128, SPXI], BF16, tag="xa16", bufs=1)
            xb16 = inb.tile([64, SPXI], BF16, tag="xb16", bufs=1)
            ya16 = inb.tile([128, SPXI], BF16, tag="ya16", bufs=1)
            yb16 = inb.tile([64, SPXI], BF16, tag="yb16", bufs=1)
            nc.vector.tensor_copy(xa16[:], xa[:])
            nc.vector.tensor_copy(xb16[:], xb[:])
            nc.vector.tensor_copy(ya16[:], ya[:])
            nc.vector.tensor_copy(yb16[:], yb[:])

            # conv -> t blocks: [q 0:128] | [q 128:192 ; k 0:64] | [k 64:192]
            for n0, n in _chunks(SPXI, MMCH):
                r0, nr = n0 // W, n // W
                ps0 = cpsum.tile([128, MMCH], F32, tag="cps0")
                ps1 = cpsum.tile([128, MMCH], F32, tag="cps1")
                ps1k = cpsum.tile([128, MMCH], F32, tag="cps1k")
                ps2 = cpsum.tile([128, MMCH], F32, tag="cps2")
                nc.tensor.matmul(ps0[:, 0:n], wq_a[:, 0:128],
                                 ya16[:, n0:n0 + n], start=True, stop=False)
                nc.tensor.matmul(ps0[:, 0:n], wq_b[:, 0:128],
                                 yb16[:, n0:n0 + n], start=False, stop=True)
                nc.tensor.matmul(ps1[0:64, 0:n], wq_a[:, 128:192],
                                 ya16[:, n0:n0 + n], start=True, stop=False)
                nc.tensor.matmul(ps1[0:64, 0:n], wq_b[:, 128:192],
                                 yb16[:, n0:n0 + n], start=False, stop=True)
                nc.tensor.matmul(ps1k[64:128, 0:n], wk_a[:, 0:64],
                                 xa16[:, n0:n0 + n], start=True, stop=False,
                                 tile_position=(0, 64))
                nc.tensor.matmul(ps1k[64:128, 0:n], wk_b[:, 0:64],
                                 xb16[:, n0:n0 + n], start=False, stop=True,
                                 tile_position=(0, 64))
                nc.tensor.matmul(ps2[:, 0:n], wk_a[:, 64:192],
                                 xa16[:, n0:n0 + n], start=True, stop=False)
                nc.tensor.matmul(ps2[:, 0:n], wk_b[:, 64:192],
                                 xb16[:, n0:n0 + n], start=False, stop=True)
                for b, (ps, lo, hi) in enumerate(((ps0, 0, 128),
                                                  (ps1, 0, 64),
                                                  (ps2, 0, 128))):
                    dst = t_blk[b][:].rearrange("p (r w) -> p r w", w=WS)
                    nc.scalar.copy(
                        dst[lo:hi, r0:r0 + nr, 2:130],
                        ps[lo:hi, 0:n].rearrange("p (r w) -> p r w", w=W))
                dst1 = t_blk[1][:].rearrange("p (r w) -> p r w", w=WS)
                nc.scalar.copy(
                    dst1[64:128, r0:r0 + nr, 2:130],
                    ps1k[64:128, 0:n].rearrange("p (r w) -> p r w", w=W))

            # t_sh = t shifted by +1 element (only DVE block 0 needs it)
            nc.vector.tensor_copy(tsh_blk[0][:, 0:LT - 2],
                                  t_blk[0][:, 1:LT - 1])

            # depthwise block 0 on DVE: 9 taps (4x products + 2x adds)
            for b in (0,):
                acc = dwq_t[b]
                prod = dwo.tile([128, SPXO], BF16, tag="prod", bufs=2)
                accv = acc[:].rearrange("p (r w) -> p r w", w=W)
                prodv = prod[:].rearrange("p (r w) -> p r w", w=W)
                for ti, (dy, dx) in enumerate(TAPS):
                    sc = dqk_t[:, b * 9 + ti:b * 9 + ti + 1]
                    if dx == 1:
                        src = t_blk[b][:].rearrange("p (r w) -> p r w", w=WS)
                        view = src[:, dy:dy + RS, 2:130]
                    else:
                        src = tsh_blk[0][:].rearrange("p (r w) -> p r w", w=WS)
                        view = src[:, dy:dy + RS, dx:dx + 128]
                    dstv = accv if ti == 0 else prodv
                    nc.vector.tensor_scalar(dstv, view, sc, None, ALU.mult)
                    if ti > 0:
                        nc.vector.tensor_tensor(acc[:], acc[:], prod[:],
                                                ALU.add)

            # depthwise block 2 on PE (channel-major diag matmuls)
            for b in (2,):
                t3 = t_blk[b][:].rearrange("p (r w) -> p r w", w=WS)
                dgt = dqkd_t[b - 1]
                for n0, n in _chunks(SPXO, MMCH):
                    r0, nr = n0 // W, n // W
                    dps = cpsum.tile([128, MMCH], F32, tag=f"dw{b}ps",
                                     name=f"dwps{b}_{s}_{n0}")
                    for ti, (dy, dx) in enumerate(TAPS):
                        nc.tensor.matmul(
                            dps[:, 0:n], dgt[:, ti * 128:(ti + 1) * 128],
                            t3[:, r0 + dy:r0 + dy + nr, 1 + dx:129 + dx],
                            start=(ti == 0), stop=(ti == 8))
                    nc.scalar.copy(dwq_t[b][:, n0:n0 + n], dps[:, 0:n])

            # sq-norm partials (junk output reuses the prod slot)
            junk = dwo.tile([128, SPXO], BF16, tag="prod",
                            name=f"junk_{s}", bufs=2)
            for b in (0, 2):
                col = b * NS + s
                nc.scalar.activation(
                    junk[:], dwq_t[b][:], ACTF.Square,
                    accum_out=qn_part[:, col:col + 1])

            # stacks: b0,b2 via DMA-xbar transpose; b1 via transposed
            # depthwise on PE (lhsT = t-window, rhs = diag)
            t31 = t_blk[1][:].rearrange("p (r w) -> p r w", w=WS)
            nchunk = SPXO // 128
            for g in range(nchunk // 4):
                stack = stck.tile([128, 4 * 384], BF16, tag="stack",
                                  name=f"stack_{s}_{g}", bufs=4)
                for ci in range(4):
                    cix = g * 4 + ci
                    p0, st0 = cix * 128, ci * 384
                    for b in (0, 2):
                        nc.sync.dma_start_transpose(
                            stack[:, st0 + b * 128:st0 + (b + 1) * 128],
                            dwq_t[b][:, p0:p0 + 128])
                    dps1 = cpsum.tile([128, 128], F32, tag="dw1ps",
                                      name=f"tdw1_{s}_{cix}")
                    for ti, (dy, dx) in enumerate(TAPS):
                        nc.tensor.matmul(
                            dps1[:], t31[:, cix + dy, 1 + dx:129 + dx],
                            dqkd_t[0][:, ti * 128:(ti + 1) * 128],
                            start=(ti == 0), stop=(ti == 8))
                    nc.scalar.copy(stack[:, st0 + 128:st0 + 256], dps1[:])
                for ci in range(4):
                    cix = g * 4 + ci
                    st0 = ci * 384
                    first = (s == 0 and cix == 0)
                    last = (s == NS - 1 and cix == nchunk - 1)
                    nc.tensor.matmul(ga_ps[:], stack[:, st0:st0 + 128],
                                     stack[:, st0 + 192:st0 + 384],
                                     start=first, stop=last)
                    nc.tensor.matmul(gb_ps[:], stack[:, st0 + 128:st0 + 256],
                                     stack[:, st0 + 128:st0 + 384],
                                     start=first, stop=last)

        # ---- gram -> SBUF, norms, bounce assembly (DMA, crosses parts)
        ga_sb = small.tile([128, C], F32, tag="gasb")
        g1_sb = small.tile([128, 256], F32, tag="g1sb")
        nc.scalar.copy(ga_sb[:], ga_ps[:])
        nc.scalar.copy(g1_sb[:], gb_ps[:])
        gb_sb = small.tile([64, C], F32, tag="gbsb")
        nc.vector.tensor_copy(gb_sb[:], g1_sb[0:64, 64:256])

        # block-1 sq-norms = diag(G1[:, 0:128]) via eye-mask + row sum
        g1m = small.tile([128, 128], F32, tag="g1m")
        nc.vector.tensor_tensor(g1m[:], g1_sb[:, 0:128], eye_t[:], ALU.mult)

    # ================= PASS 2 =========================================
    with tc.tile_pool(name="p2ps", bufs=1, space="PSUM") as pps:
        tv_tiles = {}
        for _s in range(NS):
            _tva = tbuf.tile([120, LT], BF16, tag="tv_{}".format(_s),
                             name=f"tvz_{_s}", bufs=1)
            _tvb = tbuf.tile([72, LT], BF16, tag="tvb_{}".format(_s),
                             name=f"tvbz_{_s}", bufs=1)
            nc.vector.memset(_tva[:], 0.0)
            nc.vector.memset(_tvb[:], 0.0)

        def p2_conv(s):
            i0 = s * RS * W
            tv_a = tbuf.tile([120, LT], BF16, tag="tv_{}".format(s),
                             name=f"tva_{s}", bufs=1)
            tv_b = tbuf.tile([72, LT], BF16, tag="tvb_{}".format(s),
                             name=f"tvb_{s}", bufs=1)
            tv_tiles[s] = (tv_a, tv_b)
            xa = inp.tile([128, SPXI], F32, tag="xa")
            xb = inp.tile([64, SPXI], F32, tag="xb")
            nc.gpsimd.dma_start(xa[:], io["xs"][0:128, i0:i0 + SPXI])
            nc.gpsimd.dma_start(xb[:], io["xs"][128:192, i0:i0 + SPXI])
            xa16 = inb.tile([128, SPXI], BF16, tag="xa16", bufs=1)
            xb16 = inb.tile([64, SPXI], BF16, tag="xb16", bufs=1)
            nc.vector.tensor_copy(xa16[:], xa[:])
            nc.vector.tensor_copy(xb16[:], xb[:])
            for n0, n in _chunks(SPXI, MMCH):
                r0, nr = n0 // W, n // W
                ps0 = pps.tile([120, MMCH], F32, tag="cv0")
                ps1 = pps.tile([72, MMCH], F32, tag="cv1")
                nc.tensor.matmul(ps0[:, 0:n], wv_a[:, 0:120],
                                 xa16[:, n0:n0 + n], start=True, stop=False)
                nc.tensor.matmul(ps0[:, 0:n], wv_b[:, 0:120],
                                 xb16[:, n0:n0 + n], start=False, stop=True)
                nc.tensor.matmul(ps1[:, 0:n], wv_a[:, 120:192],
                                 xa16[:, n0:n0 + n], start=True, stop=False)
                nc.tensor.matmul(ps1[:, 0:n], wv_b[:, 120:192],
                                 xb16[:, n0:n0 + n], start=False, stop=True)
                dsta = tv_a[:].rearrange("p (r w) -> p r w", w=WS)
                dstb = tv_b[:].rearrange("p (r w) -> p r w", w=WS)
                nc.scalar.copy(dsta[0:120, r0:r0 + nr, 2:130],
                               ps0[:, 0:n].rearrange("p (r w) -> p r w", w=W))
                nc.scalar.copy(dstb[0:72, r0:r0 + nr, 2:130],
                               ps1[:, 0:n].rearrange("p (r w) -> p r w", w=W))

        vb_tiles = {}

        def p2_vbdw(s):
            # v[120:192] depthwise on DVE (channel-major), PE does attn only
            tv_b = tv_tiles[s][1]
            tshb = tbuf.tile([72, LT], BF16, tag="tshb", name=f"tshb_{s}")
            nc.vector.tensor_copy(tshb[:, 0:LT - 2], tv_b[:, 1:LT - 1])
            vb = dwo.tile([72, SPXO], BF16, tag=f"vbdw{s % 2}",
                          name=f"vbdw_{s}", bufs=1)
            vb_tiles[s] = vb
            prodb = dwo.tile([72, SPXO], BF16, tag="prodb", name=f"prodb_{s}")
            vbv = vb[:].rearrange("p (r w) -> p r w", w=W)
            prodbv = prodb[:].rearrange("p (r w) -> p r w", w=W)
            for ti, (dy, dx) in enumerate(TAPS):
                sc = dvw2_t[:, ti:ti + 1]
                if dx == 1:
                    s3 = tv_b[:].rearrange("p (r w) -> p r w", w=WS)
                    view = s3[:, dy:dy + RS, 2:130]
                else:
                    s3 = tshb[:].rearrange("p (r w) -> p r w", w=WS)
                    view = s3[:, dy:dy + RS, dx:dx + 128]
                dstv = vbv if ti == 0 else prodbv
                nc.vector.tensor_scalar(dstv, view, sc, None, ALU.mult)
                if ti > 0:
                    nc.vector.tensor_tensor(vb[:], vb[:], prodb[:], ALU.add)

        def p2_attn(s):
            o0 = s * SPXO
            tv_a, tv_b = tv_tiles.pop(s)
            tva3 = tv_a[:].rearrange("p (r w) -> p r w", w=WS)
            vb = vb_tiles.pop(s)
            for n0, n in _chunks(SPXO, MMCH):
                r0, nr = n0 // W, n // W
                ops1 = pps.tile([120, MMCH], F32, tag="ops1", bufs=2)
                ops2 = pps.tile([72, MMCH], F32, tag="ops2", bufs=2)
                for ti, (dy, dx) in enumerate(TAPS):
                    nc.tensor.matmul(
                        ops1[:, 0:n], bd1t[:, ti * 120:(ti + 1) * 120],
                        tva3[0:120, r0 + dy:r0 + dy + nr, 1 + dx:129 + dx],
                        start=(ti == 0), stop=(ti == 8))
                nc.tensor.matmul(ops2[:, 0:n], bd2[:], vb[:, n0:n0 + n],
                                 start=True, stop=True)
                ao1 = dwo.tile([120, MMCH], BF16, tag="ao1", bufs=2)
                ao2 = dwo.tile([72, MMCH], BF16, tag="ao2", bufs=2)
                nc.scalar.copy(ao1[:, 0:n], ops1[:, 0:n])
                nc.scalar.copy(ao2[:, 0:n], ops2[:, 0:n])
                ppa = pps.tile([128, MMCH], F32, tag="ppa")
                ppb = pps.tile([64, MMCH], F32, tag="ppb")
                nc.tensor.matmul(ppa[:, 0:n], wp1_t[:, 0:128], ao1[:, 0:n],
                                 start=True, stop=False)
                nc.tensor.matmul(ppa[:, 0:n], wp2_t[:, 0:128], ao2[:, 0:n],
                                 start=False, stop=True)
                nc.tensor.matmul(ppb[:, 0:n], wp1_t[:, 128:192], ao1[:, 0:n],
                                 start=True, stop=False)
                nc.tensor.matmul(ppb[:, 0:n], wp2_t[:, 128:192], ao2[:, 0:n],
                                 start=False, stop=True)
                oa = outsb.tile([128, MMCH], F32, tag="oa")
                ob = outsb.tile([64, MMCH], F32, tag="ob")
                nc.scalar.copy(oa[:, 0:n], ppa[:, 0:n])
                nc.scalar.copy(ob[:, 0:n], ppb[:, 0:n])
                nc.sync.dma_start(io["outp"][0:128, o0 + n0:o0 + n0 + n],
                                  oa[:, 0:n])
                nc.sync.dma_start(io["outp"][128:192, o0 + n0:o0 + n0 + n],
                                  ob[:, 0:n])

        # all conv stripes first -> overlap the collective + attn glue
        p2_conv(0)
        p2_conv(1)
        p2_conv(2)
        p2_conv(3)
        p2_vbdw(0)
        p2_vbdw(1)
        qn_red = small.tile([128, 3], F32, tag="qnr")
        for b in (0, 2):
            nc.vector.tensor_reduce(
                qn_red[:, b:b + 1],
                qn_part[:, b * NS:(b + 1) * NS], AX.X, ALU.add)
        nc.vector.tensor_reduce(qn_red[:, 1:2], g1m[:], AX.X, ALU.add)

        bounce_in = drm.tile([C, 26], F32)
        bounce_out = drm.tile([C, 26], F32)
        # per-head qk gram blocks -> bounce cols 0:24
        for h in range(HEADS):
            r0 = h * CH
            if r0 + CH <= 128:
                nc.sync.dma_start(bounce_in[r0:r0 + CH, 0:CH],
                                  ga_sb[r0:r0 + CH, r0:r0 + CH])
            elif r0 >= 128:
                nc.sync.dma_start(bounce_in[r0:r0 + CH, 0:CH],
                                  gb_sb[r0 - 128:r0 - 128 + CH, r0:r0 + CH])
            else:  # straddles the 128 boundary (head 5)
                nc.sync.dma_start(bounce_in[r0:128, 0:CH],
                                  ga_sb[r0:128, r0:r0 + CH])
                nc.sync.dma_start(bounce_in[128:r0 + CH, 0:CH],
                                  gb_sb[0:r0 + CH - 128, r0:r0 + CH])
        # qn -> col 24  (block0 = q0:128, block1 top = q128:192)
        nc.sync.dma_start(bounce_in[0:128, 24:25], qn_red[:, 0:1])
        nc.sync.dma_start(bounce_in[128:192, 24:25], qn_red[0:64, 1:2])
        # kn -> col 25  (block1 bottom = k0:64, block2 = k64:192)
        nc.sync.dma_start(bounce_in[0:64, 25:26], qn_red[64:128, 1:2])
        nc.sync.dma_start(bounce_in[64:192, 25:26], qn_red[0:128, 2:3])

        nc.gpsimd.collective_compute(
            "AllReduce", ALU.add,
            replica_groups=[[0, 1], [2, 3], [4, 5], [6, 7]],
            ins=[bounce_in[:].opt()], outs=[bounce_out[:].opt()])

        cmp_a = small.tile([128, 26], F32, tag="cmpa")
        cmp_b = small.tile([64, 26], F32, tag="cmpb")
        nc.sync.dma_start(cmp_a[:], bounce_out[0:128, :])
        nc.sync.dma_start(cmp_b[:], bounce_out[128:192, :])

        kn8 = small.tile([HEADS, CH], F32, tag="kn8")
        nc.sync.dma_start(
            kn8[:], bounce_out[:].rearrange("(h c) k -> h c k", c=CH)[:, :, 25])

        # rq = temp/sqrt(qn); rk = 1/sqrt(kn) as [8,24]
        rq_a = small.tile([128, 3], F32, tag="rqa")
        rq_b = small.tile([64, 3], F32, tag="rqb")
        for ti, (cmp, rq, nrow) in enumerate(((cmp_a, rq_a, 128),
                                              (cmp_b, rq_b, 64))):
            nc.scalar.activation(rq[:, 0:1], cmp[:, 24:25], ACTF.Sqrt)
            nc.vector.reciprocal(rq[:, 1:2], rq[:, 0:1])
            nc.vector.tensor_scalar(rq[:, 2:3], rq[:, 1:2],
                                    tmpq_t[0:nrow, ti:ti + 1], None, ALU.mult)
        rk8 = small.tile([HEADS, 2 * CH], F32, tag="rk8")
        nc.scalar.activation(rk8[:, 0:CH], kn8[:], ACTF.Sqrt)
        nc.vector.reciprocal(rk8[:, CH:2 * CH], rk8[:, 0:CH])
        rk8b = small.tile([HEADS, CH], BF16, tag="rk8b")
        nc.vector.tensor_copy(rk8b[:], rk8[:, CH:2 * CH])

        knb_a = small.tile([128, CH], F32, tag="knba")
        knb_b = small.tile([64, CH], F32, tag="knbb")
        knb_ps = pps.tile([128, CH], F32, tag="ppa", name="knb_ps")
        nc.tensor.matmul(knb_ps[:], em_t[:, 0:128], rk8b[:],
                         start=True, stop=True)
        nc.scalar.copy(knb_a[:], knb_ps[:])
        knb_ps2 = pps.tile([128, CH], F32, tag="ppa", name="knb_ps2")
        nc.tensor.matmul(knb_ps2[0:64, :], em_t[:, 128:192], rk8b[:],
                         start=True, stop=True)
        nc.scalar.copy(knb_b[:], knb_ps2[0:64, :])

        # s = A*rq*knb ; softmax over d (free dim)
        attn16 = small.tile([128, CH], BF16, tag="att16a")
        attn16b = small.tile([64, CH], BF16, tag="att16b")
        for cmp, rq, knb, a16, nrow in ((cmp_a, rq_a, knb_a, attn16, 128),
                                        (cmp_b, rq_b, knb_b, attn16b, 64)):
            at = small.tile([128, CH], F32, tag="atf")
            sm = small.tile([128, 4], F32, tag="sm")
            nc.vector.tensor_scalar(at[0:nrow, :], cmp[0:nrow, 0:CH], rq[:, 2:3],
                                    None, ALU.mult)
            nc.vector.tensor_tensor(at[0:nrow, :], at[0:nrow, :], knb[:],
                                    ALU.mult)
            nc.vector.tensor_reduce(sm[0:nrow, 0:1], at[0:nrow, :], AX.X, ALU.max)
            nc.vector.tensor_scalar(at[0:nrow, :], at[0:nrow, :], sm[0:nrow, 0:1],
                                    None, ALU.subtract)
            nc.scalar.activation(at[0:nrow, :], at[0:nrow, :], ACTF.Exp)
            nc.vector.tensor_reduce(sm[0:nrow, 1:2], at[0:nrow, :], AX.X, ALU.add)
            nc.vector.reciprocal(sm[0:nrow, 2:3], sm[0:nrow, 1:2])
            nc.vector.tensor_scalar(a16[0:nrow, :], at[0:nrow, :],
                                    sm[0:nrow, 2:3], None, ALU.mult)

        # block-diag attn^T via DRAM round-trip (transposing strided DMAs)
        attn_d = drm.tile([C, CH], BF16)
        nc.sync.dma_start(attn_d[0:128, :], attn16[:])
        nc.sync.dma_start(attn_d[128:192, :], attn16b[:])
        bd1 = small.tile([120, 120], BF16, tag="bd1")      # heads 0-4 attn^T
        bd2 = small.tile([72, 72], BF16, tag="bd2")        # heads 5-7 attn^T
        nc.vector.memset(bd1[:], 0.0)
        nc.vector.memset(bd2[:], 0.0)
        for h in range(5):
            r0 = h * CH
            nc.sync.dma_start(bd1[r0:r0 + CH, r0:r0 + CH],
                              attn_d[r0:r0 + CH, :].rearrange("c d -> d c"))
        for h in range(5, 8):
            r0 = (h - 5) * CH
            nc.sync.dma_start(bd2[r0:r0 + CH, r0:r0 + CH],
                              attn_d[h * CH:(h + 1) * CH, :].rearrange("c d -> d c"))
        # fold depthwise-v tap weights into attn^T: bd{1,2}_tap = bd * w_v[d,tap]
        bd1t = small.tile([120, 9 * 120], BF16, tag="bd1t")
        bd2t = small.tile([72, 9 * 72], BF16, tag="bd2t")
        for ti in range(9):
            nc.vector.tensor_scalar(bd1t[:, ti * 120:(ti + 1) * 120], bd1[:],
                                    dvw1_t[:, ti:ti + 1], None, ALU.mult)
            nc.vector.tensor_scalar(bd2t[:, ti * 72:(ti + 1) * 72], bd2[:],
                                    dvw2_t[:, ti:ti + 1], None, ALU.mult)

        p2_attn(0)
        p2_vbdw(2)
        p2_attn(1)
        p2_vbdw(3)
        p2_attn(2)
        p2_attn(3)


# ======================================================================
def _prep_inputs(x, y, qkv_w, dw_w, proj_w, temperature):
    wq_t = np.ascontiguousarray(qkv_w[0:C].T).astype(BF16NP)
    wk_t = np.ascontiguousarray(qkv_w[C:2 * C].T).astype(BF16NP)
    wv_t = np.ascontiguousarray(qkv_w[2 * C:3 * C].T).astype(BF16NP)
    wp_t = np.ascontiguousarray(proj_w.T).astype(np.float32)
    wp1 = wp_t[0:120].astype(BF16NP)
    wp2 = wp_t[120:192].astype(BF16NP)
    dw = dw_w.reshape(3 * C, 9).astype(np.float32)
    dw_q, dw_k, dw_v = dw[0:C], dw[C:2 * C], dw[2 * C:3 * C]
    dqk = np.concatenate([dw_q[0:128], dw_q[128:192], dw_k[0:64],
                          dw_k[64:192]], axis=0)
    dqkd = np.zeros((2 * 9 * 128, 128), np.float32)
    for i in range(2):
        for t in range(9):
            blk = dqk[(i + 1) * 128:(i + 2) * 128, t]
            np.fill_diagonal(
                dqkd[(i * 9 + t) * 128:(i * 9 + t + 1) * 128, :], blk)
    tmpq = np.repeat(np.asarray(temperature, np.float32).reshape(HEADS),
                     CH).reshape(C, 1)
    em = np.zeros((HEADS, C), np.float32)
    for hh in range(HEADS):
        em[hh, hh * CH:(hh + 1) * CH] = 1.0

    in_maps = []
    for core in range(8):
        bi, half = core // 2, core % 2
        r0 = half * HOUT - 1
        xsl = np.zeros((C, HIN, W), np.float32)
        ysl = np.zeros((C, HIN, W), np.float32)
        lo, hi = max(r0, 0), min(r0 + HIN, 128)
        xsl[:, lo - r0:hi - r0] = x[bi, :, lo:hi]
        ysl[:, lo - r0:hi - r0] = y[bi, :, lo:hi]
        in_maps.append({
            "xs": xsl.reshape(C, PXIN), "ys": ysl.reshape(C, PXIN),
            "wq": wq_t, "wk": wk_t, "wv": wv_t,
            "wp1": wp1, "wp2": wp2,
            "dqk": dqk, "dqkd": dqkd.astype(BF16NP),
            "dvw": dw_v.astype(np.float32),
            "tmpq": tmpq, "em": em.astype(BF16NP),
            "eye": np.eye(128, dtype=np.float32),
        })
    return in_maps


def kernel(x, y, qkv_w, dw_w, proj_w, temperature, _trace=False):
    x = np.asarray(x, np.float32)
    y = np.asarray(y, np.float32)
    if "nc" not in _CACHE:
        _CACHE["nc"] = build_program()
    nc = _CACHE["nc"]
    in_maps = _prep_inputs(x, y, np.asarray(qkv_w, np.float32),
                           np.asarray(dw_w, np.float32),
                           np.asarray(proj_w, np.float32),
                           np.asarray(temperature, np.float32))
    res = bass_utils.run_bass_kernel_spmd(nc, in_maps,
                                          core_ids=list(range(8)),
                                          trace=_trace)
    _CACHE["last_result"] = res
    out = np.empty((4, C, 128, W), np.float32)
    for core in range(8):
        bi, half = core // 2, core % 2
        out[bi, :, half * HOUT:(half + 1) * HOUT] = \
            res.results[core]["outp"].reshape(C, HOUT, W)
    return out



# revision 18
# speedup vs baseline: 1.1452x; 1.1452x over previous
"""Trainium2 Bass kernel for nn_Attention (channel attention, XCA-style).

Sharding: 8 cores = (batch b=core//2) x (image half = core%2, 64 rows + halo).
Cross-core: AllReduce of tiny gram stats over core pairs [[0,1],[2,3],...].

Per-core pipeline (single NEFF, SPMD):
  pass1: conv1x1 q,k (bf16 matmuls) -> depthwise 3x3 on DVE (tensor_scalar 4x
         products + tensor_tensor 2x adds, bf16) -> DMA-xbar transpose ->
         gram matmuls accumulated in PSUM + sq-norms via ScalarE Square
  AllReduce([192,26]) over pairs; rsqrt-scale + softmax on-chip
  pass2: conv1x1 v -> depthwise via 9 diagonal matmuls on PE (PSUM acc)
         -> block-diag attn@v -> proj -> DMA out
"""

import sys
import numpy as np

sys.path.insert(0, "/opt/trn_rl_repo")

import contextlib  # noqa: E402

import ml_dtypes  # noqa: E402

from concourse import bass, bacc, tile, mybir  # noqa: E402
from concourse import bass_utils  # noqa: E402

F32 = mybir.dt.float32
BF16 = mybir.dt.bfloat16
ALU = mybir.AluOpType
ACTF = mybir.ActivationFunctionType
AX = mybir.AxisListType
BF16NP = ml_dtypes.bfloat16

C = 192
HEADS = 8
CH = 24
W = 128
HOUT = 64
HIN = HOUT + 2
WS = 132                  # padded row stride in t buffers
PXIN = HIN * W            # 8448
PXOUT = HOUT * W          # 8192

RS = 16                   # stripe out-rows
NS = HOUT // RS
RIN = RS + 2
SPXI = RIN * W            # 2304
SPXO = RS * W             # 2048
LT = RIN * WS             # t buffer length (2376)
MMCH = 512

TAPS = [(dy, dx) for dy in (0, 1, 2) for dx in (0, 1, 2)]

_CACHE = {}


def _chunks(total, step):
    out, s = [], 0
    while s < total:
        out.append((s, min(step, total - s)))
        s += step
    return out


def build_program():
    nc = bacc.Bacc("TRN2", target_bir_lowering=False, debug=False,
                   enable_asserts=False, num_devices=8)
    io = {}
    io["xs"] = nc.dram_tensor("xs", [C, PXIN], BF16, kind="ExternalInput").ap()
    io["ys"] = nc.dram_tensor("ys", [C, PXIN], BF16, kind="ExternalInput").ap()
    for nm in ("wq", "wk", "wv"):
        io[nm] = nc.dram_tensor(nm, [C, C], BF16, kind="ExternalInput").ap()
    io["wp1"] = nc.dram_tensor("wp1", [120, C], BF16, kind="ExternalInput").ap()
    io["wp2"] = nc.dram_tensor("wp2", [72, C], BF16, kind="ExternalInput").ap()
    io["dqk"] = nc.dram_tensor("dqk", [2 * C, 9], F32, kind="ExternalInput").ap()
    io["dqkd"] = nc.dram_tensor("dqkd", [2 * 9 * 128, 128], BF16,
                                kind="ExternalInput").ap()
    io["dvw"] = nc.dram_tensor("dvw", [C, 9], F32, kind="ExternalInput").ap()
    io["tmpq"] = nc.dram_tensor("tmpq", [C, 1], F32, kind="ExternalInput").ap()
    io["em"] = nc.dram_tensor("em", [HEADS, C], BF16, kind="ExternalInput").ap()
    io["eye"] = nc.dram_tensor("eye", [128, 128], F32,
                               kind="ExternalInput").ap()
    io["outp"] = nc.dram_tensor("outp", [C, PXOUT], F32,
                                kind="ExternalOutput").ap()

    with tile.TileContext(nc) as tc, contextlib.ExitStack() as es:
        _emit(nc, tc, io, es)
    nc.compile()
    return nc


def _emit(nc, tc, io, es):
    # ---------------- persistent weights ------------------------------
    wpool = es.enter_context(tc.tile_pool(name="w", bufs=1))
    wq_a = wpool.tile([128, C], BF16, tag="wqa")
    wq_b = wpool.tile([64, C], BF16, tag="wqb")
    wk_a = wpool.tile([128, C], BF16, tag="wka")
    wk_b = wpool.tile([64, C], BF16, tag="wkb")
    wv_a = wpool.tile([128, C], BF16, tag="wva")
    wv_b = wpool.tile([64, C], BF16, tag="wvb")
    wp1_t = wpool.tile([120, C], BF16, tag="wp1")
    wp2_t = wpool.tile([72, C], BF16, tag="wp2")
    nc.sync.dma_start(wq_a[:], io["wq"][0:128, :])
    nc.sync.dma_start(wq_b[:], io["wq"][128:192, :])
    nc.sync.dma_start(wk_a[:], io["wk"][0:128, :])
    nc.sync.dma_start(wk_b[:], io["wk"][128:192, :])
    nc.sync.dma_start(wv_a[:], io["wv"][0:128, :])
    nc.sync.dma_start(wv_b[:], io["wv"][128:192, :])
    nc.sync.dma_start(wp1_t[:], io["wp1"][:])
    nc.sync.dma_start(wp2_t[:], io["wp2"][:])
    dqk_t = wpool.tile([128, 3 * 9], F32, tag="dqk")
    for blk in range(3):
        nc.sync.dma_start(dqk_t[:, blk * 9:(blk + 1) * 9],
                          io["dqk"][blk * 128:(blk + 1) * 128, :])
    dqkd_t = [wpool.tile([128, 9 * 128], BF16, tag=f"dqkd{i}",
                         name=f"dqkd{i}") for i in range(2)]
    for i in range(2):
        for t in range(9):
            nc.sync.dma_start(
                dqkd_t[i][:, t * 128:(t + 1) * 128],
                io["dqkd"][(i * 9 + t) * 128:(i * 9 + t + 1) * 128, :])
    dvw1_t = wpool.tile([120, 9], F32, tag="dvw1")
    dvw2_t = wpool.tile([72, 9], F32, tag="dvw2")
    nc.sync.dma_start(dvw1_t[:], io["dvw"][0:120, :])
    nc.sync.dma_start(dvw2_t[:], io["dvw"][120:192, :])
    tmpq_t = wpool.tile([128, 2], F32, tag="tmpq")
    nc.sync.dma_start(tmpq_t[:, 0:1], io["tmpq"][0:128, :])
    nc.sync.dma_start(tmpq_t[0:64, 1:2], io["tmpq"][128:192, :])
    em_t = wpool.tile([HEADS, C], BF16, tag="em")
    nc.sync.dma_start(em_t[:], io["em"][:])
    eye_t = wpool.tile([128, 128], F32, tag="eye")
    nc.sync.dma_start(eye_t[:], io["eye"][:])
    eye16 = wpool.tile([128, 128], BF16, tag="eye16")
    nc.vector.tensor_copy(eye16[:], eye_t[:])

    # ---------------- pools -------------------------------------------
    inb = es.enter_context(tc.tile_pool(name="inb", bufs=1))     # bf16 inputs
    tbuf = es.enter_context(tc.tile_pool(name="tbuf", bufs=1))
    dwo = es.enter_context(tc.tile_pool(name="dwo", bufs=1))
    stck = es.enter_context(tc.tile_pool(name="stck", bufs=1))
    small = es.enter_context(tc.tile_pool(name="small", bufs=1))
    outsb = es.enter_context(tc.tile_pool(name="outsb", bufs=2))
    drm = es.enter_context(tc.tile_pool(name="drm", bufs=1, space="DRAM"))

    qn_part = small.tile([128, 3 * NS], F32, tag="qnp")
    x_tiles = {}
    

    # pre-zero only the pad columns of each padded-t slot (cols 0:2 and
    # 130:132 of every row; data region is fully overwritten each stripe)
    for b in range(3):
        for _sl in range(2):
            tz = tbuf.tile([128, LT], BF16, tag=f"t{b}", name=f"tz{b}_{_sl}",
                           bufs=2)
            tzv = tz[:].rearrange("p (r w) -> p r w", w=WS)
            nc.vector.memset(tzv[:, :, 0:2], 0.0)
            nc.vector.memset(tzv[:, :, 130:132], 0.0)
            if b == 0 and _sl == 0:
                sz = tbuf.tile([128, LT], BF16, tag=f"ts{b}",
                               name=f"tsz{b}_{_sl}", bufs=1)
                nc.vector.memset(sz[:, LT - 2:LT], 0.0)

    # ================= PASS 1 =========================================
    # gram layout: gab_ps = [ga: q0:128 x k0:192 | selfg2: k64:192 self]
    #              gb_ps  = [b1 self (diag -> norms) | b1 x b2]
    with tc.tile_pool(name="cps", bufs=1, space="PSUM") as cpsum, \
         tc.tile_pool(name="gps", bufs=1, space="PSUM") as gpsum:
        gab_ps = gpsum.tile([128, 320], F32, tag="gab")
        gb_ps = gpsum.tile([128, 256], F32, tag="gb")
        for s in range(NS):
            i0 = s * RS * W
            t_blk = [tbuf.tile([128, LT], BF16, tag=f"t{b}",
                               name=f"t{b}_{s}", bufs=2) for b in range(3)]
            tsh_blk = [tbuf.tile([128, LT], BF16, tag="ts0",
                                 name=f"ts0_{s}", bufs=1)]
            dwq0 = dwo.tile([128, SPXO], BF16, tag="dw0",
                            name=f"dw0_{s}", bufs=2)

            # x stripes stay resident for pass 2 (per-stripe tags)
            xa16 = inb.tile([128, SPXI], BF16, tag=f"xa16_{s}", bufs=1)
            xb16 = inb.tile([64, SPXI], BF16, tag=f"xb16_{s}", bufs=1)
            x_tiles[s] = (xa16, xb16)
            ya16 = inb.tile([128, SPXI], BF16, tag="ya16", bufs=2)
            yb16 = inb.tile([64, SPXI], BF16, tag="yb16", bufs=2)
            nc.gpsimd.dma_start(xa16[:], io["xs"][0:128, i0:i0 + SPXI])
            nc.gpsimd.dma_start(xb16[:], io["xs"][128:192, i0:i0 + SPXI])
            nc.gpsimd.dma_start(ya16[:], io["ys"][0:128, i0:i0 + SPXI])
            nc.gpsimd.dma_start(yb16[:], io["ys"][128:192, i0:i0 + SPXI])

            # conv -> t blocks: [q 0:128] | [q 128:192 ; k 0:64] | [k 64:192]
            for n0, n in _chunks(SPXI, MMCH):
                r0, nr = n0 // W, n // W
                ps0 = cpsum.tile([128, MMCH], F32, tag="cps0")
                ps1 = cpsum.tile([128, MMCH], F32, tag="cps1")
                ps2 = cpsum.tile([128, MMCH], F32, tag="cps2")
                nc.tensor.matmul(ps0[:, 0:n], wq_a[:, 0:128],
                                 ya16[:, n0:n0 + n], start=True, stop=False)
                nc.tensor.matmul(ps0[:, 0:n], wq_b[:, 0:128],
                                 yb16[:, n0:n0 + n], start=False, stop=True)
                nc.tensor.matmul(ps1[0:64, 0:n], wq_a[:, 128:192],
                                 ya16[:, n0:n0 + n], start=True, stop=False)
                nc.tensor.matmul(ps1[0:64, 0:n], wq_b[:, 128:192],
                                 yb16[:, n0:n0 + n], start=False, stop=True)
                nc.tensor.matmul(ps1[64:128, 0:n], wk_a[:, 0:64],
                                 xa16[:, n0:n0 + n], start=True, stop=False,
                                 tile_position=(0, 64))
                nc.tensor.matmul(ps1[64:128, 0:n], wk_b[:, 0:64],
                                 xb16[:, n0:n0 + n], start=False, stop=True,
                                 tile_position=(0, 64))
                nc.tensor.matmul(ps2[:, 0:n], wk_a[:, 64:192],
                                 xa16[:, n0:n0 + n], start=True, stop=False)
                nc.tensor.matmul(ps2[:, 0:n], wk_b[:, 64:192],
                                 xb16[:, n0:n0 + n], start=False, stop=True)
                for b, (ps, lo, hi) in enumerate(((ps0, 0, 128),
                                                  (ps1, 0, 128),
                                                  (ps2, 0, 128))):
                    dst = t_blk[b][:].rearrange("p (r w) -> p r w", w=WS)
                    nc.scalar.copy(
                        dst[lo:hi, r0:r0 + nr, 2:130],
                        ps[lo:hi, 0:n].rearrange("p (r w) -> p r w", w=W))

            # t_sh = t shifted by +1 element (only DVE block 0 needs it)
            nc.vector.tensor_copy(tsh_blk[0][:, 0:LT - 2],
                                  t_blk[0][:, 1:LT - 1])

            # depthwise block 0 on DVE: 9 taps (4x products + 2x adds)
            accv = dwq0[:].rearrange("p (r w) -> p r w", w=W)
            prod = dwo.tile([128, SPXO], BF16, tag="prod", bufs=2)
            prodv = prod[:].rearrange("p (r w) -> p r w", w=W)
            for ti, (dy, dx) in enumerate(TAPS):
                sc = dqk_t[:, ti:ti + 1]
                if dx == 1:
                    src = t_blk[0][:].rearrange("p (r w) -> p r w", w=WS)
                    view = src[:, dy:dy + RS, 2:130]
                else:
                    src = tsh_blk[0][:].rearrange("p (r w) -> p r w", w=WS)
                    view = src[:, dy:dy + RS, dx:dx + 128]
                dstv = accv if ti == 0 else prodv
                nc.vector.tensor_scalar(dstv, view, sc, None, ALU.mult)
                if ti > 0:
                    nc.vector.tensor_tensor(dwq0[:], dwq0[:], prod[:],
                                            ALU.add)

            # sq-norm partial for q0:128 (junk output reuses the prod slot)
            junk = dwo.tile([128, SPXO], BF16, tag="prod",
                            name=f"junk_{s}", bufs=2)
            nc.scalar.activation(
                junk[:], dwq0[:], ACTF.Square,
                accum_out=qn_part[:, s:s + 1])

            # px-major stacks: b0 via PE transpose of the DVE dw output;
            # b1,b2 via transposed depthwise on PE (lhsT = t-window)
            t31 = t_blk[1][:].rearrange("p (r w) -> p r w", w=WS)
            t32 = t_blk[2][:].rearrange("p (r w) -> p r w", w=WS)
            nchunk = SPXO // 128
            for g in range(nchunk // 4):
                psA = cpsum.tile([128, MMCH], BF16, tag="tpsA")
                psB = cpsum.tile([128, MMCH], F32, tag="tpsB")
                psC = cpsum.tile([128, MMCH], F32, tag="tpsC")
                stk = [stck.tile([128, MMCH], BF16, tag=f"stk{b}",
                                 name=f"stk{b}_{s}_{g}", bufs=2)
                       for b in range(3)]
                for ci in range(4):
                    cix = g * 4 + ci
                    p0, q0 = cix * 128, ci * 128
                    nc.tensor.transpose(psA[:, q0:q0 + 128],
                                        dwq0[:, p0:p0 + 128], eye16[:])
                    for ti, (dy, dx) in enumerate(TAPS):
                        nc.tensor.matmul(
                            psB[:, q0:q0 + 128],
                            t31[:, cix + dy, 1 + dx:129 + dx],
                            dqkd_t[0][:, ti * 128:(ti + 1) * 128],
                            start=(ti == 0), stop=(ti == 8))
                    for ti, (dy, dx) in enumerate(TAPS):
                        nc.tensor.matmul(
                            psC[:, q0:q0 + 128],
                            t32[:, cix + dy, 1 + dx:129 + dx],
                            dqkd_t[1][:, ti * 128:(ti + 1) * 128],
                            start=(ti == 0), stop=(ti == 8))
                nc.vector.tensor_copy(stk[0][:], psA[:])
                nc.scalar.copy(stk[1][:], psB[:])
                nc.vector.tensor_copy(stk[2][:], psC[:])
                for ci in range(4):
                    cix = g * 4 + ci
                    q0 = ci * 128
                    first = (s == 0 and cix == 0)
                    last = (s == NS - 1 and cix == nchunk - 1)
                    s0c = stk[0][:, q0:q0 + 128]
                    s1c = stk[1][:, q0:q0 + 128]
                    s2c = stk[2][:, q0:q0 + 128]
                    nc.tensor.matmul(gab_ps[:, 0:64], s0c,
                                     stk[1][:, q0 + 64:q0 + 128],
                                     start=first, stop=last)
                    nc.tensor.matmul(gab_ps[:, 64:192], s0c, s2c,
                                     start=first, stop=last)
                    nc.tensor.matmul(gab_ps[:, 192:320], s2c, s2c,
                                     start=first, stop=last)
                    nc.tensor.matmul(gb_ps[:, 0:128], s1c, s1c,
                                     start=first, stop=last)
                    nc.tensor.matmul(gb_ps[:, 128:256], s1c, s2c,
                                     start=first, stop=last)

        # ---- gram -> SBUF, norms, bounce assembly (DMA, crosses parts)
        ga_sb = small.tile([128, C], F32, tag="gasb")
        g1_sb = small.tile([128, 256], F32, tag="g1sb")
        nc.scalar.copy(ga_sb[:], gab_ps[:, 0:192])
        nc.scalar.copy(g1_sb[:], gb_ps[:])
        gb_sb = small.tile([64, C], F32, tag="gbsb")
        nc.vector.tensor_copy(gb_sb[:], g1_sb[0:64, 64:256])

        # block-1 sq-norms = diag(G1[:, 0:128]) via eye-mask + row sum;
        # block-2 (k64:192) sq-norms = diag(selfg2) the same way
        g1m = small.tile([128, 128], F32, tag="g1m")
        nc.vector.tensor_tensor(g1m[:], g1_sb[:, 0:128], eye_t[:], ALU.mult)
        g2m = small.tile([128, 128], F32, tag="g2m")
        nc.vector.tensor_tensor(g2m[:], gab_ps[:, 192:320], eye_t[:],
                                ALU.mult)

    # ================= PASS 2 =========================================
    with tc.tile_pool(name="p2ps", bufs=1, space="PSUM") as pps:
        tv_tiles = {}
        for _s in range(NS):
            _tva = tbuf.tile([120, LT], BF16, tag="tv_{}".format(_s),
                             name=f"tvz_{_s}", bufs=1)
            _tvb = tbuf.tile([72, LT], BF16, tag="tvb_{}".format(_s),
                             name=f"tvbz_{_s}", bufs=1)
            for _t in (_tva, _tvb):
                _tvv = _t[:].rearrange("p (r w) -> p r w", w=WS)
                nc.vector.memset(_tvv[:, :, 0:2], 0.0)
                nc.vector.memset(_tvv[:, :, 130:132], 0.0)

        def p2_conv(s):
            tv_a = tbuf.tile([120, LT], BF16, tag="tv_{}".format(s),
                             name=f"tva_{s}", bufs=1)
            tv_b = tbuf.tile([72, LT], BF16, tag="tvb_{}".format(s),
                             name=f"tvb_{s}", bufs=1)
            tv_tiles[s] = (tv_a, tv_b)
            xa16, xb16 = x_tiles.pop(s)
            for n0, n in _chunks(SPXI, MMCH):
                r0, nr = n0 // W, n // W
                ps0 = pps.tile([120, MMCH], F32, tag="cv0")
                ps1 = pps.tile([72, MMCH], F32, tag="cv1")
                nc.tensor.matmul(ps0[:, 0:n], wv_a[:, 0:120],
                                 xa16[:, n0:n0 + n], start=True, stop=False)
                nc.tensor.matmul(ps0[:, 0:n], wv_b[:, 0:120],
                                 xb16[:, n0:n0 + n], start=False, stop=True)
                nc.tensor.matmul(ps1[:, 0:n], wv_a[:, 120:192],
                                 xa16[:, n0:n0 + n], start=True, stop=False)
                nc.tensor.matmul(ps1[:, 0:n], wv_b[:, 120:192],
                                 xb16[:, n0:n0 + n], start=False, stop=True)
                dsta = tv_a[:].rearrange("p (r w) -> p r w", w=WS)
                dstb = tv_b[:].rearrange("p (r w) -> p r w", w=WS)
                nc.scalar.copy(dsta[0:120, r0:r0 + nr, 2:130],
                               ps0[:, 0:n].rearrange("p (r w) -> p r w", w=W))
                nc.scalar.copy(dstb[0:72, r0:r0 + nr, 2:130],
                               ps1[:, 0:n].rearrange("p (r w) -> p r w", w=W))

        vb_tiles = {}

        def p2_vbdw(s):
            # v[120:192] depthwise on DVE (channel-major), PE does attn only
            tv_b = tv_tiles[s][1]
            tshb = tbuf.tile([72, LT], BF16, tag="tshb", name=f"tshb_{s}")
            nc.vector.tensor_copy(tshb[:, 0:LT - 2], tv_b[:, 1:LT - 1])
            vb = dwo.tile([72, SPXO], BF16, tag=f"vbdw{s % 2}",
                          name=f"vbdw_{s}", bufs=1)
            vb_tiles[s] = vb
            prodb = dwo.tile([72, SPXO], BF16, tag="prodb", name=f"prodb_{s}")
            vbv = vb[:].rearrange("p (r w) -> p r w", w=W)
            prodbv = prodb[:].rearrange("p (r w) -> p r w", w=W)
            for ti, (dy, dx) in enumerate(TAPS):
                sc = dvw2_t[:, ti:ti + 1]
                if dx == 1:
                    s3 = tv_b[:].rearrange("p (r w) -> p r w", w=WS)
                    view = s3[:, dy:dy + RS, 2:130]
                else:
                    s3 = tshb[:].rearrange("p (r w) -> p r w", w=WS)
                    view = s3[:, dy:dy + RS, dx:dx + 128]
                dstv = vbv if ti == 0 else prodbv
                nc.vector.tensor_scalar(dstv, view, sc, None, ALU.mult)
                if ti > 0:
                    nc.vector.tensor_tensor(vb[:], vb[:], prodb[:], ALU.add)

        def p2_attn(s):
            o0 = s * SPXO
            tv_a, tv_b = tv_tiles.pop(s)
            tva3 = tv_a[:].rearrange("p (r w) -> p r w", w=WS)
            vb = vb_tiles.pop(s)
            oa = outsb.tile([128, SPXO], F32, tag="oa", name=f"oa_{s}",
                            bufs=1)
            ob = outsb.tile([64, SPXO], F32, tag="ob", name=f"ob_{s}",
                            bufs=1)
            for n0, n in _chunks(SPXO, MMCH):
                r0, nr = n0 // W, n // W
                ops1 = pps.tile([120, MMCH], F32, tag="ops1", bufs=2)
                ops2 = pps.tile([72, MMCH], F32, tag="ops2", bufs=2)
                for ti, (dy, dx) in enumerate(TAPS):
                    nc.tensor.matmul(
                        ops1[:, 0:n], bd1t[:, ti * 120:(ti + 1) * 120],
                        tva3[0:120, r0 + dy:r0 + dy + nr, 1 + dx:129 + dx],
                        start=(ti == 0), stop=(ti == 8))
                nc.tensor.matmul(ops2[:, 0:n], bd2[:], vb[:, n0:n0 + n],
                                 start=True, stop=True)
                ao1 = dwo.tile([120, MMCH], BF16, tag="ao1", bufs=2)
                ao2 = dwo.tile([72, MMCH], BF16, tag="ao2", bufs=2)
                nc.scalar.copy(ao1[:, 0:n], ops1[:, 0:n])
                nc.scalar.copy(ao2[:, 0:n], ops2[:, 0:n])
                ppa = pps.tile([128, MMCH], F32, tag="ppa")
                ppb = pps.tile([64, MMCH], F32, tag="ppb")
                nc.tensor.matmul(ppa[:, 0:n], wp1_t[:, 0:128], ao1[:, 0:n],
                                 start=True, stop=False)
                nc.tensor.matmul(ppa[:, 0:n], wp2_t[:, 0:128], ao2[:, 0:n],
                                 start=False, stop=True)
                nc.tensor.matmul(ppb[:, 0:n], wp1_t[:, 128:192], ao1[:, 0:n],
                                 start=True, stop=False)
                nc.tensor.matmul(ppb[:, 0:n], wp2_t[:, 128:192], ao2[:, 0:n],
                                 start=False, stop=True)
                nc.scalar.copy(oa[:, n0:n0 + n], ppa[:, 0:n])
                nc.vector.tensor_copy(ob[:, n0:n0 + n], ppb[:, 0:n])
            nc.gpsimd.dma_start(io["outp"][0:128, o0:o0 + SPXO], oa[:])
            nc.gpsimd.dma_start(io["outp"][128:192, o0:o0 + SPXO], ob[:])

        # all conv stripes first -> overlap the collective + attn glue
        p2_conv(0)
        p2_conv(1)
        p2_conv(2)
        p2_conv(3)
        p2_vbdw(0)
        p2_vbdw(1)
        qn_red = small.tile([128, 3], F32, tag="qnr")
        nc.vector.tensor_reduce(qn_red[:, 0:1], qn_part[:, 0:NS], AX.X,
                                ALU.add)
        nc.vector.tensor_reduce(qn_red[:, 1:2], g1m[:], AX.X, ALU.add)
        nc.vector.tensor_reduce(qn_red[:, 2:3], g2m[:], AX.X, ALU.add)

        bounce_in = drm.tile([C, 26], F32)
        bounce_out = drm.tile([C, 26], F32)
        # per-head qk gram blocks -> bounce cols 0:24
        for h in range(HEADS):
            r0 = h * CH
            if r0 + CH <= 128:
                nc.sync.dma_start(bounce_in[r0:r0 + CH, 0:CH],
                                  ga_sb[r0:r0 + CH, r0:r0 + CH])
            elif r0 >= 128:
                nc.sync.dma_start(bounce_in[r0:r0 + CH, 0:CH],
                                  gb_sb[r0 - 128:r0 - 128 + CH, r0:r0 + CH])
            else:  # straddles the 128 boundary (head 5)
                nc.sync.dma_start(bounce_in[r0:128, 0:CH],
                                  ga_sb[r0:128, r0:r0 + CH])
                nc.sync.dma_start(bounce_in[128:r0 + CH, 0:CH],
                                  gb_sb[0:r0 + CH - 128, r0:r0 + CH])
        # qn -> col 24  (block0 = q0:128, block1 top = q128:192)
        nc.sync.dma_start(bounce_in[0:128, 24:25], qn_red[:, 0:1])
        nc.sync.dma_start(bounce_in[128:192, 24:25], qn_red[0:64, 1:2])
        # kn -> col 25  (block1 bottom = k0:64, block2 = k64:192)
        nc.sync.dma_start(bounce_in[0:64, 25:26], qn_red[64:128, 1:2])
        nc.sync.dma_start(bounce_in[64:192, 25:26], qn_red[0:128, 2:3])

        nc.gpsimd.collective_compute(
            "AllReduce", ALU.add,
            replica_groups=[[0, 1], [2, 3], [4, 5], [6, 7]],
            ins=[bounce_in[:].opt()], outs=[bounce_out[:].opt()])

        cmp_a = small.tile([128, 26], F32, tag="cmpa")
        cmp_b = small.tile([64, 26], F32, tag="cmpb")
        nc.sync.dma_start(cmp_a[:], bounce_out[0:128, :])
        nc.sync.dma_start(cmp_b[:], bounce_out[128:192, :])

        kn8 = small.tile([HEADS, CH], F32, tag="kn8")
        nc.sync.dma_start(
            kn8[:], bounce_out[:].rearrange("(h c) k -> h c k", c=CH)[:, :, 25])

        # rq = temp/sqrt(qn); rk = 1/sqrt(kn) as [8,24]
        rq_a = small.tile([128, 3], F32, tag="rqa")
        rq_b = small.tile([64, 3], F32, tag="rqb")
        for ti, (cmp, rq, nrow) in enumerate(((cmp_a, rq_a, 128),
                                              (cmp_b, rq_b, 64))):
            nc.scalar.activation(rq[:, 0:1], cmp[:, 24:25], ACTF.Sqrt)
            nc.vector.reciprocal(rq[:, 1:2], rq[:, 0:1])
            nc.vector.tensor_scalar(rq[:, 2:3], rq[:, 1:2],
                                    tmpq_t[0:nrow, ti:ti + 1], None, ALU.mult)
        rk8 = small.tile([HEADS, 2 * CH], F32, tag="rk8")
        nc.scalar.activation(rk8[:, 0:CH], kn8[:], ACTF.Sqrt)
        nc.vector.reciprocal(rk8[:, CH:2 * CH], rk8[:, 0:CH])
        rk8b = small.tile([HEADS, CH], BF16, tag="rk8b")
        nc.vector.tensor_copy(rk8b[:], rk8[:, CH:2 * CH])

        knb_a = small.tile([128, CH], F32, tag="knba")
        knb_b = small.tile([64, CH], F32, tag="knbb")
        knb_ps = pps.tile([128, CH], F32, tag="ppa", name="knb_ps")
        nc.tensor.matmul(knb_ps[:], em_t[:, 0:128], rk8b[:],
                         start=True, stop=True)
        nc.scalar.copy(knb_a[:], knb_ps[:])
        knb_ps2 = pps.tile([128, CH], F32, tag="ppa", name="knb_ps2")
        nc.tensor.matmul(knb_ps2[0:64, :], em_t[:, 128:192], rk8b[:],
                         start=True, stop=True)
        nc.scalar.copy(knb_b[:], knb_ps2[0:64, :])

        # s = A*rq*knb ; softmax over d (free dim)
        attn16 = small.tile([128, CH], BF16, tag="att16a")
        attn16b = small.tile([64, CH], BF16, tag="att16b")
        for cmp, rq, knb, a16, nrow in ((cmp_a, rq_a, knb_a, attn16, 128),
                                        (cmp_b, rq_b, knb_b, attn16b, 64)):
            at = small.tile([128, CH], F32, tag="atf")
            sm = small.tile([128, 4], F32, tag="sm")
            nc.vector.tensor_scalar(at[0:nrow, :], cmp[0:nrow, 0:CH], rq[:, 2:3],
                                    None, ALU.mult)
            nc.vector.tensor_tensor(at[0:nrow, :], at[0:nrow, :], knb[:],
                                    ALU.mult)
            nc.vector.tensor_reduce(sm[0:nrow, 0:1], at[0:nrow, :], AX.X, ALU.max)
            nc.vector.tensor_scalar(at[0:nrow, :], at[0:nrow, :], sm[0:nrow, 0:1],
                                    None, ALU.subtract)
            nc.scalar.activation(at[0:nrow, :], at[0:nrow, :], ACTF.Exp)
            nc.vector.tensor_reduce(sm[0:nrow, 1:2], at[0:nrow, :], AX.X, ALU.add)
            nc.vector.reciprocal(sm[0:nrow, 2:3], sm[0:nrow, 1:2])
            nc.vector.tensor_scalar(a16[0:nrow, :], at[0:nrow, :],
                                    sm[0:nrow, 2:3], None, ALU.mult)

        # block-diag attn^T via DRAM round-trip (transposing strided DMAs)
        attn_d = drm.tile([C, CH], BF16)
        nc.sync.dma_start(attn_d[0:128, :], attn16[:])
        nc.sync.dma_start(attn_d[128:192, :], attn16b[:])
        bd1 = small.tile([120, 120], BF16, tag="bd1")      # heads 0-4 attn^T
        bd2 = small.tile([72, 72], BF16, tag="bd2")        # heads 5-7 attn^T
        nc.vector.memset(bd1[:], 0.0)
        nc.vector.memset(bd2[:], 0.0)
        for h in range(5):
            r0 = h * CH
            nc.sync.dma_start(bd1[r0:r0 + CH, r0:r0 + CH],
                              attn_d[r0:r0 + CH, :].rearrange("c d -> d c"))
        for h in range(5, 8):
            r0 = (h - 5) * CH
            nc.sync.dma_start(bd2[r0:r0 + CH, r0:r0 + CH],
                              attn_d[h * CH:(h + 1) * CH, :].rearrange("c d -> d c"))
        # fold depthwise-v tap weights into attn^T: bd{1,2}_tap = bd * w_v[d,tap]
        bd1t = small.tile([120, 9 * 120], BF16, tag="bd1t")
        bd2t = small.tile([72, 9 * 72], BF16, tag="bd2t")
        for ti in range(9):
            nc.vector.tensor_scalar(bd1t[:, ti * 120:(ti + 1) * 120], bd1[:],
                                    dvw1_t[:, ti:ti + 1], None, ALU.mult)
            nc.vector.tensor_scalar(bd2t[:, ti * 72:(ti + 1) * 72], bd2[:],
                                    dvw2_t[:, ti:ti + 1], None, ALU.mult)

        p2_attn(0)
        p2_vbdw(2)
        p2_attn(1)
        p2_vbdw(3)
        p2_attn(2)
        p2_attn(3)


# ======================================================================
def _prep_inputs(x, y, qkv_w, dw_w, proj_w, temperature):
    wq_t = np.ascontiguousarray(qkv_w[0:C].T).astype(BF16NP)
    wk_t = np.ascontiguousarray(qkv_w[C:2 * C].T).astype(BF16NP)
    wv_t = np.ascontiguousarray(qkv_w[2 * C:3 * C].T).astype(BF16NP)
    wp_t = np.ascontiguousarray(proj_w.T).astype(np.float32)
    wp1 = wp_t[0:120].astype(BF16NP)
    wp2 = wp_t[120:192].astype(BF16NP)
    dw = dw_w.reshape(3 * C, 9).astype(np.float32)
    dw_q, dw_k, dw_v = dw[0:C], dw[C:2 * C], dw[2 * C:3 * C]
    dqk = np.concatenate([dw_q[0:128], dw_q[128:192], dw_k[0:64],
                          dw_k[64:192]], axis=0)
    dqkd = np.zeros((2 * 9 * 128, 128), np.float32)
    for i in range(2):
        for t in range(9):
            blk = dqk[(i + 1) * 128:(i + 2) * 128, t]
            np.fill_diagonal(
                dqkd[(i * 9 + t) * 128:(i * 9 + t + 1) * 128, :], blk)
    tmpq = np.repeat(np.asarray(temperature, np.float32).reshape(HEADS),
                     CH).reshape(C, 1)
    em = np.zeros((HEADS, C), np.float32)
    for hh in range(HEADS):
        em[hh, hh * CH:(hh + 1) * CH] = 1.0

    in_maps = []
    for core in range(8):
        bi, half = core // 2, core % 2
        r0 = half * HOUT - 1
        xsl = np.zeros((C, HIN, W), BF16NP)
        ysl = np.zeros((C, HIN, W), BF16NP)
        lo, hi = max(r0, 0), min(r0 + HIN, 128)
        xsl[:, lo - r0:hi - r0] = x[bi, :, lo:hi].astype(BF16NP)
        ysl[:, lo - r0:hi - r0] = y[bi, :, lo:hi].astype(BF16NP)
        in_maps.append({
            "xs": xsl.reshape(C, PXIN), "ys": ysl.reshape(C, PXIN),
            "wq": wq_t, "wk": wk_t, "wv": wv_t,
            "wp1": wp1, "wp2": wp2,
            "dqk": dqk, "dqkd": dqkd.astype(BF16NP),
            "dvw": dw_v.astype(np.float32),
            "tmpq": tmpq, "em": em.astype(BF16NP),
            "eye": np.eye(128, dtype=np.float32),
        })
    return in_maps


def kernel(x, y, qkv_w, dw_w, proj_w, temperature, _trace=False):
    x = np.asarray(x, np.float32)
    y = np.asarray(y, np.float32)
    if "nc" not in _CACHE:
        _CACHE["nc"] = build_program()
    nc = _CACHE["nc"]
    in_maps = _prep_inputs(x, y, np.asarray(qkv_w, np.float32),
                           np.asarray(dw_w, np.float32),
                           np.asarray(proj_w, np.float32),
                           np.asarray(temperature, np.float32))
    res = bass_utils.run_bass_kernel_spmd(nc, in_maps,
                                          core_ids=list(range(8)),
                                          trace=_trace)
    _CACHE["last_result"] = res
    out = np.empty((4, C, 128, W), np.float32)
    for core in range(8):
        bi, half = core // 2, core % 2
        out[bi, :, half * HOUT:(half + 1) * HOUT] = \
            res.results[core]["outp"].reshape(C, HOUT, W)
    return out



# revision 27
# speedup vs baseline: 1.4089x; 1.2303x over previous
"""Trainium2 Bass kernel for nn_Attention (channel attention, XCA-style).

Sharding: 8 cores = (batch b=core//2) x (image half = core%2, 64 rows + halo).
Cross-core: AllGather of tiny gram stats over core pairs + local add.

Per-core pipeline (single NEFF, SPMD):
  pass1 (per stripe): conv1x1 q,k in fp8 DoubleRow -> padded t bufs (fp8);
    conv1x1 v in bf16 -> padded tv bufs; transposed depthwise on PE
    (fp8 DoubleRow tap pairs) -> px-major stacks; gram + self-gram matmuls
    (fp8 DoubleRow chunk pairs) accumulated in PSUM.
  AllGather([192,26]) over pairs + local add; rsqrt-scale + softmax on-chip
  tail: dw-v block-b on DVE; attn via 9-tap folded matmuls (block-a) +
    plain matmul (block-b) -> proj -> batched DMA out.
"""

import sys
import numpy as np

sys.path.insert(0, "/opt/trn_rl_repo")

import contextlib  # noqa: E402

import ml_dtypes  # noqa: E402

from concourse import bass, bacc, tile, mybir  # noqa: E402
from concourse import bass_utils  # noqa: E402

F32 = mybir.dt.float32
BF16 = mybir.dt.bfloat16
F8 = mybir.dt.float8e4
ALU = mybir.AluOpType
ACTF = mybir.ActivationFunctionType
AX = mybir.AxisListType
DR = mybir.MatmulPerfMode.DoubleRow
BF16NP = ml_dtypes.bfloat16
F8NP = ml_dtypes.float8_e4m3

C = 192
HEADS = 8
CH = 24
W = 128
HOUT = 64
HIN = HOUT + 2
WS = 144                  # padded row stride in t buffers (16-aligned for
                          # DoubleRow weight plane strides)
PXIN = HIN * W            # 8448
PXOUT = HOUT * W          # 8192

RS = 16                   # stripe out-rows
NS = HOUT // RS
RIN = RS + 2
SPXI = RIN * W            # 2304
SPXO = RS * W             # 2048
LT = RIN * WS             # t buffer length (2592)
LTX = LT
MMCH = 512

TAPS = [(dy, dx) for dy in (0, 1, 2) for dx in (0, 1, 2)]
# DoubleRow tap pairs must have 16-aligned flat-offset delta: vertical
# pairs (same dx) have delta WS=144. Taps 6,7,8 run as single fp8 MMs.
TAP_PAIRS = [(0, 3), (1, 4), (2, 5)]
TAP_SINGLES = [6, 7, 8]
DQW = 9 * 128

_CACHE = {}


def _chunks(total, step):
    out, s = [], 0
    while s < total:
        out.append((s, min(step, total - s)))
        s += step
    return out


def _tap_off(cix, ti):
    dy, dx = TAPS[ti]
    return (cix + dy) * WS + 1 + dx


def build_program():
    nc = bacc.Bacc("TRN2", target_bir_lowering=False, debug=False,
                   enable_asserts=False, num_devices=8)
    io = {}
    io["xs"] = nc.dram_tensor("xs", [C, PXIN], BF16, kind="ExternalInput").ap()
    io["xs8"] = nc.dram_tensor("xs8", [128, 2 * PXIN], F8,
                               kind="ExternalInput").ap()
    io["ys8"] = nc.dram_tensor("ys8", [128, 2 * PXIN], F8,
                               kind="ExternalInput").ap()
    io["wq8"] = nc.dram_tensor("wq8", [128, 2 * C], F8,
                               kind="ExternalInput").ap()
    io["wk8"] = nc.dram_tensor("wk8", [128, 2 * C], F8,
                               kind="ExternalInput").ap()
    io["wk16"] = nc.dram_tensor("wk16", [C, 64], BF16,
                                kind="ExternalInput").ap()
    io["wv"] = nc.dram_tensor("wv", [C, C], BF16, kind="ExternalInput").ap()
    io["wp1"] = nc.dram_tensor("wp1", [120, C], BF16, kind="ExternalInput").ap()
    io["wp2"] = nc.dram_tensor("wp2", [72, C], BF16, kind="ExternalInput").ap()
    io["dqkd"] = nc.dram_tensor("dqkd", [3 * 128, 9 * 128], F8,
                                kind="ExternalInput").ap()
    io["dvw"] = nc.dram_tensor("dvw", [C, 9], F32, kind="ExternalInput").ap()
    io["tmpq"] = nc.dram_tensor("tmpq", [C, 1], F32, kind="ExternalInput").ap()
    io["em"] = nc.dram_tensor("em", [HEADS, C], BF16, kind="ExternalInput").ap()
    io["eye"] = nc.dram_tensor("eye", [128, 128], F32,
                               kind="ExternalInput").ap()
    io["outp"] = nc.dram_tensor("outp", [C, PXOUT], F32,
                                kind="ExternalOutput").ap()

    with tile.TileContext(nc) as tc, contextlib.ExitStack() as es:
        _emit(nc, tc, io, es)
    nc.compile()
    return nc


def _emit(nc, tc, io, es):
    # ---------------- persistent weights ------------------------------
    wpool = es.enter_context(tc.tile_pool(name="w", bufs=1))
    wq8_t = wpool.tile([128, 2 * C], F8, tag="wq8")
    wk8_t = wpool.tile([128, 2 * C], F8, tag="wk8")
    nc.sync.dma_start(wq8_t[:], io["wq8"][:])
    nc.sync.dma_start(wk8_t[:], io["wk8"][:])
    # bf16 wk cols 0:64 (DoubleRow can't target out partitions 64:128)
    wk16a = wpool.tile([128, 64], BF16, tag="wk16a")
    wk16b = wpool.tile([64, 64], BF16, tag="wk16b")
    nc.sync.dma_start(wk16a[:], io["wk16"][0:128, :])
    nc.sync.dma_start(wk16b[:], io["wk16"][128:192, :])
    wv_a = wpool.tile([128, C], BF16, tag="wva")
    wv_b = wpool.tile([64, C], BF16, tag="wvb")
    nc.sync.dma_start(wv_a[:], io["wv"][0:128, :])
    nc.sync.dma_start(wv_b[:], io["wv"][128:192, :])
    wp1_t = wpool.tile([120, C], BF16, tag="wp1")
    wp2_t = wpool.tile([72, C], BF16, tag="wp2")
    nc.sync.dma_start(wp1_t[:], io["wp1"][:])
    nc.sync.dma_start(wp2_t[:], io["wp2"][:])
    dqkd_t = [wpool.tile([128, DQW], F8, tag=f"dqkd{i}",
                         name=f"dqkd{i}") for i in range(3)]
    for i in range(3):
        nc.sync.dma_start(dqkd_t[i][:, 0:9 * 128],
                          io["dqkd"][i * 128:(i + 1) * 128, :])
    dvw1_t = wpool.tile([120, 9], F32, tag="dvw1")
    dvw2_t = wpool.tile([72, 9], F32, tag="dvw2")
    nc.sync.dma_start(dvw1_t[:], io["dvw"][0:120, :])
    nc.sync.dma_start(dvw2_t[:], io["dvw"][120:192, :])
    tmpq_t = wpool.tile([128, 2], F32, tag="tmpq")
    nc.sync.dma_start(tmpq_t[:, 0:1], io["tmpq"][0:128, :])
    nc.sync.dma_start(tmpq_t[0:64, 1:2], io["tmpq"][128:192, :])
    em_t = wpool.tile([HEADS, C], BF16, tag="em")
    nc.sync.dma_start(em_t[:], io["em"][:])
    eye_t = wpool.tile([128, 128], F32, tag="eye")
    nc.sync.dma_start(eye_t[:], io["eye"][:])

    # ---------------- pools -------------------------------------------
    inb = es.enter_context(tc.tile_pool(name="inb", bufs=1))
    tbuf = es.enter_context(tc.tile_pool(name="tbuf", bufs=1))
    dwo = es.enter_context(tc.tile_pool(name="dwo", bufs=1))
    stck = es.enter_context(tc.tile_pool(name="stck", bufs=1))
    small = es.enter_context(tc.tile_pool(name="small", bufs=1))
    outsb = es.enter_context(tc.tile_pool(name="outsb", bufs=2))
    drm = es.enter_context(tc.tile_pool(name="drm", bufs=1, space="DRAM"))

    # pre-zero only the pad columns of each padded buffer slot (cols 0:2
    # and 130:132 of every row; data region is overwritten each stripe)
    for b in range(3):
        for _sl in range(2):
            tz = tbuf.tile([128, LTX], F8, tag=f"t{b}", name=f"tz{b}_{_sl}",
                           bufs=2)
            tzv = tz[:, 0:LT].rearrange("p (r w) -> p r w", w=WS)
            nc.vector.memset(tzv[:, :, 0:2], 0.0)
            nc.vector.memset(tzv[:, :, 130:132], 0.0)
    tv_tiles = {}
    for _s in range(NS):
        _tva = tbuf.tile([120, LT], BF16, tag="tv_{}".format(_s),
                         name=f"tvz_{_s}", bufs=1)
        _tvb = tbuf.tile([72, LT], BF16, tag="tvb_{}".format(_s),
                         name=f"tvbz_{_s}", bufs=1)
        for _t in (_tva, _tvb):
            _tvv = _t[:].rearrange("p (r w) -> p r w", w=WS)
            nc.vector.memset(_tvv[:, :, 0:2], 0.0)
            nc.vector.memset(_tvv[:, :, 130:132], 0.0)

    xs8v = io["xs8"].rearrange("p (two n) -> p two n", two=2)
    ys8v = io["ys8"].rearrange("p (two n) -> p two n", two=2)
    wq8v = wq8_t[:].rearrange("p (two c) -> p two c", two=2)
    wk8v = wk8_t[:].rearrange("p (two c) -> p two c", two=2)

    def pair_view(tblk, cix, ti, tj):
        o0, o1 = _tap_off(cix, ti), _tap_off(cix, tj)
        d = o1 - o0
        v = tblk[:, o0:o0 + 2 * d].rearrange("p (two d) -> p two d", d=d)
        return v[:, :, 0:128]

    def diag_pair(b, ti, tj):
        o0, d = ti * 128, (tj - ti) * 128
        v = dqkd_t[b][:, o0:o0 + 2 * d].rearrange("p (two d) -> p two d",
                                                  d=d)
        return v[:, :, 0:128]

    # ================= PASS 1 =========================================
    # gram PSUM layout:
    #   gA = [ga: q0:128 x k0:192 (192) | selfg0: q0:128 self (128)]
    #   gB = [b1 self (128) | b1 x b2 (128) | selfg2: k64:192 self (128)]
    with tc.tile_pool(name="cps", bufs=1, space="PSUM") as cpsum, \
         tc.tile_pool(name="gps", bufs=1, space="PSUM") as gpsum:
        gA_ps = gpsum.tile([128, 320], F32, tag="gA")
        gB_ps = gpsum.tile([128, 384], F32, tag="gB")
        for s in range(NS):
            i0 = s * RS * W
            t_blk = [tbuf.tile([128, LTX], F8, tag=f"t{b}",
                               name=f"t{b}_{s}", bufs=2) for b in range(3)]

            xq8 = inb.tile([128, 2 * SPXI], F8, tag="xq8", bufs=2)
            yq8 = inb.tile([128, 2 * SPXI], F8, tag="yq8", bufs=2)
            xa16 = inb.tile([128, SPXI], BF16, tag="xa16", bufs=2)
            xb16 = inb.tile([64, SPXI], BF16, tag="xb16", bufs=2)
            nc.gpsimd.dma_start(
                xq8[:].rearrange("p (two n) -> p two n", two=2),
                xs8v[:, :, i0:i0 + SPXI])
            nc.gpsimd.dma_start(
                yq8[:].rearrange("p (two n) -> p two n", two=2),
                ys8v[:, :, i0:i0 + SPXI])
            nc.gpsimd.dma_start(xa16[:], io["xs"][0:128, i0:i0 + SPXI])
            nc.gpsimd.dma_start(xb16[:], io["xs"][128:192, i0:i0 + SPXI])
            xqv = xq8[:].rearrange("p (two n) -> p two n", two=2)
            yqv = yq8[:].rearrange("p (two n) -> p two n", two=2)

            # conv q,k (fp8 DoubleRow over the 192-channel contraction)
            # -> t blocks [q 0:128] | [q 128:192 ; k 0:64] | [k 64:192]
            for n0, n in _chunks(SPXI, MMCH):
                r0, nr = n0 // W, n // W
                ps0 = cpsum.tile([128, MMCH], F32, tag="cps0", bufs=2)
                ps1 = cpsum.tile([128, MMCH], F32, tag="cps1", bufs=2)
                ps2 = cpsum.tile([128, MMCH], F32, tag="cps1", bufs=2,
                                 name=f"ps2_{s}_{n0}")
                nc.tensor.matmul(ps0[:, 0:n], wq8v[:, :, 0:128],
                                 yqv[:, :, n0:n0 + n], start=True, stop=True,
                                 perf_mode=DR)
                nc.tensor.matmul(ps1[0:64, 0:n], wq8v[:, :, 128:192],
                                 yqv[:, :, n0:n0 + n], start=True, stop=True,
                                 perf_mode=DR)
                nc.tensor.matmul(ps1[64:128, 0:n], wk16a[:],
                                 xa16[:, n0:n0 + n], start=True, stop=False,
                                 tile_position=(0, 64))
                nc.tensor.matmul(ps1[64:128, 0:n], wk16b[:],
                                 xb16[:, n0:n0 + n], start=False, stop=True,
                                 tile_position=(0, 64))
                nc.tensor.matmul(ps2[:, 0:n], wk8v[:, :, 64:192],
                                 xqv[:, :, n0:n0 + n], start=True, stop=True,
                                 perf_mode=DR)
                for b, ps, eng in ((0, ps0, nc.scalar), (1, ps1, nc.vector),
                                   (2, ps2, nc.scalar)):
                    dst = t_blk[b][:, 0:LT].rearrange("p (r w) -> p r w",
                                                      w=WS)
                    eng_copy = (eng.copy if eng is nc.scalar
                                else eng.tensor_copy)
                    eng_copy(dst[:, r0:r0 + nr, 2:130],
                             ps[:, 0:n].rearrange("p (r w) -> p r w", w=W))

            # conv v (bf16) -> padded tv buffers for the attn tail
            tv_a = tbuf.tile([120, LT], BF16, tag="tv_{}".format(s),
                             name=f"tva_{s}", bufs=1)
            tv_b = tbuf.tile([72, LT], BF16, tag="tvb_{}".format(s),
                             name=f"tvb_{s}", bufs=1)
            tv_tiles[s] = (tv_a, tv_b)
            for n0, n in _chunks(SPXI, MMCH):
                r0, nr = n0 // W, n // W
                cv0 = cpsum.tile([128, MMCH], F32, tag="cps0", bufs=2,
                                 name=f"cv0_{s}_{n0}")
                cv1 = cpsum.tile([128, MMCH], F32, tag="cps1", bufs=2,
                                 name=f"cv1_{s}_{n0}")
                nc.tensor.matmul(cv0[0:120, 0:n], wv_a[:, 0:120],
                                 xa16[:, n0:n0 + n], start=True, stop=False)
                nc.tensor.matmul(cv0[0:120, 0:n], wv_b[:, 0:120],
                                 xb16[:, n0:n0 + n], start=False, stop=True)
                nc.tensor.matmul(cv1[0:72, 0:n], wv_a[:, 120:192],
                                 xa16[:, n0:n0 + n], start=True, stop=False)
                nc.tensor.matmul(cv1[0:72, 0:n], wv_b[:, 120:192],
                                 xb16[:, n0:n0 + n], start=False, stop=True)
                dsta = tv_a[:].rearrange("p (r w) -> p r w", w=WS)
                dstb = tv_b[:].rearrange("p (r w) -> p r w", w=WS)
                nc.scalar.copy(dsta[0:120, r0:r0 + nr, 2:130],
                               cv0[0:120, 0:n].rearrange("p (r w) -> p r w",
                                                         w=W))
                nc.vector.tensor_copy(
                    dstb[0:72, r0:r0 + nr, 2:130],
                    cv1[0:72, 0:n].rearrange("p (r w) -> p r w", w=W))

            # transposed depthwise on PE (fp8 DR tap pairs) -> px-major
            # stacks, then gram accumulation (fp8 DR chunk pairs)
            nchunk = SPXO // 128
            for g in range(nchunk // 4):
                stk = [stck.tile([128, MMCH], F8, tag=f"stk{b}",
                                 name=f"stk{b}_{s}_{g}", bufs=2)
                       for b in range(3)]
                for b in range(3):
                    tp = cpsum.tile([128, MMCH], F32, tag="tps",
                                    name=f"tp{b}_{s}_{g}", bufs=2)
                    tflat = t_blk[b][:]
                    for ci in range(4):
                        cix = g * 4 + ci
                        q0 = ci * 128
                        for pi, (ti, tj) in enumerate(TAP_PAIRS):
                            nc.tensor.matmul(
                                tp[:, q0:q0 + 128],
                                pair_view(tflat, cix, ti, tj),
                                diag_pair(b, ti, tj),
                                start=(pi == 0), stop=False, perf_mode=DR)
                        for si, ti in enumerate(TAP_SINGLES):
                            o8 = _tap_off(cix, ti)
                            nc.tensor.matmul(
                                tp[:, q0:q0 + 128], tflat[:, o8:o8 + 128],
                                dqkd_t[b][:, ti * 128:(ti + 1) * 128],
                                start=False,
                                stop=(si == len(TAP_SINGLES) - 1))
                    if b == 1:
                        nc.scalar.copy(stk[b][:], tp[:])
                    else:
                        nc.vector.tensor_copy(stk[b][:], tp[:])
                for p in range(2):
                    cix = g * 4 + 2 * p
                    first = (s == 0 and cix == 0)
                    last = (s == NS - 1 and cix == nchunk - 2)
                    c0 = (2 * p) * 128
                    sp = [stk[b][:, c0:c0 + 256].rearrange(
                        "p (two c) -> p two c", two=2) for b in range(3)]
                    nc.tensor.matmul(gA_ps[:, 0:64], sp[0],
                                     sp[1][:, :, 64:128],
                                     start=first, stop=last, perf_mode=DR)
                    nc.tensor.matmul(gA_ps[:, 64:192], sp[0], sp[2],
                                     start=first, stop=last, perf_mode=DR)
                    nc.tensor.matmul(gA_ps[:, 192:320], sp[0], sp[0],
                                     start=first, stop=last, perf_mode=DR)
                    nc.tensor.matmul(gB_ps[:, 0:128], sp[1], sp[1],
                                     start=first, stop=last, perf_mode=DR)
                    nc.tensor.matmul(gB_ps[:, 128:256], sp[1], sp[2],
                                     start=first, stop=last, perf_mode=DR)
                    nc.tensor.matmul(gB_ps[:, 256:384], sp[2], sp[2],
                                     start=first, stop=last, perf_mode=DR)

        # ---- gram -> SBUF, norms, bounce assembly (DMA, crosses parts)
        ga_sb = small.tile([128, C], F32, tag="gasb")
        g1_sb = small.tile([128, 256], F32, tag="g1sb")
        nc.scalar.copy(ga_sb[:], gA_ps[:, 0:192])
        nc.scalar.copy(g1_sb[:], gB_ps[:, 0:256])
        gb_sb = small.tile([64, C], F32, tag="gbsb")
        nc.vector.tensor_copy(gb_sb[:], g1_sb[0:64, 64:256])

        # sq-norms = diag of the self-gram blocks via eye-mask + row sum
        g0m = small.tile([128, 128], F32, tag="g0m")
        nc.vector.tensor_tensor(g0m[:], gA_ps[:, 192:320], eye_t[:],
                                ALU.mult)
        g1m = small.tile([128, 128], F32, tag="g1m")
        nc.vector.tensor_tensor(g1m[:], g1_sb[:, 0:128], eye_t[:], ALU.mult)
        g2m = small.tile([128, 128], F32, tag="g2m")
        nc.vector.tensor_tensor(g2m[:], gB_ps[:, 256:384], eye_t[:],
                                ALU.mult)

    # ================= PASS 2 =========================================
    with tc.tile_pool(name="p2ps", bufs=1, space="PSUM") as pps:
        vb_tiles = {}

        def p2_vbdw(s):
            # v[120:192] depthwise on DVE (channel-major), PE does attn only
            tv_b = tv_tiles[s][1]
            tshb = tbuf.tile([72, LT], BF16, tag="tshb", name=f"tshb_{s}")
            nc.vector.tensor_copy(tshb[:, 0:LT - 2], tv_b[:, 1:LT - 1])
            vb = dwo.tile([72, SPXO], BF16, tag=f"vbdw{s % 2}",
                          name=f"vbdw_{s}", bufs=1)
            vb_tiles[s] = vb
            prodb = dwo.tile([72, SPXO], BF16, tag="prodb", name=f"prodb_{s}")
            vbv = vb[:].rearrange("p (r w) -> p r w", w=W)
            prodbv = prodb[:].rearrange("p (r w) -> p r w", w=W)
            for ti, (dy, dx) in enumerate(TAPS):
                sc = dvw2_t[:, ti:ti + 1]
                if dx == 1:
                    s3 = tv_b[:].rearrange("p (r w) -> p r w", w=WS)
                    view = s3[:, dy:dy + RS, 2:130]
                else:
                    s3 = tshb[:].rearrange("p (r w) -> p r w", w=WS)
                    view = s3[:, dy:dy + RS, dx:dx + 128]
                dstv = vbv if ti == 0 else prodbv
                nc.vector.tensor_scalar(dstv, view, sc, None, ALU.mult)
                if ti > 0:
                    nc.vector.tensor_tensor(vb[:], vb[:], prodb[:], ALU.add)

        def p2_attn(s):
            o0 = s * SPXO
            tv_a, tv_b = tv_tiles.pop(s)
            tva3 = tv_a[:].rearrange("p (r w) -> p r w", w=WS)
            vb = vb_tiles.pop(s)
            oa = outsb.tile([128, SPXO], F32, tag="oa", name=f"oa_{s}",
                            bufs=1)
            ob = outsb.tile([64, SPXO], F32, tag="ob", name=f"ob_{s}",
                            bufs=1)
            for n0, n in _chunks(SPXO, MMCH):
                r0, nr = n0 // W, n // W
                ops1 = pps.tile([120, MMCH], F32, tag="ops1", bufs=2)
                ops2 = pps.tile([72, MMCH], F32, tag="ops2", bufs=2)
                for ti, (dy, dx) in enumerate(TAPS):
                    nc.tensor.matmul(
                        ops1[:, 0:n], bd1t[:, ti * 120:(ti + 1) * 120],
                        tva3[0:120, r0 + dy:r0 + dy + nr, 1 + dx:129 + dx],
                        start=(ti == 0), stop=(ti == 8))
                nc.tensor.matmul(ops2[:, 0:n], bd2[:], vb[:, n0:n0 + n],
                                 start=True, stop=True)
                ao1 = dwo.tile([120, MMCH], BF16, tag="ao1", bufs=2)
                ao2 = dwo.tile([72, MMCH], BF16, tag="ao2", bufs=2)
                nc.scalar.copy(ao1[:, 0:n], ops1[:, 0:n])
                nc.vector.tensor_copy(ao2[:, 0:n], ops2[:, 0:n])
                ppa = pps.tile([128, MMCH], F32, tag="ppa")
                ppb = pps.tile([64, MMCH], F32, tag="ppb")
                nc.tensor.matmul(ppa[:, 0:n], wp1_t[:, 0:128], ao1[:, 0:n],
                                 start=True, stop=False)
                nc.tensor.matmul(ppa[:, 0:n], wp2_t[:, 0:128], ao2[:, 0:n],
                                 start=False, stop=True)
                nc.tensor.matmul(ppb[:, 0:n], wp1_t[:, 128:192], ao1[:, 0:n],
                                 start=True, stop=False)
                nc.tensor.matmul(ppb[:, 0:n], wp2_t[:, 128:192], ao2[:, 0:n],
                                 start=False, stop=True)
                nc.scalar.copy(oa[:, n0:n0 + n], ppa[:, 0:n])
                nc.vector.tensor_copy(ob[:, n0:n0 + n], ppb[:, 0:n])
            nc.gpsimd.dma_start(io["outp"][0:128, o0:o0 + SPXO], oa[:])
            nc.gpsimd.dma_start(io["outp"][128:192, o0:o0 + SPXO], ob[:])

        p2_vbdw(0)
        p2_vbdw(1)
        qn_red = small.tile([128, 3], F32, tag="qnr")
        nc.vector.tensor_reduce(qn_red[:, 0:1], g0m[:], AX.X, ALU.add)
        nc.vector.tensor_reduce(qn_red[:, 1:2], g1m[:], AX.X, ALU.add)
        nc.vector.tensor_reduce(qn_red[:, 2:3], g2m[:], AX.X, ALU.add)

        bounce_in = drm.tile([C, 26], F32)
        bounce_out = drm.tile([2 * C, 26], F32)
        # per-head qk gram blocks -> bounce cols 0:24
        for h in range(HEADS):
            r0 = h * CH
            if r0 + CH <= 128:
                nc.sync.dma_start(bounce_in[r0:r0 + CH, 0:CH],
                                  ga_sb[r0:r0 + CH, r0:r0 + CH])
            elif r0 >= 128:
                nc.sync.dma_start(bounce_in[r0:r0 + CH, 0:CH],
                                  gb_sb[r0 - 128:r0 - 128 + CH, r0:r0 + CH])
            else:  # straddles the 128 boundary (head 5)
                nc.sync.dma_start(bounce_in[r0:128, 0:CH],
                                  ga_sb[r0:128, r0:r0 + CH])
                nc.sync.dma_start(bounce_in[128:r0 + CH, 0:CH],
                                  gb_sb[0:r0 + CH - 128, r0:r0 + CH])
        # qn -> col 24  (q0:128 from g0m, q128:192 from g1m top)
        nc.sync.dma_start(bounce_in[0:128, 24:25], qn_red[:, 0:1])
        nc.sync.dma_start(bounce_in[128:192, 24:25], qn_red[0:64, 1:2])
        # kn -> col 25  (k0:64 from g1m bottom, k64:192 from g2m)
        nc.sync.dma_start(bounce_in[0:64, 25:26], qn_red[64:128, 1:2])
        nc.sync.dma_start(bounce_in[64:192, 25:26], qn_red[0:128, 2:3])

        nc.gpsimd.collective_compute(
            "AllGather", ALU.bypass,
            replica_groups=[[0, 1], [2, 3], [4, 5], [6, 7]],
            ins=[bounce_in[:].opt()], outs=[bounce_out[:].opt()])

        # local add of the two gathered halves
        cmp_a = small.tile([128, 26], F32, tag="cmpa")
        cmp_b = small.tile([64, 26], F32, tag="cmpb")
        cmp_a2 = small.tile([128, 26], F32, tag="cmpa2")
        cmp_b2 = small.tile([64, 26], F32, tag="cmpb2")
        nc.sync.dma_start(cmp_a[:], bounce_out[0:128, :])
        nc.sync.dma_start(cmp_b[:], bounce_out[128:192, :])
        nc.sync.dma_start(cmp_a2[:], bounce_out[192:320, :])
        nc.sync.dma_start(cmp_b2[:], bounce_out[320:384, :])
        nc.vector.tensor_tensor(cmp_a[:], cmp_a[:], cmp_a2[:], ALU.add)
        nc.vector.tensor_tensor(cmp_b[:], cmp_b[:], cmp_b2[:], ALU.add)

        kn8 = small.tile([HEADS, CH], F32, tag="kn8")
        kn8b = small.tile([HEADS, CH], F32, tag="kn8x")
        nc.sync.dma_start(
            kn8[:],
            bounce_out[0:C, :].rearrange("(h c) k -> h c k", c=CH)[:, :, 25])
        nc.sync.dma_start(
            kn8b[:],
            bounce_out[C:2 * C, :].rearrange("(h c) k -> h c k",
                                             c=CH)[:, :, 25])
        nc.vector.tensor_tensor(kn8[:], kn8[:], kn8b[:], ALU.add)

        # rq = temp/sqrt(qn); rk = 1/sqrt(kn) as [8,24]
        rq_a = small.tile([128, 3], F32, tag="rqa")
        rq_b = small.tile([64, 3], F32, tag="rqb")
        for ti, (cmp, rq, nrow) in enumerate(((cmp_a, rq_a, 128),
                                              (cmp_b, rq_b, 64))):
            nc.scalar.activation(rq[:, 0:1], cmp[:, 24:25], ACTF.Sqrt)
            nc.vector.reciprocal(rq[:, 1:2], rq[:, 0:1])
            nc.vector.tensor_scalar(rq[:, 2:3], rq[:, 1:2],
                                    tmpq_t[0:nrow, ti:ti + 1], None, ALU.mult)
        rk8 = small.tile([HEADS, 2 * CH], F32, tag="rk8")
        nc.scalar.activation(rk8[:, 0:CH], kn8[:], ACTF.Sqrt)
        nc.vector.reciprocal(rk8[:, CH:2 * CH], rk8[:, 0:CH])
        rk8b = small.tile([HEADS, CH], BF16, tag="rk8b")
        nc.vector.tensor_copy(rk8b[:], rk8[:, CH:2 * CH])

        knb_a = small.tile([128, CH], F32, tag="knba")
        knb_b = small.tile([64, CH], F32, tag="knbb")
        knb_ps = pps.tile([128, CH], F32, tag="ppa", name="knb_ps")
        nc.tensor.matmul(knb_ps[:], em_t[:, 0:128], rk8b[:],
                         start=True, stop=True)
        nc.scalar.copy(knb_a[:], knb_ps[:])
        knb_ps2 = pps.tile([128, CH], F32, tag="ppa", name="knb_ps2")
        nc.tensor.matmul(knb_ps2[0:64, :], em_t[:, 128:192], rk8b[:],
                         start=True, stop=True)
        nc.scalar.copy(knb_b[:], knb_ps2[0:64, :])

        # s = A*rq*knb ; softmax over d (free dim)
        attn16 = small.tile([128, CH], BF16, tag="att16a")
        attn16b = small.tile([64, CH], BF16, tag="att16b")
        for cmp, rq, knb, a16, nrow in ((cmp_a, rq_a, knb_a, attn16, 128),
                                        (cmp_b, rq_b, knb_b, attn16b, 64)):
            at = small.tile([128, CH], F32, tag="atf")
            sm = small.tile([128, 4], F32, tag="sm")
            nc.vector.tensor_scalar(at[0:nrow, :], cmp[0:nrow, 0:CH], rq[:, 2:3],
                                    None, ALU.mult)
            nc.vector.tensor_tensor(at[0:nrow, :], at[0:nrow, :], knb[:],
                                    ALU.mult)
            nc.vector.tensor_reduce(sm[0:nrow, 0:1], at[0:nrow, :], AX.X, ALU.max)
            nc.vector.tensor_scalar(at[0:nrow, :], at[0:nrow, :], sm[0:nrow, 0:1],
                                    None, ALU.subtract)
            nc.scalar.activation(at[0:nrow, :], at[0:nrow, :], ACTF.Exp)
            nc.vector.tensor_reduce(sm[0:nrow, 1:2], at[0:nrow, :], AX.X, ALU.add)
            nc.vector.reciprocal(sm[0:nrow, 2:3], sm[0:nrow, 1:2])
            nc.vector.tensor_scalar(a16[0:nrow, :], at[0:nrow, :],
                                    sm[0:nrow, 2:3], None, ALU.mult)

        # block-diag attn^T via DRAM round-trip (transposing strided DMAs)
        attn_d = drm.tile([C, CH], BF16)
        nc.sync.dma_start(attn_d[0:128, :], attn16[:])
        nc.sync.dma_start(attn_d[128:192, :], attn16b[:])
        bd1 = small.tile([120, 120], BF16, tag="bd1")      # heads 0-4 attn^T
        bd2 = small.tile([72, 72], BF16, tag="bd2")        # heads 5-7 attn^T
        nc.vector.memset(bd1[:], 0.0)
        nc.vector.memset(bd2[:], 0.0)
        for h in range(5):
            r0 = h * CH
            nc.sync.dma_start(bd1[r0:r0 + CH, r0:r0 + CH],
                              attn_d[r0:r0 + CH, :].rearrange("c d -> d c"))
        for h in range(5, 8):
            r0 = (h - 5) * CH
            nc.sync.dma_start(bd2[r0:r0 + CH, r0:r0 + CH],
                              attn_d[h * CH:(h + 1) * CH, :].rearrange("c d -> d c"))
        # fold depthwise-v tap weights into attn^T: bd1_tap = bd1 * w_v[d,tap]
        bd1t = small.tile([120, 9 * 120], BF16, tag="bd1t")
        for ti in range(9):
            nc.vector.tensor_scalar(bd1t[:, ti * 120:(ti + 1) * 120], bd1[:],
                                    dvw1_t[:, ti:ti + 1], None, ALU.mult)

        p2_attn(0)
        p2_vbdw(2)
        p2_attn(1)
        p2_vbdw(3)
        p2_attn(2)
        p2_attn(3)


# ======================================================================
def _prep_inputs(x, y, qkv_w, dw_w, proj_w, temperature):
    wq_t = np.ascontiguousarray(qkv_w[0:C].T)          # [in, out]
    wk_t = np.ascontiguousarray(qkv_w[C:2 * C].T)
    wv_t = np.ascontiguousarray(qkv_w[2 * C:3 * C].T).astype(BF16NP)
    wp_t = np.ascontiguousarray(proj_w.T).astype(np.float32)
    wp1 = wp_t[0:120].astype(BF16NP)
    wp2 = wp_t[120:192].astype(BF16NP)

    def planes2(w):
        out = np.zeros((128, 2, C), np.float32)
        out[:, 0, :] = w[0:128]
        out[0:64, 1, :] = w[128:192]
        return out.reshape(128, 2 * C).astype(F8NP)

    wq8 = planes2(wq_t)
    wk8 = planes2(wk_t)
    wk16 = np.ascontiguousarray(wk_t[:, 0:64]).astype(BF16NP)

    dw = dw_w.reshape(3 * C, 9).astype(np.float32)
    dw_q, dw_k, dw_v = dw[0:C], dw[C:2 * C], dw[2 * C:3 * C]
    dqk = np.concatenate([dw_q[0:128], dw_q[128:192], dw_k[0:64],
                          dw_k[64:192]], axis=0)
    dqkd = np.zeros((3 * 128, 9 * 128), np.float32)
    for i in range(3):
        for t in range(9):
            blk = dqk[i * 128:(i + 1) * 128, t]
            np.fill_diagonal(
                dqkd[i * 128:(i + 1) * 128, t * 128:(t + 1) * 128], blk)
    tmpq = np.repeat(np.asarray(temperature, np.float32).reshape(HEADS),
                     CH).reshape(C, 1)
    em = np.zeros((HEADS, C), np.float32)
    for hh in range(HEADS):
        em[hh, hh * CH:(hh + 1) * CH] = 1.0

    in_maps = []
    for core in range(8):
        bi, half = core // 2, core % 2
        r0 = half * HOUT - 1
        xsl = np.zeros((C, HIN, W), np.float32)
        ysl = np.zeros((C, HIN, W), np.float32)
        lo, hi = max(r0, 0), min(r0 + HIN, 128)
        xsl[:, lo - r0:hi - r0] = x[bi, :, lo:hi]
        ysl[:, lo - r0:hi - r0] = y[bi, :, lo:hi]
        xsl = xsl.reshape(C, PXIN)
        ysl = ysl.reshape(C, PXIN)

        def planes_px(t):
            out = np.zeros((128, 2, PXIN), np.float32)
            out[:, 0, :] = t[0:128]
            out[0:64, 1, :] = t[128:192]
            return out.reshape(128, 2 * PXIN).astype(F8NP)

        in_maps.append({
            "xs": xsl.astype(BF16NP),
            "xs8": planes_px(xsl), "ys8": planes_px(ysl),
            "wq8": wq8, "wk8": wk8, "wk16": wk16, "wv": wv_t,
            "wp1": wp1, "wp2": wp2,
            "dqkd": dqkd.astype(F8NP),
            "dvw": dw_v.astype(np.float32),
            "tmpq": tmpq, "em": em.astype(BF16NP),
            "eye": np.eye(128, dtype=np.float32),
        })
    return in_maps


def kernel(x, y, qkv_w, dw_w, proj_w, temperature, _trace=False):
    x = np.asarray(x, np.float32)
    y = np.asarray(y, np.float32)
    if "nc" not in _CACHE:
        _CACHE["nc"] = build_program()
    nc = _CACHE["nc"]
    in_maps = _prep_inputs(x, y, np.asarray(qkv_w, np.float32),
                           np.asarray(dw_w, np.float32),
                           np.asarray(proj_w, np.float32),
                           np.asarray(temperature, np.float32))
    res = bass_utils.run_bass_kernel_spmd(nc, in_maps,
                                          core_ids=list(range(8)),
                                          trace=_trace)
    _CACHE["last_result"] = res
    out = np.empty((4, C, 128, W), np.float32)
    for core in range(8):
        bi, half = core // 2, core % 2
        out[bi, :, half * HOUT:(half + 1) * HOUT] = \
            res.results[core]["outp"].reshape(C, HOUT, W)
    return out


# revision 40
# speedup vs baseline: 1.4334x; 1.0174x over previous
"""Trainium2 Bass kernel for nn_Attention (channel attention, XCA-style).

Sharding: 8 cores = (batch b=core//2) x (image half = core%2, 64 rows + halo).
Cross-core: AllGather of tiny gram stats over core pairs + local add.

All heavy matmuls run in fp8e4m3 with DoubleRow (2 contraction planes per
instruction, 0.5 cycles/row): conv q,k,v (channel planes), transposed
depthwise (vertical tap pairs, 16-aligned via WS=144), gram (pixel-chunk
pairs), attn 9-tap folds (tap pairs), proj (attn-channel planes).
"""

import sys
import numpy as np

sys.path.insert(0, "/opt/trn_rl_repo")

import contextlib  # noqa: E402

import ml_dtypes  # noqa: E402

from concourse import bass, bacc, tile, mybir  # noqa: E402
from concourse import bass_utils  # noqa: E402

F32 = mybir.dt.float32
BF16 = mybir.dt.bfloat16
F8 = mybir.dt.float8e4
ALU = mybir.AluOpType
ACTF = mybir.ActivationFunctionType
AX = mybir.AxisListType
DR = mybir.MatmulPerfMode.DoubleRow
BF16NP = ml_dtypes.bfloat16
F8NP = ml_dtypes.float8_e4m3

C = 192
HEADS = 8
CH = 24
W = 128
HOUT = 64
HIN = HOUT + 2
WS = 144                  # padded row stride (16-aligned for DoubleRow)
PXIN = HIN * W            # 8448
PXOUT = HOUT * W          # 8192

RS = 16                   # stripe out-rows
NS = HOUT // RS
RIN = RS + 2
SPXI = RIN * W            # 2304
SPXO = RS * W             # 2048
LT = RIN * WS             # padded buffer length (2592)
MMCH = 512

TAPS = [(dy, dx) for dy in (0, 1, 2) for dx in (0, 1, 2)]
# DoubleRow tap pairs must have 16-aligned flat-offset delta: vertical
# pairs (same dx) have delta WS=144. Taps 6,7,8 run as single fp8 MMs.
TAP_PAIRS = [(0, 3), (1, 4), (2, 5)]
TAP_SINGLES = [6, 7, 8]

_CACHE = {}


def _chunks(total, step):
    out, s = [], 0
    while s < total:
        out.append((s, min(step, total - s)))
        s += step
    return out


def _tap_off(cix, ti):
    dy, dx = TAPS[ti]
    return (cix + dy) * WS + 1 + dx


def pair_view(flat, cix, ti, tj):
    """[P, 2, 128] view of two tap windows (plane stride = o1-o0)."""
    o0, o1 = _tap_off(cix, ti), _tap_off(cix, tj)
    d = o1 - o0
    v = flat[:, o0:o0 + 2 * d].rearrange("p (two d) -> p two d", d=d)
    return v[:, :, 0:128]


def wide_pair(wtile, ti, tj, blkw=128):
    """[P, 2, 128] view of two tap blocks in a [P, 9*blkw] weight tile."""
    o0, d = ti * blkw, (tj - ti) * blkw
    v = wtile[:, o0:o0 + 2 * d].rearrange("p (two d) -> p two d", d=d)
    return v[:, :, 0:128]


def build_program():
    nc = bacc.Bacc("TRN2", target_bir_lowering=False, debug=False,
                   enable_asserts=False, num_devices=8)
    io = {}
    io["xs8"] = nc.dram_tensor("xs8", [128, 2 * PXIN], F8,
                               kind="ExternalInput").ap()
    io["ys8"] = nc.dram_tensor("ys8", [128, 2 * PXIN], F8,
                               kind="ExternalInput").ap()
    io["wq8"] = nc.dram_tensor("wq8", [128, 2 * C], F8,
                               kind="ExternalInput").ap()
    io["wk8"] = nc.dram_tensor("wk8", [128, 2 * C], F8,
                               kind="ExternalInput").ap()
    io["xs"] = nc.dram_tensor("xs", [C, PXIN], BF16, kind="ExternalInput").ap()
    io["wv"] = nc.dram_tensor("wv", [C, C], BF16, kind="ExternalInput").ap()
    io["wp1"] = nc.dram_tensor("wp1", [120, C], BF16,
                               kind="ExternalInput").ap()
    io["wp2"] = nc.dram_tensor("wp2", [72, C], BF16,
                               kind="ExternalInput").ap()
    io["dqkd"] = nc.dram_tensor("dqkd", [3 * 128, 9 * 128], F8,
                                kind="ExternalInput").ap()
    io["dvw"] = nc.dram_tensor("dvw", [C, 9], F32, kind="ExternalInput").ap()
    io["tmpq"] = nc.dram_tensor("tmpq", [C, 1], F32, kind="ExternalInput").ap()
    io["em"] = nc.dram_tensor("em", [HEADS, C], BF16, kind="ExternalInput").ap()
    io["eye"] = nc.dram_tensor("eye", [128, 128], F32,
                               kind="ExternalInput").ap()
    io["outp"] = nc.dram_tensor("outp", [C, PXOUT], F32,
                                kind="ExternalOutput").ap()

    with tile.TileContext(nc) as tc, contextlib.ExitStack() as es:
        _emit(nc, tc, io, es)
    nc.compile()
    return nc


def _emit(nc, tc, io, es):
    # ---------------- persistent weights ------------------------------
    wpool = es.enter_context(tc.tile_pool(name="w", bufs=1))
    wq8_t = wpool.tile([128, 2 * C], F8, tag="wq8")
    wk8_t = wpool.tile([128, 2 * C], F8, tag="wk8")
    nc.sync.dma_start(wq8_t[:], io["wq8"][:])
    nc.sync.dma_start(wk8_t[:], io["wk8"][:])
    wv_a = wpool.tile([128, C], BF16, tag="wva")
    wv_b = wpool.tile([64, C], BF16, tag="wvb")
    nc.sync.dma_start(wv_a[:], io["wv"][0:128, :])
    nc.sync.dma_start(wv_b[:], io["wv"][128:192, :])
    wp1_t = wpool.tile([120, C], BF16, tag="wp1")
    wp2_t = wpool.tile([72, C], BF16, tag="wp2")
    nc.sync.dma_start(wp1_t[:], io["wp1"][:])
    nc.sync.dma_start(wp2_t[:], io["wp2"][:])
    dqkd_t = [wpool.tile([128, 9 * 128], F8, tag=f"dqkd{i}",
                         name=f"dqkd{i}") for i in range(3)]
    for i in range(3):
        nc.sync.dma_start(dqkd_t[i][:], io["dqkd"][i * 128:(i + 1) * 128, :])
    dvw1_t = wpool.tile([120, 9], F32, tag="dvw1")
    dvw2_t = wpool.tile([72, 9], F32, tag="dvw2")
    nc.sync.dma_start(dvw1_t[:], io["dvw"][0:120, :])
    nc.sync.dma_start(dvw2_t[:], io["dvw"][120:192, :])
    tmpq_t = wpool.tile([128, 2], F32, tag="tmpq")
    nc.sync.dma_start(tmpq_t[:, 0:1], io["tmpq"][0:128, :])
    nc.sync.dma_start(tmpq_t[0:64, 1:2], io["tmpq"][128:192, :])
    em_t = wpool.tile([HEADS, C], BF16, tag="em")
    nc.sync.dma_start(em_t[:], io["em"][:])
    eye_t = wpool.tile([128, 128], F32, tag="eye")
    nc.sync.dma_start(eye_t[:], io["eye"][:])

    # ---------------- pools -------------------------------------------
    inb = es.enter_context(tc.tile_pool(name="inb", bufs=1))
    tbuf = es.enter_context(tc.tile_pool(name="tbuf", bufs=1))
    dwo = es.enter_context(tc.tile_pool(name="dwo", bufs=1))
    stck = es.enter_context(tc.tile_pool(name="stck", bufs=1))
    small = es.enter_context(tc.tile_pool(name="small", bufs=1))
    outsb = es.enter_context(tc.tile_pool(name="outsb", bufs=2))
    drm = es.enter_context(tc.tile_pool(name="drm", bufs=1, space="DRAM"))

    # pre-zero only the pad columns of each padded buffer slot (cols 0:2
    # and 130:132 of every row; data region is overwritten each stripe;
    # cols 132:144 are never read)
    for b in range(3):
        for _sl in range(2):
            tz = tbuf.tile([128, LT], F8, tag=f"t{b}", name=f"tz{b}_{_sl}",
                           bufs=2)
            tzv = tz[:].rearrange("p (r w) -> p r w", w=WS)
            nc.vector.memset(tzv[:, :, 0:2], 0.0)
            nc.vector.memset(tzv[:, :, 130:132], 0.0)
    tv_tiles = {}
    for _s in range(NS):
        _tva = tbuf.tile([120, LT], BF16, tag="tv_{}".format(_s),
                         name=f"tvz_{_s}", bufs=1)
        _tvb = tbuf.tile([72, LT], BF16, tag="tvb_{}".format(_s),
                         name=f"tvbz_{_s}", bufs=1)
        for _t in (_tva, _tvb):
            _tvv = _t[:].rearrange("p (r w) -> p r w", w=WS)
            nc.vector.memset(_tvv[:, :, 0:2], 0.0)
            nc.vector.memset(_tvv[:, :, 130:132], 0.0)

    xs8v = io["xs8"].rearrange("p (two n) -> p two n", two=2)
    ys8v = io["ys8"].rearrange("p (two n) -> p two n", two=2)
    wq8v = wq8_t[:].rearrange("p (two c) -> p two c", two=2)
    wk8v = wk8_t[:].rearrange("p (two c) -> p two c", two=2)

    # ================= PASS 1 =========================================
    # gram PSUM layout:
    #   gA = [ga: q0:128 x k0:192 (192) | selfg0: q0:128 self (128)]
    #   gB = [b1 self (128) | b1 x b2 (128) | selfg2: k64:192 self (128)]
    with tc.tile_pool(name="cps", bufs=1, space="PSUM") as cpsum, \
         tc.tile_pool(name="gps", bufs=1, space="PSUM") as gpsum:
        gA_ps = gpsum.tile([128, 320], F32, tag="gA")
        gB_ps = gpsum.tile([128, 384], F32, tag="gB")
        for s in range(NS):
            i0 = s * RS * W
            t_blk = [tbuf.tile([128, LT], F8, tag=f"t{b}",
                               name=f"t{b}_{s}", bufs=2) for b in range(3)]

            xq8 = inb.tile([128, 2 * SPXI], F8, tag="xq8", bufs=2)
            yq8 = inb.tile([128, 2 * SPXI], F8, tag="yq8", bufs=2)
            xa16 = inb.tile([128, SPXI], BF16, tag="xa16", bufs=2)
            xb16 = inb.tile([64, SPXI], BF16, tag="xb16", bufs=2)
            nc.gpsimd.dma_start(
                xq8[:].rearrange("p (two n) -> p two n", two=2),
                xs8v[:, :, i0:i0 + SPXI])
            nc.gpsimd.dma_start(
                yq8[:].rearrange("p (two n) -> p two n", two=2),
                ys8v[:, :, i0:i0 + SPXI])
            nc.gpsimd.dma_start(xa16[:], io["xs"][0:128, i0:i0 + SPXI])
            nc.gpsimd.dma_start(xb16[:], io["xs"][128:192, i0:i0 + SPXI])
            xqv = xq8[:].rearrange("p (two n) -> p two n", two=2)
            yqv = yq8[:].rearrange("p (two n) -> p two n", two=2)

            # conv q,k (fp8 DR over the 192-channel contraction), v (bf16)
            # t blocks: [q 0:128] | [q 128:192 ; k 0:64] | [k 64:192]
            tv_a = tbuf.tile([120, LT], BF16, tag="tv_{}".format(s),
                             name=f"tva_{s}", bufs=1)
            tv_b = tbuf.tile([72, LT], BF16, tag="tvb_{}".format(s),
                             name=f"tvb_{s}", bufs=1)
            tv_tiles[s] = (tv_a, tv_b)
            for n0, n in _chunks(SPXI, MMCH):
                r0, nr = n0 // W, n // W
                ps0 = cpsum.tile([128, MMCH], F32, tag="cps0", bufs=2)
                ps1 = cpsum.tile([128, MMCH], F32, tag="cps1", bufs=2)
                psk = cpsum.tile([128, MMCH], F32, tag="cps0", bufs=2,
                                 name=f"psk_{s}_{n0}")
                ps2 = cpsum.tile([128, MMCH], F32, tag="cps1", bufs=2,
                                 name=f"ps2_{s}_{n0}")
                nc.tensor.matmul(ps0[:, 0:n], wq8v[:, :, 0:128],
                                 yqv[:, :, n0:n0 + n], start=True, stop=True,
                                 perf_mode=DR)
                nc.tensor.matmul(ps1[0:64, 0:n], wq8v[:, :, 128:192],
                                 yqv[:, :, n0:n0 + n], start=True, stop=True,
                                 perf_mode=DR)
                nc.tensor.matmul(psk[0:64, 0:n], wk8v[:, :, 0:64],
                                 xqv[:, :, n0:n0 + n], start=True, stop=True,
                                 perf_mode=DR)
                nc.tensor.matmul(ps2[:, 0:n], wk8v[:, :, 64:192],
                                 xqv[:, :, n0:n0 + n], start=True, stop=True,
                                 perf_mode=DR)
                cv0 = cpsum.tile([128, MMCH], F32, tag="cps0", bufs=2,
                                 name=f"cv0_{s}_{n0}")
                cv1 = cpsum.tile([128, MMCH], F32, tag="cps1", bufs=2,
                                 name=f"cv1_{s}_{n0}")
                nc.tensor.matmul(cv0[0:120, 0:n], wv_a[:, 0:120],
                                 xa16[:, n0:n0 + n], start=True, stop=False)
                nc.tensor.matmul(cv0[0:120, 0:n], wv_b[:, 0:120],
                                 xb16[:, n0:n0 + n], start=False, stop=True)
                nc.tensor.matmul(cv1[0:72, 0:n], wv_a[:, 120:192],
                                 xa16[:, n0:n0 + n], start=True, stop=False)
                nc.tensor.matmul(cv1[0:72, 0:n], wv_b[:, 120:192],
                                 xb16[:, n0:n0 + n], start=False, stop=True)
                t0d = t_blk[0][:].rearrange("p (r w) -> p r w", w=WS)
                t1d = t_blk[1][:].rearrange("p (r w) -> p r w", w=WS)
                t2d = t_blk[2][:].rearrange("p (r w) -> p r w", w=WS)
                tvad = tv_a[:].rearrange("p (r w) -> p r w", w=WS)
                tvbd = tv_b[:].rearrange("p (r w) -> p r w", w=WS)

                def pw(ps, lo, hi):
                    return ps[lo:hi, 0:n].rearrange("p (r w) -> p r w", w=W)

                nc.scalar.copy(t0d[:, r0:r0 + nr, 2:130], pw(ps0, 0, 128))
                nc.vector.tensor_copy(t1d[0:64, r0:r0 + nr, 2:130],
                                      pw(ps1, 0, 64))
                nc.scalar.copy(t1d[64:128, r0:r0 + nr, 2:130],
                               pw(psk, 0, 64))
                nc.vector.tensor_copy(t2d[:, r0:r0 + nr, 2:130],
                                      pw(ps2, 0, 128))
                nc.scalar.copy(tvad[0:120, r0:r0 + nr, 2:130],
                               pw(cv0, 0, 120))
                nc.vector.tensor_copy(tvbd[0:72, r0:r0 + nr, 2:130],
                                      pw(cv1, 0, 72))

            # transposed depthwise on PE (fp8 DR tap pairs) -> px-major
            # stacks, then gram accumulation (fp8 DR chunk pairs)
            nchunk = SPXO // 128
            for g in range(nchunk // 4):
                stk = [stck.tile([128, MMCH], F8, tag=f"stk{b}",
                                 name=f"stk{b}_{s}_{g}", bufs=2)
                       for b in range(3)]
                for b in range(3):
                    tp = cpsum.tile([128, MMCH], F32, tag="tps",
                                    name=f"tp{b}_{s}_{g}", bufs=2)
                    tflat = t_blk[b][:]
                    for ci in range(4):
                        cix = g * 4 + ci
                        q0 = ci * 128
                        for pi, (ti, tj) in enumerate(TAP_PAIRS):
                            nc.tensor.matmul(
                                tp[:, q0:q0 + 128],
                                pair_view(tflat, cix, ti, tj),
                                wide_pair(dqkd_t[b], ti, tj),
                                start=(pi == 0), stop=False, perf_mode=DR)
                        for si, ti in enumerate(TAP_SINGLES):
                            o8 = _tap_off(cix, ti)
                            nc.tensor.matmul(
                                tp[:, q0:q0 + 128], tflat[:, o8:o8 + 128],
                                dqkd_t[b][:, ti * 128:(ti + 1) * 128],
                                start=False,
                                stop=(si == len(TAP_SINGLES) - 1))
                    if b == 1:
                        nc.scalar.copy(stk[b][:], tp[:])
                    else:
                        nc.vector.tensor_copy(stk[b][:], tp[:])
                for p in range(2):
                    cix = g * 4 + 2 * p
                    first = (s == 0 and cix == 0)
                    last = (s == NS - 1 and cix == nchunk - 2)
                    c0 = (2 * p) * 128
                    sp = [stk[b][:, c0:c0 + 256].rearrange(
                        "p (two c) -> p two c", two=2) for b in range(3)]
                    nc.tensor.matmul(gA_ps[:, 0:64], sp[0],
                                     sp[1][:, :, 64:128],
                                     start=first, stop=last, perf_mode=DR)
                    nc.tensor.matmul(gA_ps[:, 64:192], sp[0], sp[2],
                                     start=first, stop=last, perf_mode=DR)
                    nc.tensor.matmul(gA_ps[:, 192:320], sp[0], sp[0],
                                     start=first, stop=last, perf_mode=DR)
                    nc.tensor.matmul(gB_ps[:, 0:128], sp[1], sp[1],
                                     start=first, stop=last, perf_mode=DR)
                    nc.tensor.matmul(gB_ps[:, 128:256], sp[1], sp[2],
                                     start=first, stop=last, perf_mode=DR)
                    nc.tensor.matmul(gB_ps[:, 256:384], sp[2], sp[2],
                                     start=first, stop=last, perf_mode=DR)

        # ---- gram -> SBUF, norms, bounce assembly (DMA, crosses parts)
        ga_sb = small.tile([128, C], F32, tag="gasb")
        g1_sb = small.tile([128, 256], F32, tag="g1sb")
        nc.scalar.copy(ga_sb[:], gA_ps[:, 0:192])
        nc.scalar.copy(g1_sb[:], gB_ps[:, 0:256])
        gb_sb = small.tile([64, C], F32, tag="gbsb")
        nc.vector.tensor_copy(gb_sb[:], g1_sb[0:64, 64:256])

        # sq-norms = diag of the self-gram blocks via eye-mask + row sum
        g0m = small.tile([128, 128], F32, tag="g0m")
        nc.vector.tensor_tensor(g0m[:], gA_ps[:, 192:320], eye_t[:],
                                ALU.mult)
        g1m = small.tile([128, 128], F32, tag="g1m")
        nc.vector.tensor_tensor(g1m[:], g1_sb[:, 0:128], eye_t[:], ALU.mult)
        g2m = small.tile([128, 128], F32, tag="g2m")
        nc.vector.tensor_tensor(g2m[:], gB_ps[:, 256:384], eye_t[:],
                                ALU.mult)

    # ================= PASS 2 =========================================
    with tc.tile_pool(name="p2ps", bufs=1, space="PSUM") as pps:
        vb_tiles = {}

        def p2_vbdw(s):
            # v[120:192] depthwise on DVE (channel-major), PE does attn only
            tv_b = tv_tiles[s][1]
            tshb = tbuf.tile([72, LT], BF16, tag="tshb", name=f"tshb_{s}")
            nc.vector.tensor_copy(tshb[:, 0:LT - 2], tv_b[:, 1:LT - 1])
            vb = dwo.tile([72, SPXO], BF16, tag=f"vbdw{s % 2}",
                          name=f"vbdw_{s}", bufs=1)
            vb_tiles[s] = vb
            prodb = dwo.tile([72, SPXO], BF16, tag="prodb", name=f"prodb_{s}")
            vbv = vb[:].rearrange("p (r w) -> p r w", w=W)
            prodbv = prodb[:].rearrange("p (r w) -> p r w", w=W)
            for ti, (dy, dx) in enumerate(TAPS):
                sc = dvw2_t[:, ti:ti + 1]
                if dx == 1:
                    s3 = tv_b[:].rearrange("p (r w) -> p r w", w=WS)
                    view = s3[:, dy:dy + RS, 2:130]
                else:
                    s3 = tshb[:].rearrange("p (r w) -> p r w", w=WS)
                    view = s3[:, dy:dy + RS, dx:dx + 128]
                dstv = vbv if ti == 0 else prodbv
                nc.vector.tensor_scalar(dstv, view, sc, None, ALU.mult)
                if ti > 0:
                    nc.vector.tensor_tensor(vb[:], vb[:], prodb[:], ALU.add)

        def p2_attn(s):
            o0 = s * SPXO
            tv_a, tv_b = tv_tiles.pop(s)
            tva3 = tv_a[:].rearrange("p (r w) -> p r w", w=WS)
            vb = vb_tiles.pop(s)
            oa = outsb.tile([128, SPXO], F32, tag="oa", name=f"oa_{s}",
                            bufs=1)
            ob = outsb.tile([64, SPXO], F32, tag="ob", name=f"ob_{s}",
                            bufs=1)
            for n0, n in _chunks(SPXO, MMCH):
                r0, nr = n0 // W, n // W
                ops1 = pps.tile([120, MMCH], F32, tag="ops1", bufs=2)
                ops2 = pps.tile([72, MMCH], F32, tag="ops2", bufs=2)
                for ti, (dy, dx) in enumerate(TAPS):
                    nc.tensor.matmul(
                        ops1[:, 0:n], bd1t[:, ti * 120:(ti + 1) * 120],
                        tva3[0:120, r0 + dy:r0 + dy + nr, 1 + dx:129 + dx],
                        start=(ti == 0), stop=(ti == 8))
                nc.tensor.matmul(ops2[:, 0:n], bd2[:], vb[:, n0:n0 + n],
                                 start=True, stop=True)
                ao1 = dwo.tile([120, MMCH], BF16, tag="ao1", bufs=2)
                ao2 = dwo.tile([72, MMCH], BF16, tag="ao2", bufs=2)
                nc.scalar.copy(ao1[:, 0:n], ops1[:, 0:n])
                nc.vector.tensor_copy(ao2[:, 0:n], ops2[:, 0:n])
                ppa = pps.tile([128, MMCH], F32, tag="ppa")
                ppb = pps.tile([64, MMCH], F32, tag="ppb")
                nc.tensor.matmul(ppa[:, 0:n], wp1_t[:, 0:128], ao1[:, 0:n],
                                 start=True, stop=False)
                nc.tensor.matmul(ppa[:, 0:n], wp2_t[:, 0:128], ao2[:, 0:n],
                                 start=False, stop=True)
                nc.tensor.matmul(ppb[:, 0:n], wp1_t[:, 128:192], ao1[:, 0:n],
                                 start=True, stop=False)
                nc.tensor.matmul(ppb[:, 0:n], wp2_t[:, 128:192], ao2[:, 0:n],
                                 start=False, stop=True)
                nc.scalar.copy(oa[:, n0:n0 + n], ppa[:, 0:n])
                nc.vector.tensor_copy(ob[:, n0:n0 + n], ppb[:, 0:n])
            nc.gpsimd.dma_start(io["outp"][0:128, o0:o0 + SPXO], oa[:])
            nc.gpsimd.dma_start(io["outp"][128:192, o0:o0 + SPXO], ob[:])

        p2_vbdw(0)
        p2_vbdw(1)
        qn_red = small.tile([128, 3], F32, tag="qnr")
        nc.vector.tensor_reduce(qn_red[:, 0:1], g0m[:], AX.X, ALU.add)
        nc.vector.tensor_reduce(qn_red[:, 1:2], g1m[:], AX.X, ALU.add)
        nc.vector.tensor_reduce(qn_red[:, 2:3], g2m[:], AX.X, ALU.add)

        bounce_in = drm.tile([C, 26], F32)
        bounce_out = drm.tile([2 * C, 26], F32)
        # per-head qk gram blocks -> bounce cols 0:24
        for h in range(HEADS):
            r0 = h * CH
            if r0 + CH <= 128:
                nc.sync.dma_start(bounce_in[r0:r0 + CH, 0:CH],
                                  ga_sb[r0:r0 + CH, r0:r0 + CH])
            elif r0 >= 128:
                nc.sync.dma_start(bounce_in[r0:r0 + CH, 0:CH],
                                  gb_sb[r0 - 128:r0 - 128 + CH, r0:r0 + CH])
            else:  # straddles the 128 boundary (head 5)
                nc.sync.dma_start(bounce_in[r0:128, 0:CH],
                                  ga_sb[r0:128, r0:r0 + CH])
                nc.sync.dma_start(bounce_in[128:r0 + CH, 0:CH],
                                  gb_sb[0:r0 + CH - 128, r0:r0 + CH])
        # qn -> col 24  (q0:128 from g0m, q128:192 from g1m top)
        nc.sync.dma_start(bounce_in[0:128, 24:25], qn_red[:, 0:1])
        nc.sync.dma_start(bounce_in[128:192, 24:25], qn_red[0:64, 1:2])
        # kn -> col 25  (k0:64 from g1m bottom, k64:192 from g2m)
        nc.sync.dma_start(bounce_in[0:64, 25:26], qn_red[64:128, 1:2])
        nc.sync.dma_start(bounce_in[64:192, 25:26], qn_red[0:128, 2:3])

        nc.gpsimd.collective_compute(
            "AllGather", ALU.bypass,
            replica_groups=[[0, 1], [2, 3], [4, 5], [6, 7]],
            ins=[bounce_in[:].opt()], outs=[bounce_out[:].opt()])

        # local add of the two gathered halves
        cmp_a = small.tile([128, 26], F32, tag="cmpa")
        cmp_b = small.tile([64, 26], F32, tag="cmpb")
        cmp_a2 = small.tile([128, 26], F32, tag="cmpa2")
        cmp_b2 = small.tile([64, 26], F32, tag="cmpb2")
        nc.sync.dma_start(cmp_a[:], bounce_out[0:128, :])
        nc.sync.dma_start(cmp_b[:], bounce_out[128:192, :])
        nc.sync.dma_start(cmp_a2[:], bounce_out[192:320, :])
        nc.sync.dma_start(cmp_b2[:], bounce_out[320:384, :])
        nc.vector.tensor_tensor(cmp_a[:], cmp_a[:], cmp_a2[:], ALU.add)
        nc.vector.tensor_tensor(cmp_b[:], cmp_b[:], cmp_b2[:], ALU.add)

        kn8 = small.tile([HEADS, CH], F32, tag="kn8")
        kn8b = small.tile([HEADS, CH], F32, tag="kn8x")
        nc.sync.dma_start(
            kn8[:],
            bounce_out[0:C, :].rearrange("(h c) k -> h c k", c=CH)[:, :, 25])
        nc.sync.dma_start(
            kn8b[:],
            bounce_out[C:2 * C, :].rearrange("(h c) k -> h c k",
                                             c=CH)[:, :, 25])
        nc.vector.tensor_tensor(kn8[:], kn8[:], kn8b[:], ALU.add)

        # rq = temp/sqrt(qn); rk = 1/sqrt(kn) as [8,24]
        rq_a = small.tile([128, 3], F32, tag="rqa")
        rq_b = small.tile([64, 3], F32, tag="rqb")
        for ti, (cmp, rq, nrow) in enumerate(((cmp_a, rq_a, 128),
                                              (cmp_b, rq_b, 64))):
            nc.scalar.activation(rq[:, 0:1], cmp[:, 24:25], ACTF.Sqrt)
            nc.vector.reciprocal(rq[:, 1:2], rq[:, 0:1])
            nc.vector.tensor_scalar(rq[:, 2:3], rq[:, 1:2],
                                    tmpq_t[0:nrow, ti:ti + 1], None, ALU.mult)
        rk8 = small.tile([HEADS, 2 * CH], F32, tag="rk8")
        nc.scalar.activation(rk8[:, 0:CH], kn8[:], ACTF.Sqrt)
        nc.vector.reciprocal(rk8[:, CH:2 * CH], rk8[:, 0:CH])
        rk8b = small.tile([HEADS, CH], BF16, tag="rk8b")
        nc.vector.tensor_copy(rk8b[:], rk8[:, CH:2 * CH])

        knb_a = small.tile([128, CH], F32, tag="knba")
        knb_b = small.tile([64, CH], F32, tag="knbb")
        knb_ps = pps.tile([128, CH], F32, tag="ppa", name="knb_ps")
        nc.tensor.matmul(knb_ps[:], em_t[:, 0:128], rk8b[:],
                         start=True, stop=True)
        nc.scalar.copy(knb_a[:], knb_ps[:])
        knb_ps2 = pps.tile([128, CH], F32, tag="ppa", name="knb_ps2")
        nc.tensor.matmul(knb_ps2[0:64, :], em_t[:, 128:192], rk8b[:],
                         start=True, stop=True)
        nc.scalar.copy(knb_b[:], knb_ps2[0:64, :])

        # s = A*rq*knb ; softmax over d (free dim)
        attn16 = small.tile([128, CH], BF16, tag="att16a")
        attn16b = small.tile([64, CH], BF16, tag="att16b")
        for cmp, rq, knb, a16, nrow in ((cmp_a, rq_a, knb_a, attn16, 128),
                                        (cmp_b, rq_b, knb_b, attn16b, 64)):
            at = small.tile([128, CH], F32, tag="atf")
            sm = small.tile([128, 4], F32, tag="sm")
            nc.vector.tensor_scalar(at[0:nrow, :], cmp[0:nrow, 0:CH], rq[:, 2:3],
                                    None, ALU.mult)
            nc.vector.tensor_tensor(at[0:nrow, :], at[0:nrow, :], knb[:],
                                    ALU.mult)
            nc.vector.tensor_reduce(sm[0:nrow, 0:1], at[0:nrow, :], AX.X, ALU.max)
            nc.vector.tensor_scalar(at[0:nrow, :], at[0:nrow, :], sm[0:nrow, 0:1],
                                    None, ALU.subtract)
            nc.scalar.activation(at[0:nrow, :], at[0:nrow, :], ACTF.Exp)
            nc.vector.tensor_reduce(sm[0:nrow, 1:2], at[0:nrow, :], AX.X, ALU.add)
            nc.vector.reciprocal(sm[0:nrow, 2:3], sm[0:nrow, 1:2])
            nc.vector.tensor_scalar(a16[0:nrow, :], at[0:nrow, :],
                                    sm[0:nrow, 2:3], None, ALU.mult)

        # block-diag attn^T via DRAM round-trip (transposing strided DMAs)
        attn_d = drm.tile([C, CH], BF16)
        nc.sync.dma_start(attn_d[0:128, :], attn16[:])
        nc.sync.dma_start(attn_d[128:192, :], attn16b[:])
        bd1 = small.tile([120, 120], BF16, tag="bd1")      # heads 0-4 attn^T
        bd2 = small.tile([72, 72], BF16, tag="bd2")        # heads 5-7 attn^T
        nc.vector.memset(bd1[:], 0.0)
        nc.vector.memset(bd2[:], 0.0)
        for h in range(5):
            r0 = h * CH
            nc.sync.dma_start(bd1[r0:r0 + CH, r0:r0 + CH],
                              attn_d[r0:r0 + CH, :].rearrange("c d -> d c"))
        for h in range(5, 8):
            r0 = (h - 5) * CH
            nc.sync.dma_start(bd2[r0:r0 + CH, r0:r0 + CH],
                              attn_d[h * CH:(h + 1) * CH, :].rearrange("c d -> d c"))
        # fold depthwise-v tap weights into attn^T: bd1_tap = bd1 * w_v[d,t]
        bd1t = small.tile([120, 9 * 120], BF16, tag="bd1t")
        for ti in range(9):
            nc.vector.tensor_scalar(bd1t[:, ti * 120:(ti + 1) * 120], bd1[:],
                                    dvw1_t[:, ti:ti + 1], None, ALU.mult)

        p2_attn(0)
        p2_vbdw(2)
        p2_attn(1)
        p2_vbdw(3)
        p2_attn(2)
        p2_attn(3)


# ======================================================================
def _prep_inputs(x, y, qkv_w, dw_w, proj_w, temperature):
    wq_t = np.ascontiguousarray(qkv_w[0:C].T)          # [in, out]
    wk_t = np.ascontiguousarray(qkv_w[C:2 * C].T)
    wv_t = np.ascontiguousarray(qkv_w[2 * C:3 * C].T)
    wp_t = np.ascontiguousarray(proj_w.T)

    def planes2(w):
        out = np.zeros((128, 2, C), np.float32)
        out[:, 0, :] = w[0:128]
        out[0:64, 1, :] = w[128:192]
        return out.reshape(128, 2 * C).astype(F8NP)

    wq8, wk8 = planes2(wq_t), planes2(wk_t)
    wv16 = wv_t.astype(BF16NP)
    wp1 = wp_t[0:120].astype(BF16NP)
    wp2 = wp_t[120:192].astype(BF16NP)

    dw = dw_w.reshape(3 * C, 9).astype(np.float32)
    dw_q, dw_k, dw_v = dw[0:C], dw[C:2 * C], dw[2 * C:3 * C]
    dqk = np.concatenate([dw_q[0:128], dw_q[128:192], dw_k[0:64],
                          dw_k[64:192]], axis=0)
    dqkd = np.zeros((3 * 128, 9 * 128), np.float32)
    for i in range(3):
        for t in range(9):
            blk = dqk[i * 128:(i + 1) * 128, t]
            np.fill_diagonal(
                dqkd[i * 128:(i + 1) * 128, t * 128:(t + 1) * 128], blk)
    tmpq = np.repeat(np.asarray(temperature, np.float32).reshape(HEADS),
                     CH).reshape(C, 1)
    em = np.zeros((HEADS, C), np.float32)
    for hh in range(HEADS):
        em[hh, hh * CH:(hh + 1) * CH] = 1.0

    in_maps = []
    for core in range(8):
        bi, half = core // 2, core % 2
        r0 = half * HOUT - 1
        xsl = np.zeros((C, HIN, W), np.float32)
        ysl = np.zeros((C, HIN, W), np.float32)
        lo, hi = max(r0, 0), min(r0 + HIN, 128)
        xsl[:, lo - r0:hi - r0] = x[bi, :, lo:hi]
        ysl[:, lo - r0:hi - r0] = y[bi, :, lo:hi]
        xsl = xsl.reshape(C, PXIN)
        ysl = ysl.reshape(C, PXIN)

        def planes_px(t):
            out = np.zeros((128, 2, PXIN), np.float32)
            out[:, 0, :] = t[0:128]
            out[0:64, 1, :] = t[128:192]
            return out.reshape(128, 2 * PXIN).astype(F8NP)

        in_maps.append({
            "xs": xsl.astype(BF16NP),
            "xs8": planes_px(xsl), "ys8": planes_px(ysl),
            "wq8": wq8, "wk8": wk8, "wv": wv16,
            "wp1": wp1, "wp2": wp2,
            "dqkd": dqkd.astype(F8NP),
            "dvw": dw_v.astype(np.float32),
            "tmpq": tmpq, "em": em.astype(BF16NP),
            "eye": np.eye(128, dtype=np.float32),
        })
    return in_maps


def kernel(x, y, qkv_w, dw_w, proj_w, temperature, _trace=False):
    x = np.asarray(x, np.float32)
    y = np.asarray(y, np.float32)
    if "nc" not in _CACHE:
        _CACHE["nc"] = build_program()
    nc = _CACHE["nc"]
    in_maps = _prep_inputs(x, y, np.asarray(qkv_w, np.float32),
                           np.asarray(dw_w, np.float32),
                           np.asarray(proj_w, np.float32),
                           np.asarray(temperature, np.float32))
    res = bass_utils.run_bass_kernel_spmd(nc, in_maps,
                                          core_ids=list(range(8)),
                                          trace=_trace)
    _CACHE["last_result"] = res
    out = np.empty((4, C, 128, W), np.float32)
    for core in range(8):
        bi, half = core // 2, core % 2
        out[bi, :, half * HOUT:(half + 1) * HOUT] = \
            res.results[core]["outp"].reshape(C, HOUT, W)
    return out


# revision 54
# speedup vs baseline: 1.5659x; 1.0924x over previous
"""Trainium2 Bass kernel for nn_Attention (channel attention, XCA-style).

Sharding: 8 cores = (batch b=core//2) x (image half = core%2, 64 rows + halo).
Cross-core: AllGather of tiny gram stats over core pairs + local add.

All heavy matmuls run in fp8e4m3 with DoubleRow (2 contraction planes per
instruction, 0.5 cycles/row): conv q,k,v (channel planes), transposed
depthwise (vertical tap pairs, 16-aligned via WS=144), gram (pixel-chunk
pairs), attn 9-tap folds (tap pairs), proj (attn-channel planes).
"""

import sys
import numpy as np

sys.path.insert(0, "/opt/trn_rl_repo")

import contextlib  # noqa: E402

import ml_dtypes  # noqa: E402

from concourse import bass, bacc, tile, mybir  # noqa: E402
from concourse import bass_utils  # noqa: E402

F32 = mybir.dt.float32
BF16 = mybir.dt.bfloat16
F8 = mybir.dt.float8e4
ALU = mybir.AluOpType
ACTF = mybir.ActivationFunctionType
AX = mybir.AxisListType
DR = mybir.MatmulPerfMode.DoubleRow
BF16NP = ml_dtypes.bfloat16
F8NP = ml_dtypes.float8_e4m3

C = 192
HEADS = 8
CH = 24
W = 128
HOUT = 64
HIN = HOUT + 2
WS = 144                  # padded row stride (16-aligned for DoubleRow)
PXIN = HIN * W            # 8448
PXOUT = HOUT * W          # 8192

RS = 16                   # stripe out-rows
NS = HOUT // RS
RIN = RS + 2
SPXI = RIN * W            # 2304
SPXO = RS * W             # 2048
LT = RIN * WS             # padded buffer length (2592)
MMCH = 512

TAPS = [(dy, dx) for dy in (0, 1, 2) for dx in (0, 1, 2)]
# DoubleRow tap pairs must have 16-aligned flat-offset delta: vertical
# pairs (same dx) have delta WS=144. Taps 6,7,8 run as single fp8 MMs.
TAP_PAIRS = [(0, 3), (1, 4), (2, 5)]
TAP_SINGLES = [6, 7, 8]

_CACHE = {}


def _chunks(total, step):
    out, s = [], 0
    while s < total:
        out.append((s, min(step, total - s)))
        s += step
    return out


def _tap_off(cix, ti):
    dy, dx = TAPS[ti]
    return (cix + dy) * WS + 1 + dx


def pair_view(flat, cix, ti, tj):
    """[P, 2, 128] view of two tap windows (plane stride = o1-o0)."""
    o0, o1 = _tap_off(cix, ti), _tap_off(cix, tj)
    d = o1 - o0
    v = flat[:, o0:o0 + 2 * d].rearrange("p (two d) -> p two d", d=d)
    return v[:, :, 0:128]


def wide_pair(wtile, ti, tj, blkw=128):
    """[P, 2, 128] view of two tap blocks in a [P, 9*blkw] weight tile."""
    o0, d = ti * blkw, (tj - ti) * blkw
    v = wtile[:, o0:o0 + 2 * d].rearrange("p (two d) -> p two d", d=d)
    return v[:, :, 0:128]


def build_program():
    nc = bacc.Bacc("TRN2", target_bir_lowering=False, debug=False,
                   enable_asserts=False, num_devices=8)
    io = {}
    io["xs8"] = nc.dram_tensor("xs8", [128, 2 * PXIN], F8,
                               kind="ExternalInput").ap()
    io["ys8"] = nc.dram_tensor("ys8", [128, 2 * PXIN], F8,
                               kind="ExternalInput").ap()
    io["wq8"] = nc.dram_tensor("wq8", [128, 2 * C], F8,
                               kind="ExternalInput").ap()
    io["wk8"] = nc.dram_tensor("wk8", [128, 2 * C], F8,
                               kind="ExternalInput").ap()
    io["xs"] = nc.dram_tensor("xs", [C, PXIN], BF16, kind="ExternalInput").ap()
    io["wv"] = nc.dram_tensor("wv", [C, C], BF16, kind="ExternalInput").ap()
    io["wp1"] = nc.dram_tensor("wp1", [120, C], BF16,
                               kind="ExternalInput").ap()
    io["wp2"] = nc.dram_tensor("wp2", [72, C], BF16,
                               kind="ExternalInput").ap()
    io["dqkd"] = nc.dram_tensor("dqkd", [3 * 128, 9 * 128], F8,
                                kind="ExternalInput").ap()
    io["dvw"] = nc.dram_tensor("dvw", [C, 9], F32, kind="ExternalInput").ap()
    io["tmpq"] = nc.dram_tensor("tmpq", [C, 1], F32, kind="ExternalInput").ap()
    io["em"] = nc.dram_tensor("em", [HEADS, C], BF16, kind="ExternalInput").ap()
    io["eye"] = nc.dram_tensor("eye", [128, 128], F32,
                               kind="ExternalInput").ap()
    io["outp"] = nc.dram_tensor("outp", [C, PXOUT], F32,
                                kind="ExternalOutput").ap()

    with tile.TileContext(nc) as tc, contextlib.ExitStack() as es:
        _emit(nc, tc, io, es)
    nc.compile()
    return nc


def _emit(nc, tc, io, es):
    # ---------------- persistent weights ------------------------------
    wpool = es.enter_context(tc.tile_pool(name="w", bufs=1))
    wq8_t = wpool.tile([128, 2 * C], F8, tag="wq8")
    wk8_t = wpool.tile([128, 2 * C], F8, tag="wk8")
    nc.sync.dma_start(wq8_t[:], io["wq8"][:])
    nc.sync.dma_start(wk8_t[:], io["wk8"][:])
    wv_a = wpool.tile([128, C], BF16, tag="wva")
    wv_b = wpool.tile([64, C], BF16, tag="wvb")
    nc.sync.dma_start(wv_a[:], io["wv"][0:128, :])
    nc.sync.dma_start(wv_b[:], io["wv"][128:192, :])
    wp1_t = wpool.tile([120, C], BF16, tag="wp1")
    wp2_t = wpool.tile([72, C], BF16, tag="wp2")
    nc.sync.dma_start(wp1_t[:], io["wp1"][:])
    nc.sync.dma_start(wp2_t[:], io["wp2"][:])
    dqkd_t = [wpool.tile([128, 9 * 128], F8, tag=f"dqkd{i}",
                         name=f"dqkd{i}") for i in range(3)]
    for i in range(3):
        nc.sync.dma_start(dqkd_t[i][:], io["dqkd"][i * 128:(i + 1) * 128, :])
    dvw1_t = wpool.tile([120, 9], F32, tag="dvw1")
    dvw2_t = wpool.tile([72, 9], F32, tag="dvw2")
    nc.sync.dma_start(dvw1_t[:], io["dvw"][0:120, :])
    nc.sync.dma_start(dvw2_t[:], io["dvw"][120:192, :])
    tmpq_t = wpool.tile([128, 2], F32, tag="tmpq")
    nc.sync.dma_start(tmpq_t[:, 0:1], io["tmpq"][0:128, :])
    nc.sync.dma_start(tmpq_t[0:64, 1:2], io["tmpq"][128:192, :])
    em_t = wpool.tile([HEADS, C], BF16, tag="em")
    nc.sync.dma_start(em_t[:], io["em"][:])
    eye_t = wpool.tile([128, 128], F32, tag="eye")
    nc.sync.dma_start(eye_t[:], io["eye"][:])

    # ---------------- pools -------------------------------------------
    inb = es.enter_context(tc.tile_pool(name="inb", bufs=1))
    tbuf = es.enter_context(tc.tile_pool(name="tbuf", bufs=1))
    dwo = es.enter_context(tc.tile_pool(name="dwo", bufs=1))
    stck = es.enter_context(tc.tile_pool(name="stck", bufs=1))
    small = es.enter_context(tc.tile_pool(name="small", bufs=1))
    outsb = es.enter_context(tc.tile_pool(name="outsb", bufs=2))
    drm = es.enter_context(tc.tile_pool(name="drm", bufs=1, space="DRAM"))

    # pre-zero only the pad columns of each padded buffer slot (cols 0:2
    # and 130:132 of every row; data region is overwritten each stripe;
    # cols 132:144 are never read)
    for b in range(3):
        for _sl in range(2):
            tz = tbuf.tile([128, LT], F8, tag=f"t{b}", name=f"tz{b}_{_sl}",
                           bufs=2)
            tzv = tz[:].rearrange("p (r w) -> p r w", w=WS)
            nc.vector.memset(tzv[:, :, 0:2], 0.0)
            nc.vector.memset(tzv[:, :, 130:132], 0.0)
    tv_tiles = {}
    for _s in range(NS):
        _tva = tbuf.tile([120, LT], BF16, tag="tv_{}".format(_s),
                         name=f"tvz_{_s}", bufs=1)
        _tvb = tbuf.tile([72, LT], BF16, tag="tvb_{}".format(_s),
                         name=f"tvbz_{_s}", bufs=1)
        for _t in (_tva, _tvb):
            _tvv = _t[:].rearrange("p (r w) -> p r w", w=WS)
            nc.vector.memset(_tvv[:, :, 0:2], 0.0)
            nc.vector.memset(_tvv[:, :, 130:132], 0.0)

    xs8v = io["xs8"].rearrange("p (two n) -> p two n", two=2)
    ys8v = io["ys8"].rearrange("p (two n) -> p two n", two=2)
    wq8v = wq8_t[:].rearrange("p (two c) -> p two c", two=2)
    wk8v = wk8_t[:].rearrange("p (two c) -> p two c", two=2)

    # ================= PASS 1 =========================================
    # gram PSUM layout:
    #   gA  = [selfg0 (128) | b1 self (128) | selfg2 (128)]  (diag -> norms)
    #   hAB = per-head q x k blocks at partition base 0:
    #         [h0..h4 (5x24) | h5-main 8 rows (24) | h6 (24) | h7 (24) |
    #          h5-aux 16 rows (24)]
    x_tiles = {}
    with tc.tile_pool(name="cps", bufs=1, space="PSUM") as cpsum, \
         tc.tile_pool(name="gps", bufs=1, space="PSUM") as gpsum:
        gA_ps = gpsum.tile([128, 384], F32, tag="gA")
        hAB_ps = gpsum.tile([24, 216], F32, tag="hAB")

        def _qpieces(h):          # (stk idx, lo, hi, row_off)
            q0 = 24 * h
            if q0 + 24 <= 128:
                return [(0, q0, q0 + 24, 0)]
            if q0 >= 128:
                return [(1, q0 - 128, q0 - 104, 0)]
            return [(0, q0, 128, 0), (1, 0, q0 - 104, 128 - q0)]

        def _kpieces(h):          # (stk idx, lo, hi, col_off)
            k0 = 24 * h
            if k0 + 24 <= 64:
                return [(1, 64 + k0, 88 + k0, 0)]
            if k0 >= 64:
                return [(2, k0 - 64, k0 - 40, 0)]
            return [(1, 64 + k0, 128, 0), (2, 0, k0 - 40, 64 - k0)]

        def _hout(h, roff, rs, coff, cs):
            if h <= 4:
                c0 = 24 * h
            elif h == 5:
                c0 = 120 if roff == 0 else 192
            else:
                c0 = 120 + 24 * (h - 5)
            return hAB_ps[0:rs, c0 + coff:c0 + coff + cs]

        for s in range(NS):
            i0 = s * RS * W
            t_blk = [tbuf.tile([128, LT], F8, tag=f"t{b}",
                               name=f"t{b}_{s}", bufs=2) for b in range(3)]

            xq8 = inb.tile([128, 2 * SPXI], F8, tag="xq8", bufs=2)
            yq8 = inb.tile([128, 2 * SPXI], F8, tag="yq8", bufs=2)
            xa16 = inb.tile([128, SPXI], BF16, tag=f"xa16_{s}", bufs=1)
            xb16 = inb.tile([64, SPXI], BF16, tag=f"xb16_{s}", bufs=1)
            x_tiles[s] = (xa16, xb16)
            nc.gpsimd.dma_start(
                xq8[:].rearrange("p (two n) -> p two n", two=2),
                xs8v[:, :, i0:i0 + SPXI])
            nc.gpsimd.dma_start(
                yq8[:].rearrange("p (two n) -> p two n", two=2),
                ys8v[:, :, i0:i0 + SPXI])
            nc.gpsimd.dma_start(xa16[:], io["xs"][0:128, i0:i0 + SPXI])
            nc.gpsimd.dma_start(xb16[:], io["xs"][128:192, i0:i0 + SPXI])
            xqv = xq8[:].rearrange("p (two n) -> p two n", two=2)
            yqv = yq8[:].rearrange("p (two n) -> p two n", two=2)

            # conv q,k (fp8 DR over the 192-channel contraction), v (bf16)
            # t blocks: [q 0:128] | [q 128:192 ; k 0:64] | [k 64:192]
            tv_a = tbuf.tile([120, LT], BF16, tag="tv_{}".format(s),
                             name=f"tva_{s}", bufs=1)
            tv_b = tbuf.tile([72, LT], BF16, tag="tvb_{}".format(s),
                             name=f"tvb_{s}", bufs=1)
            tv_tiles[s] = (tv_a, tv_b)
            for n0, n in _chunks(SPXI, MMCH):
                r0, nr = n0 // W, n // W
                ps0 = cpsum.tile([128, MMCH], F32, tag="cps0", bufs=2)
                ps1 = cpsum.tile([128, MMCH], F32, tag="cps1", bufs=2)
                psk = cpsum.tile([128, MMCH], F32, tag="cps0", bufs=2,
                                 name=f"psk_{s}_{n0}")
                ps2 = cpsum.tile([128, MMCH], F32, tag="cps1", bufs=2,
                                 name=f"ps2_{s}_{n0}")
                nc.tensor.matmul(ps0[:, 0:n], wq8v[:, :, 0:128],
                                 yqv[:, :, n0:n0 + n], start=True, stop=True,
                                 perf_mode=DR)
                nc.tensor.matmul(ps1[0:64, 0:n], wq8v[:, :, 128:192],
                                 yqv[:, :, n0:n0 + n], start=True, stop=True,
                                 perf_mode=DR)
                nc.tensor.matmul(psk[0:64, 0:n], wk8v[:, :, 0:64],
                                 xqv[:, :, n0:n0 + n], start=True, stop=True,
                                 perf_mode=DR)
                nc.tensor.matmul(ps2[:, 0:n], wk8v[:, :, 64:192],
                                 xqv[:, :, n0:n0 + n], start=True, stop=True,
                                 perf_mode=DR)
                t0d = t_blk[0][:].rearrange("p (r w) -> p r w", w=WS)
                t1d = t_blk[1][:].rearrange("p (r w) -> p r w", w=WS)
                t2d = t_blk[2][:].rearrange("p (r w) -> p r w", w=WS)

                def pw(ps, lo, hi):
                    return ps[lo:hi, 0:n].rearrange("p (r w) -> p r w", w=W)

                nc.scalar.copy(t0d[:, r0:r0 + nr, 2:130], pw(ps0, 0, 128))
                nc.vector.tensor_copy(t1d[0:64, r0:r0 + nr, 2:130],
                                      pw(ps1, 0, 64))
                nc.scalar.copy(t1d[64:128, r0:r0 + nr, 2:130],
                               pw(psk, 0, 64))
                nc.scalar.copy(t2d[:, r0:r0 + nr, 2:130], pw(ps2, 0, 128))
                if s < 2:
                    cv0 = cpsum.tile([128, MMCH], F32, tag="cps0", bufs=2,
                                     name=f"cv0_{s}_{n0}")
                    cv1 = cpsum.tile([128, MMCH], F32, tag="cps1", bufs=2,
                                     name=f"cv1_{s}_{n0}")
                    nc.tensor.matmul(cv0[0:120, 0:n], wv_a[:, 0:120],
                                     xa16[:, n0:n0 + n], start=True,
                                     stop=False)
                    nc.tensor.matmul(cv0[0:120, 0:n], wv_b[:, 0:120],
                                     xb16[:, n0:n0 + n], start=False,
                                     stop=True)
                    nc.tensor.matmul(cv1[0:72, 0:n], wv_a[:, 120:192],
                                     xa16[:, n0:n0 + n], start=True,
                                     stop=False)
                    nc.tensor.matmul(cv1[0:72, 0:n], wv_b[:, 120:192],
                                     xb16[:, n0:n0 + n], start=False,
                                     stop=True)
                    tvad = tv_a[:].rearrange("p (r w) -> p r w", w=WS)
                    tvbd = tv_b[:].rearrange("p (r w) -> p r w", w=WS)
                    nc.scalar.copy(tvad[0:120, r0:r0 + nr, 2:130],
                                   pw(cv0, 0, 120))
                    nc.vector.tensor_copy(tvbd[0:72, r0:r0 + nr, 2:130],
                                          pw(cv1, 0, 72))

            # transposed depthwise on PE (fp8 DR tap pairs) -> px-major
            # stacks, then gram accumulation (fp8 DR chunk pairs)
            nchunk = SPXO // 128
            for g in range(nchunk // 4):
                stk = [stck.tile([128, MMCH], F8, tag=f"stk{b}",
                                 name=f"stk{b}_{s}_{g}", bufs=2)
                       for b in range(3)]
                for b in range(3):
                    tp = cpsum.tile([128, MMCH], F32, tag="tps",
                                    name=f"tp{b}_{s}_{g}", bufs=2)
                    tflat = t_blk[b][:]
                    for ci in range(4):
                        cix = g * 4 + ci
                        q0 = ci * 128
                        for pi, (ti, tj) in enumerate(TAP_PAIRS):
                            nc.tensor.matmul(
                                tp[:, q0:q0 + 128],
                                pair_view(tflat, cix, ti, tj),
                                wide_pair(dqkd_t[b], ti, tj),
                                start=(pi == 0), stop=False, perf_mode=DR)
                        for si, ti in enumerate(TAP_SINGLES):
                            o8 = _tap_off(cix, ti)
                            nc.tensor.matmul(
                                tp[:, q0:q0 + 128], tflat[:, o8:o8 + 128],
                                dqkd_t[b][:, ti * 128:(ti + 1) * 128],
                                start=False,
                                stop=(si == len(TAP_SINGLES) - 1))
                    if b == 1:
                        nc.scalar.copy(stk[b][:], tp[:])
                    else:
                        nc.vector.tensor_copy(stk[b][:], tp[:])
                for p in range(2):
                    cix = g * 4 + 2 * p
                    first = (s == 0 and cix == 0)
                    last = (s == NS - 1 and cix == nchunk - 2)
                    c0 = (2 * p) * 128
                    sp = [stk[b][:, c0:c0 + 256].rearrange(
                        "p (two c) -> p two c", two=2) for b in range(3)]
                    nc.tensor.matmul(gA_ps[:, 0:128], sp[0], sp[0],
                                     start=first, stop=last, perf_mode=DR)
                    nc.tensor.matmul(gA_ps[:, 128:256], sp[1], sp[1],
                                     start=first, stop=last, perf_mode=DR)
                    nc.tensor.matmul(gA_ps[:, 256:384], sp[2], sp[2],
                                     start=first, stop=last, perf_mode=DR)
                    for h in range(HEADS):
                        for (lt, llo, lhi, roff) in _qpieces(h):
                            for (rt, rlo, rhi, coff) in _kpieces(h):
                                nc.tensor.matmul(
                                    _hout(h, roff, lhi - llo, coff,
                                          rhi - rlo),
                                    sp[lt][:, :, llo:lhi],
                                    sp[rt][:, :, rlo:rhi],
                                    start=first, stop=last, perf_mode=DR)

        # ---- norms from self-gram diagonals; per-head blocks -> SBUF
        g0m = small.tile([128, 128], F32, tag="g0m")
        nc.vector.tensor_tensor(g0m[:], gA_ps[:, 0:128], eye_t[:], ALU.mult)
        g1m = small.tile([128, 128], F32, tag="g1m")
        nc.vector.tensor_tensor(g1m[:], gA_ps[:, 128:256], eye_t[:],
                                ALU.mult)
        g2m = small.tile([128, 128], F32, tag="g2m")
        nc.vector.tensor_tensor(g2m[:], gA_ps[:, 256:384], eye_t[:],
                                ALU.mult)
        hAB_sb = small.tile([24, 216], F32, tag="hab")
        nc.scalar.copy(hAB_sb[:], hAB_ps[:])

    # ================= PASS 2 =========================================
    with tc.tile_pool(name="p2ps", bufs=1, space="PSUM") as pps:
        vb_tiles = {}

        def p2_vbdw(s):
            # v[120:192] depthwise on DVE (channel-major), PE does attn only
            tv_b = tv_tiles[s][1]
            tshb = tbuf.tile([72, LT], BF16, tag="tshb", name=f"tshb_{s}")
            nc.vector.tensor_copy(tshb[:, 0:LT - 2], tv_b[:, 1:LT - 1])
            vb = dwo.tile([72, SPXO], BF16, tag=f"vbdw{s % 2}",
                          name=f"vbdw_{s}", bufs=1)
            vb_tiles[s] = vb
            prodb = dwo.tile([72, SPXO], BF16, tag="prodb", name=f"prodb_{s}")
            vbv = vb[:].rearrange("p (r w) -> p r w", w=W)
            prodbv = prodb[:].rearrange("p (r w) -> p r w", w=W)
            for ti, (dy, dx) in enumerate(TAPS):
                sc = dvw2_t[:, ti:ti + 1]
                if dx == 1:
                    s3 = tv_b[:].rearrange("p (r w) -> p r w", w=WS)
                    view = s3[:, dy:dy + RS, 2:130]
                else:
                    s3 = tshb[:].rearrange("p (r w) -> p r w", w=WS)
                    view = s3[:, dy:dy + RS, dx:dx + 128]
                dstv = vbv if ti == 0 else prodbv
                nc.vector.tensor_scalar(dstv, view, sc, None, ALU.mult)
                if ti > 0:
                    nc.vector.tensor_tensor(vb[:], vb[:], prodb[:], ALU.add)

        def p2_attn(s):
            o0 = s * SPXO
            tv_a, tv_b = tv_tiles.pop(s)
            tva3 = tv_a[:].rearrange("p (r w) -> p r w", w=WS)
            vb = vb_tiles.pop(s)
            oa = outsb.tile([128, SPXO], F32, tag="oa", name=f"oa_{s}",
                            bufs=2)
            ob = outsb.tile([64, SPXO], F32, tag="ob", name=f"ob_{s}",
                            bufs=2)
            for n0, n in _chunks(SPXO, MMCH):
                r0, nr = n0 // W, n // W
                ops1 = pps.tile([120, MMCH], F32, tag="ops1", bufs=2)
                ops2 = pps.tile([72, MMCH], F32, tag="ops2", bufs=2)
                for ti, (dy, dx) in enumerate(TAPS):
                    nc.tensor.matmul(
                        ops1[:, 0:n], bd1t[:, ti * 120:(ti + 1) * 120],
                        tva3[0:120, r0 + dy:r0 + dy + nr, 1 + dx:129 + dx],
                        start=(ti == 0), stop=(ti == 8))
                nc.tensor.matmul(ops2[:, 0:n], bd2[:], vb[:, n0:n0 + n],
                                 start=True, stop=True)
                ao1 = dwo.tile([120, MMCH], BF16, tag="ao1", bufs=2)
                ao2 = dwo.tile([72, MMCH], BF16, tag="ao2", bufs=2)
                nc.scalar.copy(ao1[:, 0:n], ops1[:, 0:n])
                nc.vector.tensor_copy(ao2[:, 0:n], ops2[:, 0:n])
                ppa = pps.tile([128, MMCH], F32, tag="ppa", bufs=2)
                ppb = pps.tile([64, MMCH], F32, tag="ppb", bufs=2)
                nc.tensor.matmul(ppa[:, 0:n], wp1_t[:, 0:128], ao1[:, 0:n],
                                 start=True, stop=False)
                nc.tensor.matmul(ppa[:, 0:n], wp2_t[:, 0:128], ao2[:, 0:n],
                                 start=False, stop=True)
                nc.tensor.matmul(ppb[:, 0:n], wp1_t[:, 128:192], ao1[:, 0:n],
                                 start=True, stop=False)
                nc.tensor.matmul(ppb[:, 0:n], wp2_t[:, 128:192], ao2[:, 0:n],
                                 start=False, stop=True)
                nc.scalar.copy(oa[:, n0:n0 + n], ppa[:, 0:n])
                nc.vector.tensor_copy(ob[:, n0:n0 + n], ppb[:, 0:n])
            nc.gpsimd.dma_start(io["outp"][0:128, o0:o0 + SPXO], oa[:])
            nc.gpsimd.dma_start(io["outp"][128:192, o0:o0 + SPXO], ob[:])

        # deferred conv-v for stripes 2,3: fills PE during the collective
        def p2_conv(s):
            tv_a, tv_b = tv_tiles[s]
            xa16, xb16 = x_tiles.pop(s)
            for n0, n in _chunks(SPXI, MMCH):
                r0, nr = n0 // W, n // W
                cv0 = pps.tile([120, MMCH], F32, tag="ops1", bufs=2,
                               name=f"dcv0_{s}_{n0}")
                cv1 = pps.tile([72, MMCH], F32, tag="ops2", bufs=2,
                               name=f"dcv1_{s}_{n0}")
                nc.tensor.matmul(cv0[:, 0:n], wv_a[:, 0:120],
                                 xa16[:, n0:n0 + n], start=True, stop=False)
                nc.tensor.matmul(cv0[:, 0:n], wv_b[:, 0:120],
                                 xb16[:, n0:n0 + n], start=False, stop=True)
                nc.tensor.matmul(cv1[:, 0:n], wv_a[:, 120:192],
                                 xa16[:, n0:n0 + n], start=True, stop=False)
                nc.tensor.matmul(cv1[:, 0:n], wv_b[:, 120:192],
                                 xb16[:, n0:n0 + n], start=False, stop=True)
                tvad = tv_a[:].rearrange("p (r w) -> p r w", w=WS)
                tvbd = tv_b[:].rearrange("p (r w) -> p r w", w=WS)
                nc.scalar.copy(
                    tvad[0:120, r0:r0 + nr, 2:130],
                    cv0[:, 0:n].rearrange("p (r w) -> p r w", w=W))
                nc.vector.tensor_copy(
                    tvbd[0:72, r0:r0 + nr, 2:130],
                    cv1[:, 0:n].rearrange("p (r w) -> p r w", w=W))

        qn_red = small.tile([128, 3], F32, tag="qnr")
        nc.vector.tensor_reduce(qn_red[:, 0:1], g0m[:], AX.X, ALU.add)
        nc.vector.tensor_reduce(qn_red[:, 1:2], g1m[:], AX.X, ALU.add)
        nc.vector.tensor_reduce(qn_red[:, 2:3], g2m[:], AX.X, ALU.add)
        # norm staging: na = [qn(q0:128) | kn(k0:128)], nb = rows 128:192
        na = small.tile([128, 2], F32, tag="na")
        nb = small.tile([64, 2], F32, tag="nb")
        nc.vector.tensor_copy(na[:, 0:1], qn_red[:, 0:1])
        nc.scalar.copy(na[0:64, 1:2], qn_red[64:128, 1:2])
        nc.scalar.copy(na[64:128, 1:2], qn_red[0:64, 2:3])
        nc.vector.tensor_copy(nb[:, 0:1], qn_red[0:64, 1:2])
        nc.scalar.copy(nb[:, 1:2], qn_red[64:128, 2:3])

        bounce_in = drm.tile([C, 26], F32)
        bounce_out = drm.tile([2 * C, 26], F32)
        nc.sync.dma_start(
            bounce_in[0:120, 0:CH].rearrange("(h c) k -> c h k", c=CH),
            hAB_sb[:, 0:120].rearrange("c (h k) -> c h k", h=5))
        nc.sync.dma_start(bounce_in[120:128, 0:CH], hAB_sb[0:8, 120:144])
        nc.sync.dma_start(bounce_in[128:144, 0:CH], hAB_sb[0:16, 192:216])
        nc.sync.dma_start(
            bounce_in[144:192, 0:CH].rearrange("(h c) k -> c h k", c=CH),
            hAB_sb[:, 144:192].rearrange("c (h k) -> c h k", h=2))
        nc.sync.dma_start(bounce_in[0:128, 24:26], na[:])
        nc.sync.dma_start(bounce_in[128:192, 24:26], nb[:])
        p2_conv(2)
        p2_conv(3)
        p2_vbdw(0)
        p2_vbdw(1)

        nc.gpsimd.collective_compute(
            "AllGather", ALU.bypass,
            replica_groups=[[0, 1], [2, 3], [4, 5], [6, 7]],
            ins=[bounce_in[:].opt()], outs=[bounce_out[:].opt()])

        # local add of the two gathered halves
        cmp_a = small.tile([128, 26], F32, tag="cmpa")
        cmp_b = small.tile([64, 26], F32, tag="cmpb")
        cmp_a2 = small.tile([128, 26], F32, tag="cmpa2")
        cmp_b2 = small.tile([64, 26], F32, tag="cmpb2")
        nc.sync.dma_start(cmp_a[:], bounce_out[0:128, :])
        nc.sync.dma_start(cmp_b[:], bounce_out[128:192, :])
        nc.sync.dma_start(cmp_a2[:], bounce_out[192:320, :])
        nc.sync.dma_start(cmp_b2[:], bounce_out[320:384, :])
        nc.vector.tensor_tensor(cmp_a[:], cmp_a[:], cmp_a2[:], ALU.add)
        nc.vector.tensor_tensor(cmp_b[:], cmp_b[:], cmp_b2[:], ALU.add)

        kn8 = small.tile([HEADS, CH], F32, tag="kn8")
        kn8b = small.tile([HEADS, CH], F32, tag="kn8x")
        nc.gpsimd.dma_start(
            kn8[:],
            bounce_out[0:C, :].rearrange("(h c) k -> h c k", c=CH)[:, :, 25])
        nc.gpsimd.dma_start(
            kn8b[:],
            bounce_out[C:2 * C, :].rearrange("(h c) k -> h c k",
                                             c=CH)[:, :, 25])
        nc.vector.tensor_tensor(kn8[:], kn8[:], kn8b[:], ALU.add)

        # rq = temp/sqrt(qn); rk = 1/sqrt(kn) as [8,24]
        rq_a = small.tile([128, 3], F32, tag="rqa")
        rq_b = small.tile([64, 3], F32, tag="rqb")
        for ti, (cmp, rq, nrow) in enumerate(((cmp_a, rq_a, 128),
                                              (cmp_b, rq_b, 64))):
            nc.scalar.activation(rq[:, 0:1], cmp[:, 24:25], ACTF.Sqrt)
            nc.vector.reciprocal(rq[:, 1:2], rq[:, 0:1])
            nc.vector.tensor_scalar(rq[:, 2:3], rq[:, 1:2],
                                    tmpq_t[0:nrow, ti:ti + 1], None, ALU.mult)
        rk8 = small.tile([HEADS, 2 * CH], F32, tag="rk8")
        nc.scalar.activation(rk8[:, 0:CH], kn8[:], ACTF.Sqrt)
        nc.vector.reciprocal(rk8[:, CH:2 * CH], rk8[:, 0:CH])
        rk8b = small.tile([HEADS, CH], BF16, tag="rk8b")
        nc.vector.tensor_copy(rk8b[:], rk8[:, CH:2 * CH])

        knb_a = small.tile([128, CH], F32, tag="knba")
        knb_b = small.tile([64, CH], F32, tag="knbb")
        knb_ps = pps.tile([128, CH], F32, tag="ppa", name="knb_ps", bufs=2)
        nc.tensor.matmul(knb_ps[:], em_t[:, 0:128], rk8b[:],
                         start=True, stop=True)
        nc.scalar.copy(knb_a[:], knb_ps[:])
        knb_ps2 = pps.tile([128, CH], F32, tag="ppa", name="knb_ps2", bufs=2)
        nc.tensor.matmul(knb_ps2[0:64, :], em_t[:, 128:192], rk8b[:],
                         start=True, stop=True)
        nc.scalar.copy(knb_b[:], knb_ps2[0:64, :])

        # s = A*rq*knb ; softmax over d (free dim)
        attn16 = small.tile([128, CH], BF16, tag="att16a")
        attn16b = small.tile([64, CH], BF16, tag="att16b")
        for cmp, rq, knb, a16, nrow in ((cmp_a, rq_a, knb_a, attn16, 128),
                                        (cmp_b, rq_b, knb_b, attn16b, 64)):
            at = small.tile([128, CH], F32, tag="atf")
            sm = small.tile([128, 4], F32, tag="sm")
            nc.vector.tensor_scalar(at[0:nrow, :], cmp[0:nrow, 0:CH], rq[:, 2:3],
                                    None, ALU.mult)
            nc.vector.tensor_tensor(at[0:nrow, :], at[0:nrow, :], knb[:],
                                    ALU.mult)
            nc.vector.tensor_reduce(sm[0:nrow, 0:1], at[0:nrow, :], AX.X, ALU.max)
            nc.vector.tensor_scalar(at[0:nrow, :], at[0:nrow, :], sm[0:nrow, 0:1],
                                    None, ALU.subtract)
            nc.scalar.activation(at[0:nrow, :], at[0:nrow, :], ACTF.Exp)
            nc.vector.tensor_reduce(sm[0:nrow, 1:2], at[0:nrow, :], AX.X, ALU.add)
            nc.vector.reciprocal(sm[0:nrow, 2:3], sm[0:nrow, 1:2])
            nc.vector.tensor_scalar(a16[0:nrow, :], at[0:nrow, :],
                                    sm[0:nrow, 2:3], None, ALU.mult)

        # block-diag attn^T via DRAM round-trip (transposing strided DMAs),
        # split across the HWDGE (sync) and SWDGE (gpsimd) queues
        attn_d = drm.tile([C, CH], BF16)
        nc.sync.dma_start(attn_d[0:128, :], attn16[:])
        nc.gpsimd.dma_start(attn_d[128:192, :], attn16b[:])
        bd1 = small.tile([120, 120], BF16, tag="bd1")      # heads 0-4 attn^T
        bd2 = small.tile([72, 72], BF16, tag="bd2")        # heads 5-7 attn^T
        nc.vector.memset(bd1[:], 0.0)
        nc.vector.memset(bd2[:], 0.0)
        for h in range(5):
            r0 = h * CH
            nc.sync.dma_start(
                bd1[r0:r0 + CH, r0:r0 + CH],
                attn_d[r0:r0 + CH, :].rearrange("c d -> d c"))
        for h in range(5, 8):
            r0 = (h - 5) * CH
            nc.gpsimd.dma_start(
                bd2[r0:r0 + CH, r0:r0 + CH],
                attn_d[h * CH:(h + 1) * CH, :].rearrange("c d -> d c"))
        # fold depthwise-v tap weights into attn^T: bd1_tap = bd1 * w_v[d,t]
        bd1t = small.tile([120, 9 * 120], BF16, tag="bd1t")
        for ti in range(9):
            nc.vector.tensor_scalar(bd1t[:, ti * 120:(ti + 1) * 120], bd1[:],
                                    dvw1_t[:, ti:ti + 1], None, ALU.mult)

        p2_attn(0)
        p2_vbdw(2)
        p2_attn(1)
        p2_vbdw(3)
        p2_attn(2)
        p2_attn(3)


# ======================================================================
def _prep_inputs(x, y, qkv_w, dw_w, proj_w, temperature):
    wq_t = np.ascontiguousarray(qkv_w[0:C].T)          # [in, out]
    wk_t = np.ascontiguousarray(qkv_w[C:2 * C].T)
    wv_t = np.ascontiguousarray(qkv_w[2 * C:3 * C].T)
    wp_t = np.ascontiguousarray(proj_w.T)

    def planes2(w):
        out = np.zeros((128, 2, C), np.float32)
        out[:, 0, :] = w[0:128]
        out[0:64, 1, :] = w[128:192]
        return out.reshape(128, 2 * C).astype(F8NP)

    wq8, wk8 = planes2(wq_t), planes2(wk_t)
    wv16 = wv_t.astype(BF16NP)
    wp1 = wp_t[0:120].astype(BF16NP)
    wp2 = wp_t[120:192].astype(BF16NP)

    dw = dw_w.reshape(3 * C, 9).astype(np.float32)
    dw_q, dw_k, dw_v = dw[0:C], dw[C:2 * C], dw[2 * C:3 * C]
    dqk = np.concatenate([dw_q[0:128], dw_q[128:192], dw_k[0:64],
                          dw_k[64:192]], axis=0)
    dqkd = np.zeros((3 * 128, 9 * 128), np.float32)
    for i in range(3):
        for t in range(9):
            blk = dqk[i * 128:(i + 1) * 128, t]
            np.fill_diagonal(
                dqkd[i * 128:(i + 1) * 128, t * 128:(t + 1) * 128], blk)
    tmpq = np.repeat(np.asarray(temperature, np.float32).reshape(HEADS),
                     CH).reshape(C, 1)
    em = np.zeros((HEADS, C), np.float32)
    for hh in range(HEADS):
        em[hh, hh * CH:(hh + 1) * CH] = 1.0

    in_maps = []
    for core in range(8):
        bi, half = core // 2, core % 2
        r0 = half * HOUT - 1
        xsl = np.zeros((C, HIN, W), np.float32)
        ysl = np.zeros((C, HIN, W), np.float32)
        lo, hi = max(r0, 0), min(r0 + HIN, 128)
        xsl[:, lo - r0:hi - r0] = x[bi, :, lo:hi]
        ysl[:, lo - r0:hi - r0] = y[bi, :, lo:hi]
        xsl = xsl.reshape(C, PXIN)
        ysl = ysl.reshape(C, PXIN)

        def planes_px(t):
            out = np.zeros((128, 2, PXIN), np.float32)
            out[:, 0, :] = t[0:128]
            out[0:64, 1, :] = t[128:192]
            return out.reshape(128, 2 * PXIN).astype(F8NP)

        in_maps.append({
            "xs": xsl.astype(BF16NP),
            "xs8": planes_px(xsl), "ys8": planes_px(ysl),
            "wq8": wq8, "wk8": wk8, "wv": wv16,
            "wp1": wp1, "wp2": wp2,
            "dqkd": dqkd.astype(F8NP),
            "dvw": dw_v.astype(np.float32),
            "tmpq": tmpq, "em": em.astype(BF16NP),
            "eye": np.eye(128, dtype=np.float32),
        })
    return in_maps


def kernel(x, y, qkv_w, dw_w, proj_w, temperature, _trace=False):
    x = np.asarray(x, np.float32)
    y = np.asarray(y, np.float32)
    if "nc" not in _CACHE:
        _CACHE["nc"] = build_program()
    nc = _CACHE["nc"]
    in_maps = _prep_inputs(x, y, np.asarray(qkv_w, np.float32),
                           np.asarray(dw_w, np.float32),
                           np.asarray(proj_w, np.float32),
                           np.asarray(temperature, np.float32))
    res = bass_utils.run_bass_kernel_spmd(nc, in_maps,
                                          core_ids=list(range(8)),
                                          trace=_trace)
    _CACHE["last_result"] = res
    out = np.empty((4, C, 128, W), np.float32)
    for core in range(8):
        bi, half = core // 2, core % 2
        out[bi, :, half * HOUT:(half + 1) * HOUT] = \
            res.results[core]["outp"].reshape(C, HOUT, W)
    return out


# revision 75
# speedup vs baseline: 1.5708x; 1.0031x over previous
"""Trainium2 Bass kernel for nn_Attention (channel attention, XCA-style).

Sharding: 8 cores = (batch b=core//2) x (image half = core%2, 64 rows + halo).
Cross-core: AllGather of tiny gram stats over core pairs + local add.

All heavy matmuls run in fp8e4m3 with DoubleRow (2 contraction planes per
instruction, 0.5 cycles/row): conv q,k,v (channel planes), transposed
depthwise (vertical tap pairs, 16-aligned via WS=144), gram (pixel-chunk
pairs), attn 9-tap folds (tap pairs), proj (attn-channel planes).
"""

import sys
import numpy as np

sys.path.insert(0, "/opt/trn_rl_repo")

import contextlib  # noqa: E402

import ml_dtypes  # noqa: E402

from concourse import bass, bacc, tile, mybir  # noqa: E402
from concourse import bass_utils  # noqa: E402

F32 = mybir.dt.float32
BF16 = mybir.dt.bfloat16
F8 = mybir.dt.float8e4
ALU = mybir.AluOpType
ACTF = mybir.ActivationFunctionType
AX = mybir.AxisListType
DR = mybir.MatmulPerfMode.DoubleRow
BF16NP = ml_dtypes.bfloat16
F8NP = ml_dtypes.float8_e4m3

C = 192
HEADS = 8
CH = 24
W = 128
HOUT = 64
HIN = HOUT + 2
WS = 144                  # padded row stride (16-aligned for DoubleRow)
PXIN = HIN * W            # 8448
PXOUT = HOUT * W          # 8192

RS = 16                   # stripe out-rows
NS = HOUT // RS
RIN = RS + 2
SPXI = RIN * W            # 2304
SPXO = RS * W             # 2048
LT = RIN * WS             # padded buffer length (2592)
MMCH = 512

TAPS = [(dy, dx) for dy in (0, 1, 2) for dx in (0, 1, 2)]
# DoubleRow tap pairs must have 16-aligned flat-offset delta: vertical
# pairs (same dx) have delta WS=144. Taps 6,7,8 run as single fp8 MMs.
TAP_PAIRS = [(0, 3), (1, 4), (2, 5)]
TAP_SINGLES = [6, 7, 8]

_CACHE = {}


def _chunks(total, step):
    out, s = [], 0
    while s < total:
        out.append((s, min(step, total - s)))
        s += step
    return out


def _tap_off(cix, ti):
    dy, dx = TAPS[ti]
    return (cix + dy) * WS + 1 + dx


def pair_view(flat, cix, ti, tj):
    """[P, 2, 128] view of two tap windows (plane stride = o1-o0)."""
    o0, o1 = _tap_off(cix, ti), _tap_off(cix, tj)
    d = o1 - o0
    v = flat[:, o0:o0 + 2 * d].rearrange("p (two d) -> p two d", d=d)
    return v[:, :, 0:128]


def wide_pair(wtile, ti, tj, blkw=128):
    """[P, 2, 128] view of two tap blocks in a [P, 9*blkw] weight tile."""
    o0, d = ti * blkw, (tj - ti) * blkw
    v = wtile[:, o0:o0 + 2 * d].rearrange("p (two d) -> p two d", d=d)
    return v[:, :, 0:128]


def build_program():
    nc = bacc.Bacc("TRN2", target_bir_lowering=False, debug=False,
                   enable_asserts=False, num_devices=8)
    io = {}
    io["xs8"] = nc.dram_tensor("xs8", [128, 2 * PXIN], F8,
                               kind="ExternalInput").ap()
    io["ys8"] = nc.dram_tensor("ys8", [128, 2 * PXIN], F8,
                               kind="ExternalInput").ap()
    io["wq8"] = nc.dram_tensor("wq8", [128, 2 * C], F8,
                               kind="ExternalInput").ap()
    io["wk8"] = nc.dram_tensor("wk8", [128, 2 * C], F8,
                               kind="ExternalInput").ap()
    io["xs"] = nc.dram_tensor("xs", [C, PXIN], BF16, kind="ExternalInput").ap()
    io["wv"] = nc.dram_tensor("wv", [C, C], BF16, kind="ExternalInput").ap()
    io["wp1"] = nc.dram_tensor("wp1", [120, C], BF16,
                               kind="ExternalInput").ap()
    io["wp2"] = nc.dram_tensor("wp2", [72, C], BF16,
                               kind="ExternalInput").ap()
    io["dqkd"] = nc.dram_tensor("dqkd", [3 * 128, 9 * 128], F8,
                                kind="ExternalInput").ap()
    io["dvw"] = nc.dram_tensor("dvw", [C, 9], F32, kind="ExternalInput").ap()
    io["tmpq"] = nc.dram_tensor("tmpq", [C, 1], F32, kind="ExternalInput").ap()
    io["em"] = nc.dram_tensor("em", [HEADS, C], BF16, kind="ExternalInput").ap()
    io["eye"] = nc.dram_tensor("eye", [128, 128], F32,
                               kind="ExternalInput").ap()
    io["outp"] = nc.dram_tensor("outp", [C, PXOUT], F32,
                                kind="ExternalOutput").ap()

    with tile.TileContext(nc) as tc, contextlib.ExitStack() as es:
        _emit(nc, tc, io, es)
    nc.compile()
    return nc


def _emit(nc, tc, io, es):
    # ---------------- persistent weights ------------------------------
    wpool = es.enter_context(tc.tile_pool(name="w", bufs=1))
    wq8_t = wpool.tile([128, 2 * C], F8, tag="wq8")
    wk8_t = wpool.tile([128, 2 * C], F8, tag="wk8")
    nc.sync.dma_start(wq8_t[:], io["wq8"][:])
    nc.sync.dma_start(wk8_t[:], io["wk8"][:])
    wv_a = wpool.tile([128, C], BF16, tag="wva")
    wv_b = wpool.tile([64, C], BF16, tag="wvb")
    nc.sync.dma_start(wv_a[:], io["wv"][0:128, :])
    nc.sync.dma_start(wv_b[:], io["wv"][128:192, :])
    wp1_t = wpool.tile([120, C], BF16, tag="wp1")
    wp2_t = wpool.tile([72, C], BF16, tag="wp2")
    nc.sync.dma_start(wp1_t[:], io["wp1"][:])
    nc.sync.dma_start(wp2_t[:], io["wp2"][:])
    dqkd_t = [wpool.tile([128, 9 * 128], F8, tag=f"dqkd{i}",
                         name=f"dqkd{i}") for i in range(3)]
    for i in range(3):
        nc.sync.dma_start(dqkd_t[i][:], io["dqkd"][i * 128:(i + 1) * 128, :])
    dvw1_t = wpool.tile([120, 9], F32, tag="dvw1")
    dvw2_t = wpool.tile([72, 9], F32, tag="dvw2")
    nc.sync.dma_start(dvw1_t[:], io["dvw"][0:120, :])
    nc.sync.dma_start(dvw2_t[:], io["dvw"][120:192, :])
    tmpq_t = wpool.tile([128, 2], F32, tag="tmpq")
    nc.sync.dma_start(tmpq_t[:, 0:1], io["tmpq"][0:128, :])
    nc.sync.dma_start(tmpq_t[0:64, 1:2], io["tmpq"][128:192, :])
    em_t = wpool.tile([HEADS, C], BF16, tag="em")
    nc.sync.dma_start(em_t[:], io["em"][:])
    eye_t = wpool.tile([128, 128], F32, tag="eye")
    nc.sync.dma_start(eye_t[:], io["eye"][:])

    # ---------------- pools -------------------------------------------
    inb = es.enter_context(tc.tile_pool(name="inb", bufs=1))
    tbuf = es.enter_context(tc.tile_pool(name="tbuf", bufs=1))
    dwo = es.enter_context(tc.tile_pool(name="dwo", bufs=1))
    stck = es.enter_context(tc.tile_pool(name="stck", bufs=1))
    small = es.enter_context(tc.tile_pool(name="small", bufs=1))
    outsb = es.enter_context(tc.tile_pool(name="outsb", bufs=2))
    drm = es.enter_context(tc.tile_pool(name="drm", bufs=1, space="DRAM"))

    # pre-zero only the pad columns of each padded buffer slot (cols 0:2
    # and 130:132 of every row; data region is overwritten each stripe;
    # cols 132:144 are never read)
    for b in range(3):
        for _sl in range(2):
            tz = tbuf.tile([128, LT], F8, tag=f"t{b}", name=f"tz{b}_{_sl}",
                           bufs=2)
            tzv = tz[:].rearrange("p (r w) -> p r w", w=WS)
            nc.vector.memset(tzv[:, :, 0:2], 0.0)
            nc.vector.memset(tzv[:, :, 130:132], 0.0)
    tv_tiles = {}
    for _s in range(NS):
        _tva = tbuf.tile([120, LT], BF16, tag="tv_{}".format(_s),
                         name=f"tvz_{_s}", bufs=1)
        _tvb = tbuf.tile([72, LT], BF16, tag="tvb_{}".format(_s),
                         name=f"tvbz_{_s}", bufs=1)
        for _t in (_tva, _tvb):
            _tvv = _t[:].rearrange("p (r w) -> p r w", w=WS)
            nc.vector.memset(_tvv[:, :, 0:2], 0.0)
            nc.vector.memset(_tvv[:, :, 130:132], 0.0)

    xs8v = io["xs8"].rearrange("p (two n) -> p two n", two=2)
    ys8v = io["ys8"].rearrange("p (two n) -> p two n", two=2)
    wq8v = wq8_t[:].rearrange("p (two c) -> p two c", two=2)
    wk8v = wk8_t[:].rearrange("p (two c) -> p two c", two=2)

    # ================= PASS 1 =========================================
    # gram PSUM layout:
    #   gA  = [selfg0 (128) | b1 self (128) | selfg2 (128)]  (diag -> norms)
    #   hAB = per-head q x k blocks at partition base 0:
    #         [h0..h4 (5x24) | h5-main 8 rows (24) | h6 (24) | h7 (24) |
    #          h5-aux 16 rows (24)]
    x_tiles = {}
    with tc.tile_pool(name="cps", bufs=1, space="PSUM") as cpsum, \
         tc.tile_pool(name="gps", bufs=1, space="PSUM") as gpsum:
        gA_ps = gpsum.tile([128, 384], F32, tag="gA")
        hAB_ps = gpsum.tile([24, 216], F32, tag="hAB")

        def _qpieces(h):          # (stk idx, lo, hi, row_off)
            q0 = 24 * h
            if q0 + 24 <= 128:
                return [(0, q0, q0 + 24, 0)]
            if q0 >= 128:
                return [(1, q0 - 128, q0 - 104, 0)]
            return [(0, q0, 128, 0), (1, 0, q0 - 104, 128 - q0)]

        def _kpieces(h):          # (stk idx, lo, hi, col_off)
            k0 = 24 * h
            if k0 + 24 <= 64:
                return [(1, 64 + k0, 88 + k0, 0)]
            if k0 >= 64:
                return [(2, k0 - 64, k0 - 40, 0)]
            return [(1, 64 + k0, 128, 0), (2, 0, k0 - 40, 64 - k0)]

        def _hout(h, roff, rs, coff, cs):
            if h <= 4:
                c0 = 24 * h
            elif h == 5:
                c0 = 120 if roff == 0 else 192
            else:
                c0 = 120 + 24 * (h - 5)
            return hAB_ps[0:rs, c0 + coff:c0 + coff + cs]

        for s in range(NS):
            i0 = s * RS * W
            t_blk = [tbuf.tile([128, LT], F8, tag=f"t{b}",
                               name=f"t{b}_{s}", bufs=2) for b in range(3)]

            xq8 = inb.tile([128, 2 * SPXI], F8, tag="xq8", bufs=2)
            yq8 = inb.tile([128, 2 * SPXI], F8, tag="yq8", bufs=2)
            xa16 = inb.tile([128, SPXI], BF16, tag=f"xa16_{s}", bufs=1)
            xb16 = inb.tile([64, SPXI], BF16, tag=f"xb16_{s}", bufs=1)
            x_tiles[s] = (xa16, xb16)
            nc.gpsimd.dma_start(
                yq8[:].rearrange("p (two n) -> p two n", two=2),
                ys8v[:, :, i0:i0 + SPXI])
            nc.gpsimd.dma_start(
                xq8[:].rearrange("p (two n) -> p two n", two=2),
                xs8v[:, :, i0:i0 + SPXI])
            nc.gpsimd.dma_start(xa16[:], io["xs"][0:128, i0:i0 + SPXI])
            nc.gpsimd.dma_start(xb16[:], io["xs"][128:192, i0:i0 + SPXI])
            xqv = xq8[:].rearrange("p (two n) -> p two n", two=2)
            yqv = yq8[:].rearrange("p (two n) -> p two n", two=2)

            # conv q,k (fp8 DR over the 192-channel contraction), v (bf16)
            # t blocks: [q 0:128] | [q 128:192 ; k 0:64] | [k 64:192]
            tv_a = tbuf.tile([120, LT], BF16, tag="tv_{}".format(s),
                             name=f"tva_{s}", bufs=1)
            tv_b = tbuf.tile([72, LT], BF16, tag="tvb_{}".format(s),
                             name=f"tvb_{s}", bufs=1)
            tv_tiles[s] = (tv_a, tv_b)
            for n0, n in _chunks(SPXI, MMCH):
                r0, nr = n0 // W, n // W
                ps0 = cpsum.tile([128, MMCH], F32, tag="cps0", bufs=2)
                ps1 = cpsum.tile([128, MMCH], F32, tag="cps1", bufs=2)
                psk = cpsum.tile([128, MMCH], F32, tag="cps0", bufs=2,
                                 name=f"psk_{s}_{n0}")
                ps2 = cpsum.tile([128, MMCH], F32, tag="cps1", bufs=2,
                                 name=f"ps2_{s}_{n0}")
                nc.tensor.matmul(ps0[:, 0:n], wq8v[:, :, 0:128],
                                 yqv[:, :, n0:n0 + n], start=True, stop=True,
                                 perf_mode=DR)
                nc.tensor.matmul(ps1[0:64, 0:n], wq8v[:, :, 128:192],
                                 yqv[:, :, n0:n0 + n], start=True, stop=True,
                                 perf_mode=DR)
                nc.tensor.matmul(psk[0:64, 0:n], wk8v[:, :, 0:64],
                                 xqv[:, :, n0:n0 + n], start=True, stop=True,
                                 perf_mode=DR)
                nc.tensor.matmul(ps2[:, 0:n], wk8v[:, :, 64:192],
                                 xqv[:, :, n0:n0 + n], start=True, stop=True,
                                 perf_mode=DR)
                t0d = t_blk[0][:].rearrange("p (r w) -> p r w", w=WS)
                t1d = t_blk[1][:].rearrange("p (r w) -> p r w", w=WS)
                t2d = t_blk[2][:].rearrange("p (r w) -> p r w", w=WS)

                def pw(ps, lo, hi):
                    return ps[lo:hi, 0:n].rearrange("p (r w) -> p r w", w=W)

                nc.scalar.copy(t0d[:, r0:r0 + nr, 2:130], pw(ps0, 0, 128))
                nc.vector.tensor_copy(t1d[0:64, r0:r0 + nr, 2:130],
                                      pw(ps1, 0, 64))
                nc.scalar.copy(t1d[64:128, r0:r0 + nr, 2:130],
                               pw(psk, 0, 64))
                nc.scalar.copy(t2d[:, r0:r0 + nr, 2:130], pw(ps2, 0, 128))
                if s < 2:
                    cv0 = cpsum.tile([128, MMCH], F32, tag="cps0", bufs=2,
                                     name=f"cv0_{s}_{n0}")
                    cv1 = cpsum.tile([128, MMCH], F32, tag="cps1", bufs=2,
                                     name=f"cv1_{s}_{n0}")
                    nc.tensor.matmul(cv0[0:120, 0:n], wv_a[:, 0:120],
                                     xa16[:, n0:n0 + n], start=True,
                                     stop=False)
                    nc.tensor.matmul(cv0[0:120, 0:n], wv_b[:, 0:120],
                                     xb16[:, n0:n0 + n], start=False,
                                     stop=True)
                    nc.tensor.matmul(cv1[0:72, 0:n], wv_a[:, 120:192],
                                     xa16[:, n0:n0 + n], start=True,
                                     stop=False)
                    nc.tensor.matmul(cv1[0:72, 0:n], wv_b[:, 120:192],
                                     xb16[:, n0:n0 + n], start=False,
                                     stop=True)
                    tvad = tv_a[:].rearrange("p (r w) -> p r w", w=WS)
                    tvbd = tv_b[:].rearrange("p (r w) -> p r w", w=WS)
                    nc.scalar.copy(tvad[0:120, r0:r0 + nr, 2:130],
                                   pw(cv0, 0, 120))
                    nc.vector.tensor_copy(tvbd[0:72, r0:r0 + nr, 2:130],
                                          pw(cv1, 0, 72))

            # transposed depthwise on PE (fp8 DR tap pairs) -> px-major
            # stacks, then gram accumulation (fp8 DR chunk pairs)
            nchunk = SPXO // 128
            for g in range(nchunk // 4):
                stk = [stck.tile([128, MMCH], F8, tag=f"stk{b}",
                                 name=f"stk{b}_{s}_{g}", bufs=2)
                       for b in range(3)]
                for b in range(3):
                    tp = cpsum.tile([128, MMCH], F32, tag="tps",
                                    name=f"tp{b}_{s}_{g}", bufs=2)
                    tflat = t_blk[b][:]
                    for ci in range(4):
                        cix = g * 4 + ci
                        q0 = ci * 128
                        for pi, (ti, tj) in enumerate(TAP_PAIRS):
                            nc.tensor.matmul(
                                tp[:, q0:q0 + 128],
                                pair_view(tflat, cix, ti, tj),
                                wide_pair(dqkd_t[b], ti, tj),
                                start=(pi == 0), stop=False, perf_mode=DR)
                        for si, ti in enumerate(TAP_SINGLES):
                            o8 = _tap_off(cix, ti)
                            nc.tensor.matmul(
                                tp[:, q0:q0 + 128], tflat[:, o8:o8 + 128],
                                dqkd_t[b][:, ti * 128:(ti + 1) * 128],
                                start=False,
                                stop=(si == len(TAP_SINGLES) - 1))
                    if b == 1:
                        nc.scalar.copy(stk[b][:], tp[:])
                    else:
                        nc.vector.tensor_copy(stk[b][:], tp[:])
                for p in range(2):
                    cix = g * 4 + 2 * p
                    first = (s == 0 and cix == 0)
                    last = (s == NS - 1 and cix == nchunk - 2)
                    c0 = (2 * p) * 128
                    sp = [stk[b][:, c0:c0 + 256].rearrange(
                        "p (two c) -> p two c", two=2) for b in range(3)]
                    nc.tensor.matmul(gA_ps[:, 0:128], sp[0], sp[0],
                                     start=first, stop=last, perf_mode=DR)
                    nc.tensor.matmul(gA_ps[:, 128:256], sp[1], sp[1],
                                     start=first, stop=last, perf_mode=DR)
                    nc.tensor.matmul(gA_ps[:, 256:384], sp[2], sp[2],
                                     start=first, stop=last, perf_mode=DR)
                    for h in range(HEADS):
                        for (lt, llo, lhi, roff) in _qpieces(h):
                            for (rt, rlo, rhi, coff) in _kpieces(h):
                                nc.tensor.matmul(
                                    _hout(h, roff, lhi - llo, coff,
                                          rhi - rlo),
                                    sp[lt][:, :, llo:lhi],
                                    sp[rt][:, :, rlo:rhi],
                                    start=first, stop=last, perf_mode=DR)

        # ---- norms from self-gram diagonals; per-head blocks -> SBUF
        g0m = small.tile([128, 128], F32, tag="g0m")
        nc.vector.tensor_tensor(g0m[:], gA_ps[:, 0:128], eye_t[:], ALU.mult)
        g1m = small.tile([128, 128], F32, tag="g1m")
        nc.vector.tensor_tensor(g1m[:], gA_ps[:, 128:256], eye_t[:],
                                ALU.mult)
        g2m = small.tile([128, 128], F32, tag="g2m")
        nc.vector.tensor_tensor(g2m[:], gA_ps[:, 256:384], eye_t[:],
                                ALU.mult)
        hAB_sb = small.tile([24, 216], F32, tag="hab")
        nc.scalar.copy(hAB_sb[:], hAB_ps[:])

    # ================= PASS 2 =========================================
    with tc.tile_pool(name="p2ps", bufs=1, space="PSUM") as pps:
        vb_tiles = {}

        def p2_vbdw(s):
            # v[120:192] depthwise on DVE (channel-major), PE does attn only
            tv_b = tv_tiles[s][1]
            tshb = tbuf.tile([72, LT], BF16, tag="tshb", name=f"tshb_{s}")
            nc.vector.tensor_copy(tshb[:, 0:LT - 2], tv_b[:, 1:LT - 1])
            vb = dwo.tile([72, SPXO], BF16, tag=f"vbdw{s % 2}",
                          name=f"vbdw_{s}", bufs=1)
            vb_tiles[s] = vb
            prodb = dwo.tile([72, SPXO], BF16, tag="prodb", name=f"prodb_{s}")
            vbv = vb[:].rearrange("p (r w) -> p r w", w=W)
            prodbv = prodb[:].rearrange("p (r w) -> p r w", w=W)
            for ti, (dy, dx) in enumerate(TAPS):
                sc = dvw2_t[:, ti:ti + 1]
                if dx == 1:
                    s3 = tv_b[:].rearrange("p (r w) -> p r w", w=WS)
                    view = s3[:, dy:dy + RS, 2:130]
                else:
                    s3 = tshb[:].rearrange("p (r w) -> p r w", w=WS)
                    view = s3[:, dy:dy + RS, dx:dx + 128]
                dstv = vbv if ti == 0 else prodbv
                nc.vector.tensor_scalar(dstv, view, sc, None, ALU.mult)
                if ti > 0:
                    nc.vector.tensor_tensor(vb[:], vb[:], prodb[:], ALU.add)

        def p2_attn(s):
            o0 = s * SPXO
            tv_a, tv_b = tv_tiles.pop(s)
            tva3 = tv_a[:].rearrange("p (r w) -> p r w", w=WS)
            vb = vb_tiles.pop(s)
            oa = outsb.tile([128, SPXO], F32, tag="oa", name=f"oa_{s}",
                            bufs=2)
            ob = outsb.tile([64, SPXO], F32, tag="ob", name=f"ob_{s}",
                            bufs=2)
            for n0, n in _chunks(SPXO, MMCH):
                r0, nr = n0 // W, n // W
                ops1 = pps.tile([120, MMCH], F32, tag="ops1", bufs=2)
                ops2 = pps.tile([72, MMCH], F32, tag="ops2", bufs=2)
                for ti, (dy, dx) in enumerate(TAPS):
                    nc.tensor.matmul(
                        ops1[:, 0:n], bd1t[:, ti * 120:(ti + 1) * 120],
                        tva3[0:120, r0 + dy:r0 + dy + nr, 1 + dx:129 + dx],
                        start=(ti == 0), stop=(ti == 8))
                nc.tensor.matmul(ops2[:, 0:n], bd2[:], vb[:, n0:n0 + n],
                                 start=True, stop=True)
                ao1 = dwo.tile([120, MMCH], BF16, tag="ao1", bufs=2)
                ao2 = dwo.tile([72, MMCH], BF16, tag="ao2", bufs=2)
                nc.scalar.copy(ao1[:, 0:n], ops1[:, 0:n])
                nc.vector.tensor_copy(ao2[:, 0:n], ops2[:, 0:n])
                ppa = pps.tile([128, MMCH], F32, tag="ppa", bufs=2)
                ppb = pps.tile([64, MMCH], F32, tag="ppb", bufs=2)
                nc.tensor.matmul(ppa[:, 0:n], wp1_t[:, 0:128], ao1[:, 0:n],
                                 start=True, stop=False)
                nc.tensor.matmul(ppa[:, 0:n], wp2_t[:, 0:128], ao2[:, 0:n],
                                 start=False, stop=True)
                nc.tensor.matmul(ppb[:, 0:n], wp1_t[:, 128:192], ao1[:, 0:n],
                                 start=True, stop=False)
                nc.tensor.matmul(ppb[:, 0:n], wp2_t[:, 128:192], ao2[:, 0:n],
                                 start=False, stop=True)
                nc.scalar.copy(oa[:, n0:n0 + n], ppa[:, 0:n])
                nc.vector.tensor_copy(ob[:, n0:n0 + n], ppb[:, 0:n])
                if n0 + n in (SPXO // 2, SPXO):
                    h0 = 0 if n0 + n == SPXO // 2 else SPXO // 2
                    nc.gpsimd.dma_start(
                        io["outp"][0:128, o0 + h0:o0 + n0 + n],
                        oa[:, h0:n0 + n])
                    nc.gpsimd.dma_start(
                        io["outp"][128:192, o0 + h0:o0 + n0 + n],
                        ob[:, h0:n0 + n])

        # deferred conv-v for stripes 2,3: fills PE during the collective
        def p2_conv(s):
            tv_a, tv_b = tv_tiles[s]
            xa16, xb16 = x_tiles.pop(s)
            for n0, n in _chunks(SPXI, MMCH):
                r0, nr = n0 // W, n // W
                cv0 = pps.tile([120, MMCH], F32, tag="ops1", bufs=2,
                               name=f"dcv0_{s}_{n0}")
                cv1 = pps.tile([72, MMCH], F32, tag="ops2", bufs=2,
                               name=f"dcv1_{s}_{n0}")
                nc.tensor.matmul(cv0[:, 0:n], wv_a[:, 0:120],
                                 xa16[:, n0:n0 + n], start=True, stop=False)
                nc.tensor.matmul(cv0[:, 0:n], wv_b[:, 0:120],
                                 xb16[:, n0:n0 + n], start=False, stop=True)
                nc.tensor.matmul(cv1[:, 0:n], wv_a[:, 120:192],
                                 xa16[:, n0:n0 + n], start=True, stop=False)
                nc.tensor.matmul(cv1[:, 0:n], wv_b[:, 120:192],
                                 xb16[:, n0:n0 + n], start=False, stop=True)
                tvad = tv_a[:].rearrange("p (r w) -> p r w", w=WS)
                tvbd = tv_b[:].rearrange("p (r w) -> p r w", w=WS)
                nc.scalar.copy(
                    tvad[0:120, r0:r0 + nr, 2:130],
                    cv0[:, 0:n].rearrange("p (r w) -> p r w", w=W))
                nc.vector.tensor_copy(
                    tvbd[0:72, r0:r0 + nr, 2:130],
                    cv1[:, 0:n].rearrange("p (r w) -> p r w", w=W))

        qn_red = small.tile([128, 3], F32, tag="qnr")
        nc.vector.tensor_reduce(qn_red[:, 0:1], g0m[:], AX.X, ALU.add)
        nc.vector.tensor_reduce(qn_red[:, 1:2], g1m[:], AX.X, ALU.add)
        nc.vector.tensor_reduce(qn_red[:, 2:3], g2m[:], AX.X, ALU.add)
        # norm staging: na = [qn(q0:128) | kn(k0:128)], nb = rows 128:192
        na = small.tile([128, 2], F32, tag="na")
        nb = small.tile([64, 2], F32, tag="nb")
        nc.vector.tensor_copy(na[:, 0:1], qn_red[:, 0:1])
        nc.scalar.copy(na[0:64, 1:2], qn_red[64:128, 1:2])
        nc.scalar.copy(na[64:128, 1:2], qn_red[0:64, 2:3])
        nc.vector.tensor_copy(nb[:, 0:1], qn_red[0:64, 1:2])
        nc.scalar.copy(nb[:, 1:2], qn_red[64:128, 2:3])

        bounce_in = drm.tile([C, 26], F32)
        bounce_out = drm.tile([2 * C, 26], F32)
        nc.sync.dma_start(
            bounce_in[0:120, 0:CH].rearrange("(h c) k -> c h k", c=CH),
            hAB_sb[:, 0:120].rearrange("c (h k) -> c h k", h=5))
        nc.sync.dma_start(bounce_in[120:128, 0:CH], hAB_sb[0:8, 120:144])
        nc.sync.dma_start(bounce_in[128:144, 0:CH], hAB_sb[0:16, 192:216])
        nc.sync.dma_start(
            bounce_in[144:192, 0:CH].rearrange("(h c) k -> c h k", c=CH),
            hAB_sb[:, 144:192].rearrange("c (h k) -> c h k", h=2))
        nc.sync.dma_start(bounce_in[0:128, 24:26], na[:])
        nc.sync.dma_start(bounce_in[128:192, 24:26], nb[:])

        nc.gpsimd.collective_compute(
            "AllGather", ALU.bypass,
            replica_groups=[[0, 1], [2, 3], [4, 5], [6, 7]],
            ins=[bounce_in[:].opt()], outs=[bounce_out[:].opt()])

        with tc.tile_wait_until(0.088):
            p2_conv(2)
            p2_conv(3)
        p2_vbdw(0)
        p2_vbdw(1)

        # one DMA pulls all 384 gathered rows as [128, 3, 26]; the local
        # add then combines row j*128+p blocks (64-aligned cross-base)
        cmp3 = small.tile([128, 3 * 26], F32, tag="cmp3")
        nc.sync.dma_start(
            cmp3[:].rearrange("p (j k) -> p j k", j=3),
            bounce_out[:].rearrange("(j p) k -> p j k", j=3))
        c3v = cmp3[:].rearrange("p (j k) -> p j k", j=3)
        cmp_a = small.tile([128, 26], F32, tag="cmpa")
        cmp_b = small.tile([64, 26], F32, tag="cmpb")
        tmp_ab = small.tile([128, 26], F32, tag="cmptmp")
        nc.vector.tensor_copy(tmp_ab[0:64, :], c3v[64:128, 1, :])
        nc.vector.tensor_copy(tmp_ab[64:128, :], c3v[0:64, 2, :])
        nc.vector.tensor_tensor(cmp_a[:], c3v[:, 0, :], tmp_ab[:], ALU.add)
        tmp_b = small.tile([64, 26], F32, tag="cmptmpb")
        nc.scalar.copy(tmp_b[:], c3v[64:128, 2, :])
        nc.vector.tensor_tensor(cmp_b[:], c3v[0:64, 1, :], tmp_b[:],
                                ALU.add)

        kn8 = small.tile([HEADS, CH], F32, tag="kn8")
        kn8b = small.tile([HEADS, CH], F32, tag="kn8x")
        nc.gpsimd.dma_start(
            kn8[:],
            bounce_out[0:C, :].rearrange("(h c) k -> h c k", c=CH)[:, :, 25])
        nc.gpsimd.dma_start(
            kn8b[:],
            bounce_out[C:2 * C, :].rearrange("(h c) k -> h c k",
                                             c=CH)[:, :, 25])
        nc.vector.tensor_tensor(kn8[:], kn8[:], kn8b[:], ALU.add)

        # rq = temp/sqrt(qn); rk = 1/sqrt(kn) as [8,24]
        rq_a = small.tile([128, 3], F32, tag="rqa")
        rq_b = small.tile([64, 3], F32, tag="rqb")
        for ti, (cmp, rq, nrow) in enumerate(((cmp_a, rq_a, 128),
                                              (cmp_b, rq_b, 64))):
            nc.scalar.activation(rq[:, 0:1], cmp[:, 24:25], ACTF.Sqrt)
            nc.vector.reciprocal(rq[:, 1:2], rq[:, 0:1])
            nc.vector.tensor_scalar(rq[:, 2:3], rq[:, 1:2],
                                    tmpq_t[0:nrow, ti:ti + 1], None, ALU.mult)
        rk8 = small.tile([HEADS, 2 * CH], F32, tag="rk8")
        nc.scalar.activation(rk8[:, 0:CH], kn8[:], ACTF.Sqrt)
        nc.vector.reciprocal(rk8[:, CH:2 * CH], rk8[:, 0:CH])
        rk8b = small.tile([HEADS, CH], BF16, tag="rk8b")
        nc.vector.tensor_copy(rk8b[:], rk8[:, CH:2 * CH])

        knb_a = small.tile([128, CH], F32, tag="knba")
        knb_b = small.tile([64, CH], F32, tag="knbb")
        knb_ps = pps.tile([128, CH], F32, tag="ppa", name="knb_ps", bufs=2)
        nc.tensor.matmul(knb_ps[:], em_t[:, 0:128], rk8b[:],
                         start=True, stop=True)
        nc.scalar.copy(knb_a[:], knb_ps[:])
        knb_ps2 = pps.tile([128, CH], F32, tag="ppa", name="knb_ps2", bufs=2)
        nc.tensor.matmul(knb_ps2[0:64, :], em_t[:, 128:192], rk8b[:],
                         start=True, stop=True)
        nc.scalar.copy(knb_b[:], knb_ps2[0:64, :])

        # s = A*rq*knb ; softmax over d (free dim)
        attn16 = small.tile([128, CH], BF16, tag="att16a")
        attn16b = small.tile([64, CH], BF16, tag="att16b")
        for cmp, rq, knb, a16, nrow in ((cmp_a, rq_a, knb_a, attn16, 128),
                                        (cmp_b, rq_b, knb_b, attn16b, 64)):
            at = small.tile([128, CH], F32, tag="atf")
            sm = small.tile([128, 4], F32, tag="sm")
            nc.vector.tensor_scalar(at[0:nrow, :], cmp[0:nrow, 0:CH], rq[:, 2:3],
                                    None, ALU.mult)
            nc.vector.tensor_tensor(at[0:nrow, :], at[0:nrow, :], knb[:],
                                    ALU.mult)
            nc.vector.tensor_reduce(sm[0:nrow, 0:1], at[0:nrow, :], AX.X, ALU.max)
            nc.vector.tensor_scalar(at[0:nrow, :], at[0:nrow, :], sm[0:nrow, 0:1],
                                    None, ALU.subtract)
            nc.scalar.activation(at[0:nrow, :], at[0:nrow, :], ACTF.Exp)
            nc.vector.tensor_reduce(sm[0:nrow, 1:2], at[0:nrow, :], AX.X, ALU.add)
            nc.vector.reciprocal(sm[0:nrow, 2:3], sm[0:nrow, 1:2])
            nc.vector.tensor_scalar(a16[0:nrow, :], at[0:nrow, :],
                                    sm[0:nrow, 2:3], None, ALU.mult)

        # block-diag attn^T via DRAM round-trip (transposing strided DMAs),
        # split across the HWDGE (sync) and SWDGE (gpsimd) queues
        attn_d = drm.tile([C, CH], BF16)
        nc.sync.dma_start(attn_d[0:128, :], attn16[:])
        nc.gpsimd.dma_start(attn_d[128:192, :], attn16b[:])
        bd1 = small.tile([120, 120], BF16, tag="bd1")      # heads 0-4 attn^T
        bd2 = small.tile([72, 72], BF16, tag="bd2")        # heads 5-7 attn^T
        nc.vector.memset(bd1[:], 0.0)
        nc.vector.memset(bd2[:], 0.0)
        for h in range(5):
            r0 = h * CH
            nc.sync.dma_start(
                bd1[r0:r0 + CH, r0:r0 + CH],
                attn_d[r0:r0 + CH, :].rearrange("c d -> d c"))
        for h in range(5, 8):
            r0 = (h - 5) * CH
            nc.gpsimd.dma_start(
                bd2[r0:r0 + CH, r0:r0 + CH],
                attn_d[h * CH:(h + 1) * CH, :].rearrange("c d -> d c"))
        # fold depthwise-v tap weights into attn^T: bd1_tap = bd1 * w_v[d,t]
        bd1t = small.tile([120, 9 * 120], BF16, tag="bd1t")
        for ti in range(9):
            nc.vector.tensor_scalar(bd1t[:, ti * 120:(ti + 1) * 120], bd1[:],
                                    dvw1_t[:, ti:ti + 1], None, ALU.mult)

        p2_attn(0)
        p2_vbdw(2)
        p2_attn(1)
        p2_vbdw(3)
        p2_attn(2)
        p2_attn(3)


# ======================================================================
def _prep_inputs(x, y, qkv_w, dw_w, proj_w, temperature):
    wq_t = np.ascontiguousarray(qkv_w[0:C].T)          # [in, out]
    wk_t = np.ascontiguousarray(qkv_w[C:2 * C].T)
    wv_t = np.ascontiguousarray(qkv_w[2 * C:3 * C].T)
    wp_t = np.ascontiguousarray(proj_w.T)

    def planes2(w):
        out = np.zeros((128, 2, C), np.float32)
        out[:, 0, :] = w[0:128]
        out[0:64, 1, :] = w[128:192]
        return out.reshape(128, 2 * C).astype(F8NP)

    wq8, wk8 = planes2(wq_t), planes2(wk_t)
    wv16 = wv_t.astype(BF16NP)
    wp1 = wp_t[0:120].astype(BF16NP)
    wp2 = wp_t[120:192].astype(BF16NP)

    dw = dw_w.reshape(3 * C, 9).astype(np.float32)
    dw_q, dw_k, dw_v = dw[0:C], dw[C:2 * C], dw[2 * C:3 * C]
    dqk = np.concatenate([dw_q[0:128], dw_q[128:192], dw_k[0:64],
                          dw_k[64:192]], axis=0)
    dqkd = np.zeros((3 * 128, 9 * 128), np.float32)
    for i in range(3):
        for t in range(9):
            blk = dqk[i * 128:(i + 1) * 128, t]
            np.fill_diagonal(
                dqkd[i * 128:(i + 1) * 128, t * 128:(t + 1) * 128], blk)
    tmpq = np.repeat(np.asarray(temperature, np.float32).reshape(HEADS),
                     CH).reshape(C, 1)
    em = np.zeros((HEADS, C), np.float32)
    for hh in range(HEADS):
        em[hh, hh * CH:(hh + 1) * CH] = 1.0

    in_maps = []
    for core in range(8):
        bi, half = core // 2, core % 2
        r0 = half * HOUT - 1
        xsl = np.zeros((C, HIN, W), np.float32)
        ysl = np.zeros((C, HIN, W), np.float32)
        lo, hi = max(r0, 0), min(r0 + HIN, 128)
        xsl[:, lo - r0:hi - r0] = x[bi, :, lo:hi]
        ysl[:, lo - r0:hi - r0] = y[bi, :, lo:hi]
        xsl = xsl.reshape(C, PXIN)
        ysl = ysl.reshape(C, PXIN)

        def planes_px(t):
            out = np.zeros((128, 2, PXIN), np.float32)
            out[:, 0, :] = t[0:128]
            out[0:64, 1, :] = t[128:192]
            return out.reshape(128, 2 * PXIN).astype(F8NP)

        in_maps.append({
            "xs": xsl.astype(BF16NP),
            "xs8": planes_px(xsl), "ys8": planes_px(ysl),
            "wq8": wq8, "wk8": wk8, "wv": wv16,
            "wp1": wp1, "wp2": wp2,
            "dqkd": dqkd.astype(F8NP),
            "dvw": dw_v.astype(np.float32),
            "tmpq": tmpq, "em": em.astype(BF16NP),
            "eye": np.eye(128, dtype=np.float32),
        })
    return in_maps


def kernel(x, y, qkv_w, dw_w, proj_w, temperature, _trace=False):
    x = np.asarray(x, np.float32)
    y = np.asarray(y, np.float32)
    if "nc" not in _CACHE:
        _CACHE["nc"] = build_program()
    nc = _CACHE["nc"]
    in_maps = _prep_inputs(x, y, np.asarray(qkv_w, np.float32),
                           np.asarray(dw_w, np.float32),
                           np.asarray(proj_w, np.float32),
                           np.asarray(temperature, np.float32))
    res = bass_utils.run_bass_kernel_spmd(nc, in_maps,
                                          core_ids=list(range(8)),
                                          trace=_trace)
    _CACHE["last_result"] = res
    out = np.empty((4, C, 128, W), np.float32)
    for core in range(8):
        bi, half = core // 2, core % 2
        out[bi, :, half * HOUT:(half + 1) * HOUT] = \
            res.results[core]["outp"].reshape(C, HOUT, W)
    return out


# revision 78
# speedup vs baseline: 1.5728x; 1.0013x over previous
"""Trainium2 Bass kernel for nn_Attention (channel attention, XCA-style).

Sharding: 8 cores = (batch b=core//2) x (image half = core%2, 64 rows + halo).
Cross-core: AllGather of tiny gram stats over core pairs + local add.

All heavy matmuls run in fp8e4m3 with DoubleRow (2 contraction planes per
instruction, 0.5 cycles/row): conv q,k,v (channel planes), transposed
depthwise (vertical tap pairs, 16-aligned via WS=144), gram (pixel-chunk
pairs), attn 9-tap folds (tap pairs), proj (attn-channel planes).
"""

import sys
import numpy as np

sys.path.insert(0, "/opt/trn_rl_repo")

import contextlib  # noqa: E402

import ml_dtypes  # noqa: E402

from concourse import bass, bacc, tile, mybir  # noqa: E402
from concourse import bass_utils  # noqa: E402

F32 = mybir.dt.float32
BF16 = mybir.dt.bfloat16
F8 = mybir.dt.float8e4
ALU = mybir.AluOpType
ACTF = mybir.ActivationFunctionType
AX = mybir.AxisListType
DR = mybir.MatmulPerfMode.DoubleRow
BF16NP = ml_dtypes.bfloat16
F8NP = ml_dtypes.float8_e4m3

C = 192
HEADS = 8
CH = 24
W = 128
HOUT = 64
HIN = HOUT + 2
WS = 144                  # padded row stride (16-aligned for DoubleRow)
PXIN = HIN * W            # 8448
PXOUT = HOUT * W          # 8192

RS = 16                   # stripe out-rows
NS = HOUT // RS
RIN = RS + 2
SPXI = RIN * W            # 2304
SPXO = RS * W             # 2048
LT = RIN * WS             # padded buffer length (2592)
MMCH = 512

TAPS = [(dy, dx) for dy in (0, 1, 2) for dx in (0, 1, 2)]
# DoubleRow tap pairs must have 16-aligned flat-offset delta: vertical
# pairs (same dx) have delta WS=144. Taps 6,7,8 run as single fp8 MMs.
TAP_PAIRS = [(0, 3), (1, 4), (2, 5)]
TAP_SINGLES = [6, 7, 8]

_CACHE = {}


def _chunks(total, step):
    out, s = [], 0
    while s < total:
        out.append((s, min(step, total - s)))
        s += step
    return out


def _tap_off(cix, ti):
    dy, dx = TAPS[ti]
    return (cix + dy) * WS + 1 + dx


def pair_view(flat, cix, ti, tj):
    """[P, 2, 128] view of two tap windows (plane stride = o1-o0)."""
    o0, o1 = _tap_off(cix, ti), _tap_off(cix, tj)
    d = o1 - o0
    v = flat[:, o0:o0 + 2 * d].rearrange("p (two d) -> p two d", d=d)
    return v[:, :, 0:128]


def wide_pair(wtile, ti, tj, blkw=128):
    """[P, 2, 128] view of two tap blocks in a [P, 9*blkw] weight tile."""
    o0, d = ti * blkw, (tj - ti) * blkw
    v = wtile[:, o0:o0 + 2 * d].rearrange("p (two d) -> p two d", d=d)
    return v[:, :, 0:128]


def build_program():
    nc = bacc.Bacc("TRN2", target_bir_lowering=False, debug=False,
                   enable_asserts=False, num_devices=8)
    io = {}
    io["xs8"] = nc.dram_tensor("xs8", [128, 2 * PXIN], F8,
                               kind="ExternalInput").ap()
    io["ys8"] = nc.dram_tensor("ys8", [128, 2 * PXIN], F8,
                               kind="ExternalInput").ap()
    io["wq8"] = nc.dram_tensor("wq8", [128, 2 * C], F8,
                               kind="ExternalInput").ap()
    io["wk8"] = nc.dram_tensor("wk8", [128, 2 * C], F8,
                               kind="ExternalInput").ap()
    io["xs"] = nc.dram_tensor("xs", [C, PXIN], BF16, kind="ExternalInput").ap()
    io["wv"] = nc.dram_tensor("wv", [C, C], BF16, kind="ExternalInput").ap()
    io["wp1"] = nc.dram_tensor("wp1", [120, C], BF16,
                               kind="ExternalInput").ap()
    io["wp2"] = nc.dram_tensor("wp2", [72, C], BF16,
                               kind="ExternalInput").ap()
    io["dqkd"] = nc.dram_tensor("dqkd", [3 * 128, 9 * 128], F8,
                                kind="ExternalInput").ap()
    io["dvw"] = nc.dram_tensor("dvw", [C, 9], F32, kind="ExternalInput").ap()
    io["tmpq"] = nc.dram_tensor("tmpq", [C, 1], F32, kind="ExternalInput").ap()
    io["em"] = nc.dram_tensor("em", [HEADS, C], BF16, kind="ExternalInput").ap()
    io["eye"] = nc.dram_tensor("eye", [128, 128], F32,
                               kind="ExternalInput").ap()
    io["outp"] = nc.dram_tensor("outp", [C, PXOUT], F32,
                                kind="ExternalOutput").ap()

    with tile.TileContext(nc) as tc, contextlib.ExitStack() as es:
        _emit(nc, tc, io, es)
    nc.compile()
    return nc


def _emit(nc, tc, io, es):
    # ---------------- persistent weights ------------------------------
    wpool = es.enter_context(tc.tile_pool(name="w", bufs=1))
    wq8_t = wpool.tile([128, 2 * C], F8, tag="wq8")
    wk8_t = wpool.tile([128, 2 * C], F8, tag="wk8")
    nc.sync.dma_start(wq8_t[:], io["wq8"][:])
    nc.sync.dma_start(wk8_t[:], io["wk8"][:])
    wv_a = wpool.tile([128, C], BF16, tag="wva")
    wv_b = wpool.tile([64, C], BF16, tag="wvb")
    nc.sync.dma_start(wv_a[:], io["wv"][0:128, :])
    nc.sync.dma_start(wv_b[:], io["wv"][128:192, :])
    wp1_t = wpool.tile([120, C], BF16, tag="wp1")
    wp2_t = wpool.tile([72, C], BF16, tag="wp2")
    nc.sync.dma_start(wp1_t[:], io["wp1"][:])
    nc.sync.dma_start(wp2_t[:], io["wp2"][:])
    dqkd_t = [wpool.tile([128, 9 * 128], F8, tag=f"dqkd{i}",
                         name=f"dqkd{i}") for i in range(3)]
    for i in range(3):
        nc.sync.dma_start(dqkd_t[i][:], io["dqkd"][i * 128:(i + 1) * 128, :])
    dvw1_t = wpool.tile([120, 9], F32, tag="dvw1")
    dvw2_t = wpool.tile([72, 9], F32, tag="dvw2")
    nc.sync.dma_start(dvw1_t[:], io["dvw"][0:120, :])
    nc.sync.dma_start(dvw2_t[:], io["dvw"][120:192, :])
    tmpq_t = wpool.tile([128, 2], F32, tag="tmpq")
    nc.sync.dma_start(tmpq_t[:, 0:1], io["tmpq"][0:128, :])
    nc.sync.dma_start(tmpq_t[0:64, 1:2], io["tmpq"][128:192, :])
    em_t = wpool.tile([HEADS, C], BF16, tag="em")
    nc.sync.dma_start(em_t[:], io["em"][:])
    eye_t = wpool.tile([128, 128], F32, tag="eye")
    nc.sync.dma_start(eye_t[:], io["eye"][:])

    # ---------------- pools -------------------------------------------
    inb = es.enter_context(tc.tile_pool(name="inb", bufs=1))
    tbuf = es.enter_context(tc.tile_pool(name="tbuf", bufs=1))
    dwo = es.enter_context(tc.tile_pool(name="dwo", bufs=1))
    stck = es.enter_context(tc.tile_pool(name="stck", bufs=1))
    small = es.enter_context(tc.tile_pool(name="small", bufs=1))
    outsb = es.enter_context(tc.tile_pool(name="outsb", bufs=2))
    drm = es.enter_context(tc.tile_pool(name="drm", bufs=1, space="DRAM"))

    # pre-zero only the pad columns of each padded buffer slot (cols 0:2
    # and 130:132 of every row; data region is overwritten each stripe;
    # cols 132:144 are never read)
    for b in range(3):
        for _sl in range(2):
            tz = tbuf.tile([128, LT], F8, tag=f"t{b}", name=f"tz{b}_{_sl}",
                           bufs=2)
            tzv = tz[:].rearrange("p (r w) -> p r w", w=WS)
            nc.vector.memset(tzv[:, :, 0:2], 0.0)
            nc.vector.memset(tzv[:, :, 130:132], 0.0)
    tv_tiles = {}
    for _s in range(NS):
        _tva = tbuf.tile([120, LT], BF16, tag="tv_{}".format(_s),
                         name=f"tvz_{_s}", bufs=1)
        _tvb = tbuf.tile([72, LT], BF16, tag="tvb_{}".format(_s),
                         name=f"tvbz_{_s}", bufs=1)
        for _t in (_tva, _tvb):
            _tvv = _t[:].rearrange("p (r w) -> p r w", w=WS)
            nc.vector.memset(_tvv[:, :, 0:2], 0.0)
            nc.vector.memset(_tvv[:, :, 130:132], 0.0)

    xs8v = io["xs8"].rearrange("p (two n) -> p two n", two=2)
    ys8v = io["ys8"].rearrange("p (two n) -> p two n", two=2)
    wq8v = wq8_t[:].rearrange("p (two c) -> p two c", two=2)
    wk8v = wk8_t[:].rearrange("p (two c) -> p two c", two=2)

    # ================= PASS 1 =========================================
    # gram PSUM layout:
    #   gA  = [selfg0 (128) | b1 self (128) | selfg2 (128)]  (diag -> norms)
    #   hAB = per-head q x k blocks at partition base 0:
    #         [h0..h4 (5x24) | h5-main 8 rows (24) | h6 (24) | h7 (24) |
    #          h5-aux 16 rows (24)]
    x_tiles = {}
    with tc.tile_pool(name="cps", bufs=1, space="PSUM") as cpsum, \
         tc.tile_pool(name="gps", bufs=1, space="PSUM") as gpsum:
        gA_ps = gpsum.tile([128, 384], F32, tag="gA")
        hAB_ps = gpsum.tile([24, 216], F32, tag="hAB")

        def _qpieces(h):          # (stk idx, lo, hi, row_off)
            q0 = 24 * h
            if q0 + 24 <= 128:
                return [(0, q0, q0 + 24, 0)]
            if q0 >= 128:
                return [(1, q0 - 128, q0 - 104, 0)]
            return [(0, q0, 128, 0), (1, 0, q0 - 104, 128 - q0)]

        def _kpieces(h):          # (stk idx, lo, hi, col_off)
            k0 = 24 * h
            if k0 + 24 <= 64:
                return [(1, 64 + k0, 88 + k0, 0)]
            if k0 >= 64:
                return [(2, k0 - 64, k0 - 40, 0)]
            return [(1, 64 + k0, 128, 0), (2, 0, k0 - 40, 64 - k0)]

        def _hout(h, roff, rs, coff, cs):
            if h <= 4:
                c0 = 24 * h
            elif h == 5:
                c0 = 120 if roff == 0 else 192
            else:
                c0 = 120 + 24 * (h - 5)
            return hAB_ps[0:rs, c0 + coff:c0 + coff + cs]

        for s in range(NS):
            i0 = s * RS * W
            t_blk = [tbuf.tile([128, LT], F8, tag=f"t{b}",
                               name=f"t{b}_{s}", bufs=2) for b in range(3)]

            xq8 = inb.tile([128, 2 * SPXI], F8, tag="xq8", bufs=2)
            yq8 = inb.tile([128, 2 * SPXI], F8, tag="yq8", bufs=2)
            xa16 = inb.tile([128, SPXI], BF16, tag=f"xa16_{s}", bufs=1)
            xb16 = inb.tile([64, SPXI], BF16, tag=f"xb16_{s}", bufs=1)
            x_tiles[s] = (xa16, xb16)
            nc.gpsimd.dma_start(
                yq8[:].rearrange("p (two n) -> p two n", two=2),
                ys8v[:, :, i0:i0 + SPXI])
            nc.gpsimd.dma_start(
                xq8[:].rearrange("p (two n) -> p two n", two=2),
                xs8v[:, :, i0:i0 + SPXI])
            nc.gpsimd.dma_start(xa16[:], io["xs"][0:128, i0:i0 + SPXI])
            nc.gpsimd.dma_start(xb16[:], io["xs"][128:192, i0:i0 + SPXI])
            xqv = xq8[:].rearrange("p (two n) -> p two n", two=2)
            yqv = yq8[:].rearrange("p (two n) -> p two n", two=2)

            # conv q,k (fp8 DR over the 192-channel contraction), v (bf16)
            # t blocks: [q 0:128] | [q 128:192 ; k 0:64] | [k 64:192]
            tv_a = tbuf.tile([120, LT], BF16, tag="tv_{}".format(s),
                             name=f"tva_{s}", bufs=1)
            tv_b = tbuf.tile([72, LT], BF16, tag="tvb_{}".format(s),
                             name=f"tvb_{s}", bufs=1)
            tv_tiles[s] = (tv_a, tv_b)
            for n0, n in _chunks(SPXI, MMCH):
                r0, nr = n0 // W, n // W
                ps0 = cpsum.tile([128, MMCH], F32, tag="cps0", bufs=2)
                ps1 = cpsum.tile([128, MMCH], F32, tag="cps1", bufs=2)
                psk = cpsum.tile([128, MMCH], F32, tag="cps0", bufs=2,
                                 name=f"psk_{s}_{n0}")
                ps2 = cpsum.tile([128, MMCH], F32, tag="cps1", bufs=2,
                                 name=f"ps2_{s}_{n0}")
                nc.tensor.matmul(ps0[:, 0:n], wq8v[:, :, 0:128],
                                 yqv[:, :, n0:n0 + n], start=True, stop=True,
                                 perf_mode=DR)
                nc.tensor.matmul(ps1[0:64, 0:n], wq8v[:, :, 128:192],
                                 yqv[:, :, n0:n0 + n], start=True, stop=True,
                                 perf_mode=DR)
                nc.tensor.matmul(psk[0:64, 0:n], wk8v[:, :, 0:64],
                                 xqv[:, :, n0:n0 + n], start=True, stop=True,
                                 perf_mode=DR)
                nc.tensor.matmul(ps2[:, 0:n], wk8v[:, :, 64:192],
                                 xqv[:, :, n0:n0 + n], start=True, stop=True,
                                 perf_mode=DR)
                t0d = t_blk[0][:].rearrange("p (r w) -> p r w", w=WS)
                t1d = t_blk[1][:].rearrange("p (r w) -> p r w", w=WS)
                t2d = t_blk[2][:].rearrange("p (r w) -> p r w", w=WS)

                def pw(ps, lo, hi):
                    return ps[lo:hi, 0:n].rearrange("p (r w) -> p r w", w=W)

                nc.scalar.copy(t0d[:, r0:r0 + nr, 2:130], pw(ps0, 0, 128))
                nc.vector.tensor_copy(t1d[0:64, r0:r0 + nr, 2:130],
                                      pw(ps1, 0, 64))
                nc.scalar.copy(t1d[64:128, r0:r0 + nr, 2:130],
                               pw(psk, 0, 64))
                nc.scalar.copy(t2d[:, r0:r0 + nr, 2:130], pw(ps2, 0, 128))
                if s < 2:
                    cv0 = cpsum.tile([128, MMCH], F32, tag="cps0", bufs=2,
                                     name=f"cv0_{s}_{n0}")
                    cv1 = cpsum.tile([128, MMCH], F32, tag="cps1", bufs=2,
                                     name=f"cv1_{s}_{n0}")
                    nc.tensor.matmul(cv0[0:120, 0:n], wv_a[:, 0:120],
                                     xa16[:, n0:n0 + n], start=True,
                                     stop=False)
                    nc.tensor.matmul(cv0[0:120, 0:n], wv_b[:, 0:120],
                                     xb16[:, n0:n0 + n], start=False,
                                     stop=True)
                    nc.tensor.matmul(cv1[0:72, 0:n], wv_a[:, 120:192],
                                     xa16[:, n0:n0 + n], start=True,
                                     stop=False)
                    nc.tensor.matmul(cv1[0:72, 0:n], wv_b[:, 120:192],
                                     xb16[:, n0:n0 + n], start=False,
                                     stop=True)
                    tvad = tv_a[:].rearrange("p (r w) -> p r w", w=WS)
                    tvbd = tv_b[:].rearrange("p (r w) -> p r w", w=WS)
                    nc.scalar.copy(tvad[0:120, r0:r0 + nr, 2:130],
                                   pw(cv0, 0, 120))
                    nc.vector.tensor_copy(tvbd[0:72, r0:r0 + nr, 2:130],
                                          pw(cv1, 0, 72))

            # transposed depthwise on PE (fp8 DR tap pairs) -> px-major
            # stacks, then gram accumulation (fp8 DR chunk pairs)
            nchunk = SPXO // 128
            for g in range(nchunk // 4):
                stk = [stck.tile([128, MMCH], F8, tag=f"stk{b}",
                                 name=f"stk{b}_{s}_{g}", bufs=2)
                       for b in range(3)]
                for b in range(3):
                    tp = cpsum.tile([128, MMCH], F32, tag="tps",
                                    name=f"tp{b}_{s}_{g}", bufs=2)
                    tflat = t_blk[b][:]
                    for ci in range(4):
                        cix = g * 4 + ci
                        q0 = ci * 128
                        for pi, (ti, tj) in enumerate(TAP_PAIRS):
                            nc.tensor.matmul(
                                tp[:, q0:q0 + 128],
                                pair_view(tflat, cix, ti, tj),
                                wide_pair(dqkd_t[b], ti, tj),
                                start=(pi == 0), stop=False, perf_mode=DR)
                        for si, ti in enumerate(TAP_SINGLES):
                            o8 = _tap_off(cix, ti)
                            nc.tensor.matmul(
                                tp[:, q0:q0 + 128], tflat[:, o8:o8 + 128],
                                dqkd_t[b][:, ti * 128:(ti + 1) * 128],
                                start=False,
                                stop=(si == len(TAP_SINGLES) - 1))
                    if b == 1:
                        nc.scalar.copy(stk[b][:], tp[:])
                    else:
                        nc.vector.tensor_copy(stk[b][:], tp[:])
                for p in range(2):
                    cix = g * 4 + 2 * p
                    first = (s == 0 and cix == 0)
                    last = (s == NS - 1 and cix == nchunk - 2)
                    c0 = (2 * p) * 128
                    sp = [stk[b][:, c0:c0 + 256].rearrange(
                        "p (two c) -> p two c", two=2) for b in range(3)]
                    nc.tensor.matmul(gA_ps[:, 0:128], sp[0], sp[0],
                                     start=first, stop=last, perf_mode=DR)
                    nc.tensor.matmul(gA_ps[:, 128:256], sp[1], sp[1],
                                     start=first, stop=last, perf_mode=DR)
                    nc.tensor.matmul(gA_ps[:, 256:384], sp[2], sp[2],
                                     start=first, stop=last, perf_mode=DR)
                    for h in range(HEADS):
                        for (lt, llo, lhi, roff) in _qpieces(h):
                            for (rt, rlo, rhi, coff) in _kpieces(h):
                                nc.tensor.matmul(
                                    _hout(h, roff, lhi - llo, coff,
                                          rhi - rlo),
                                    sp[lt][:, :, llo:lhi],
                                    sp[rt][:, :, rlo:rhi],
                                    start=first, stop=last, perf_mode=DR)

        # ---- norms from self-gram diagonals; per-head blocks -> SBUF
        g0m = small.tile([128, 128], F32, tag="g0m")
        nc.vector.tensor_tensor(g0m[:], gA_ps[:, 0:128], eye_t[:], ALU.mult)
        g1m = small.tile([128, 128], F32, tag="g1m")
        nc.vector.tensor_tensor(g1m[:], gA_ps[:, 128:256], eye_t[:],
                                ALU.mult)
        g2m = small.tile([128, 128], F32, tag="g2m")
        nc.vector.tensor_tensor(g2m[:], gA_ps[:, 256:384], eye_t[:],
                                ALU.mult)
        hAB_sb = small.tile([24, 216], F32, tag="hab")
        nc.scalar.copy(hAB_sb[:], hAB_ps[:])

    # ================= PASS 2 =========================================
    with tc.tile_pool(name="p2ps", bufs=1, space="PSUM") as pps:
        vb_tiles = {}

        def p2_vbdw(s):
            # v[120:192] depthwise on DVE (channel-major), PE does attn only
            tv_b = tv_tiles[s][1]
            tshb = tbuf.tile([72, LT], BF16, tag="tshb", name=f"tshb_{s}")
            nc.vector.tensor_copy(tshb[:, 0:LT - 2], tv_b[:, 1:LT - 1])
            vb = dwo.tile([72, SPXO], BF16, tag=f"vbdw{s % 2}",
                          name=f"vbdw_{s}", bufs=1)
            vb_tiles[s] = vb
            prodb = dwo.tile([72, SPXO], BF16, tag="prodb", name=f"prodb_{s}")
            vbv = vb[:].rearrange("p (r w) -> p r w", w=W)
            prodbv = prodb[:].rearrange("p (r w) -> p r w", w=W)
            for ti, (dy, dx) in enumerate(TAPS):
                sc = dvw2_t[:, ti:ti + 1]
                if dx == 1:
                    s3 = tv_b[:].rearrange("p (r w) -> p r w", w=WS)
                    view = s3[:, dy:dy + RS, 2:130]
                else:
                    s3 = tshb[:].rearrange("p (r w) -> p r w", w=WS)
                    view = s3[:, dy:dy + RS, dx:dx + 128]
                dstv = vbv if ti == 0 else prodbv
                nc.vector.tensor_scalar(dstv, view, sc, None, ALU.mult)
                if ti > 0:
                    nc.vector.tensor_tensor(vb[:], vb[:], prodb[:], ALU.add)

        def p2_attn(s):
            o0 = s * SPXO
            tv_a, tv_b = tv_tiles.pop(s)
            tva3 = tv_a[:].rearrange("p (r w) -> p r w", w=WS)
            vb = vb_tiles.pop(s)
            oa = outsb.tile([128, SPXO], F32, tag="oa", name=f"oa_{s}",
                            bufs=2)
            ob = outsb.tile([64, SPXO], F32, tag="ob", name=f"ob_{s}",
                            bufs=2)
            for n0, n in _chunks(SPXO, MMCH):
                r0, nr = n0 // W, n // W
                ops1 = pps.tile([120, MMCH], F32, tag="ops1", bufs=2)
                ops2 = pps.tile([72, MMCH], F32, tag="ops2", bufs=2)
                for ti, (dy, dx) in enumerate(TAPS):
                    nc.tensor.matmul(
                        ops1[:, 0:n], bd1t[:, ti * 120:(ti + 1) * 120],
                        tva3[0:120, r0 + dy:r0 + dy + nr, 1 + dx:129 + dx],
                        start=(ti == 0), stop=(ti == 8))
                nc.tensor.matmul(ops2[:, 0:n], bd2[:], vb[:, n0:n0 + n],
                                 start=True, stop=True)
                ao1 = dwo.tile([120, MMCH], BF16, tag="ao1", bufs=2)
                ao2 = dwo.tile([72, MMCH], BF16, tag="ao2", bufs=2)
                nc.scalar.copy(ao1[:, 0:n], ops1[:, 0:n])
                nc.vector.tensor_copy(ao2[:, 0:n], ops2[:, 0:n])
                ppa = pps.tile([128, MMCH], F32, tag="ppa", bufs=2)
                ppb = pps.tile([64, MMCH], F32, tag="ppb", bufs=2)
                nc.tensor.matmul(ppa[:, 0:n], wp1_t[:, 0:128], ao1[:, 0:n],
                                 start=True, stop=False)
                nc.tensor.matmul(ppa[:, 0:n], wp2_t[:, 0:128], ao2[:, 0:n],
                                 start=False, stop=True)
                nc.tensor.matmul(ppb[:, 0:n], wp1_t[:, 128:192], ao1[:, 0:n],
                                 start=True, stop=False)
                nc.tensor.matmul(ppb[:, 0:n], wp2_t[:, 128:192], ao2[:, 0:n],
                                 start=False, stop=True)
                nc.scalar.copy(oa[:, n0:n0 + n], ppa[:, 0:n])
                nc.vector.tensor_copy(ob[:, n0:n0 + n], ppb[:, 0:n])
                if n0 + n in (SPXO // 2, SPXO):
                    h0 = 0 if n0 + n == SPXO // 2 else SPXO // 2
                    nc.gpsimd.dma_start(
                        io["outp"][0:128, o0 + h0:o0 + n0 + n],
                        oa[:, h0:n0 + n])
                    nc.gpsimd.dma_start(
                        io["outp"][128:192, o0 + h0:o0 + n0 + n],
                        ob[:, h0:n0 + n])

        # deferred conv-v for stripes 2,3: fills PE during the collective
        def p2_conv(s):
            tv_a, tv_b = tv_tiles[s]
            xa16, xb16 = x_tiles.pop(s)
            for n0, n in _chunks(SPXI, MMCH):
                r0, nr = n0 // W, n // W
                cv0 = pps.tile([120, MMCH], F32, tag="ops1", bufs=2,
                               name=f"dcv0_{s}_{n0}")
                cv1 = pps.tile([72, MMCH], F32, tag="ops2", bufs=2,
                               name=f"dcv1_{s}_{n0}")
                nc.tensor.matmul(cv0[:, 0:n], wv_a[:, 0:120],
                                 xa16[:, n0:n0 + n], start=True, stop=False)
                nc.tensor.matmul(cv0[:, 0:n], wv_b[:, 0:120],
                                 xb16[:, n0:n0 + n], start=False, stop=True)
                nc.tensor.matmul(cv1[:, 0:n], wv_a[:, 120:192],
                                 xa16[:, n0:n0 + n], start=True, stop=False)
                nc.tensor.matmul(cv1[:, 0:n], wv_b[:, 120:192],
                                 xb16[:, n0:n0 + n], start=False, stop=True)
                tvad = tv_a[:].rearrange("p (r w) -> p r w", w=WS)
                tvbd = tv_b[:].rearrange("p (r w) -> p r w", w=WS)
                nc.scalar.copy(
                    tvad[0:120, r0:r0 + nr, 2:130],
                    cv0[:, 0:n].rearrange("p (r w) -> p r w", w=W))
                nc.vector.tensor_copy(
                    tvbd[0:72, r0:r0 + nr, 2:130],
                    cv1[:, 0:n].rearrange("p (r w) -> p r w", w=W))

        qn_red = small.tile([128, 3], F32, tag="qnr")
        nc.vector.tensor_reduce(qn_red[:, 0:1], g0m[:], AX.X, ALU.add)
        nc.vector.tensor_reduce(qn_red[:, 1:2], g1m[:], AX.X, ALU.add)
        nc.vector.tensor_reduce(qn_red[:, 2:3], g2m[:], AX.X, ALU.add)
        # norm staging: na = [qn(q0:128) | kn(k0:128)], nb = rows 128:192
        na = small.tile([128, 2], F32, tag="na")
        nb = small.tile([64, 2], F32, tag="nb")
        nc.vector.tensor_copy(na[:, 0:1], qn_red[:, 0:1])
        nc.scalar.copy(na[0:64, 1:2], qn_red[64:128, 1:2])
        nc.scalar.copy(na[64:128, 1:2], qn_red[0:64, 2:3])
        nc.vector.tensor_copy(nb[:, 0:1], qn_red[0:64, 1:2])
        nc.scalar.copy(nb[:, 1:2], qn_red[64:128, 2:3])

        bounce_in = drm.tile([C, 26], F32)
        bounce_out = drm.tile([2 * C, 26], F32)
        nc.sync.dma_start(
            bounce_in[0:120, 0:CH].rearrange("(h c) k -> c h k", c=CH),
            hAB_sb[:, 0:120].rearrange("c (h k) -> c h k", h=5))
        nc.sync.dma_start(bounce_in[120:128, 0:CH], hAB_sb[0:8, 120:144])
        nc.sync.dma_start(bounce_in[128:144, 0:CH], hAB_sb[0:16, 192:216])
        nc.sync.dma_start(
            bounce_in[144:192, 0:CH].rearrange("(h c) k -> c h k", c=CH),
            hAB_sb[:, 144:192].rearrange("c (h k) -> c h k", h=2))
        nc.sync.dma_start(bounce_in[0:128, 24:26], na[:])
        nc.sync.dma_start(bounce_in[128:192, 24:26], nb[:])

        nc.gpsimd.collective_compute(
            "AllGather", ALU.bypass,
            replica_groups=[[0, 1], [2, 3], [4, 5], [6, 7]],
            ins=[bounce_in[:].opt()], outs=[bounce_out[:].opt()])

        with tc.tile_wait_until(0.090):
            p2_conv(2)
            p2_conv(3)
        p2_vbdw(0)
        p2_vbdw(1)
        p2_vbdw(2)
        p2_vbdw(3)

        # one DMA pulls all 384 gathered rows as [128, 3, 26]; the local
        # add then combines row j*128+p blocks (64-aligned cross-base)
        cmp3 = small.tile([128, 3 * 26], F32, tag="cmp3")
        nc.sync.dma_start(
            cmp3[:].rearrange("p (j k) -> p j k", j=3),
            bounce_out[:].rearrange("(j p) k -> p j k", j=3))
        c3v = cmp3[:].rearrange("p (j k) -> p j k", j=3)
        cmp_a = small.tile([128, 26], F32, tag="cmpa")
        cmp_b = small.tile([64, 26], F32, tag="cmpb")
        tmp_ab = small.tile([128, 26], F32, tag="cmptmp")
        nc.vector.tensor_copy(tmp_ab[0:64, :], c3v[64:128, 1, :])
        nc.vector.tensor_copy(tmp_ab[64:128, :], c3v[0:64, 2, :])
        nc.vector.tensor_tensor(cmp_a[:], c3v[:, 0, :], tmp_ab[:], ALU.add)
        tmp_b = small.tile([64, 26], F32, tag="cmptmpb")
        nc.scalar.copy(tmp_b[:], c3v[64:128, 2, :])
        nc.vector.tensor_tensor(cmp_b[:], c3v[0:64, 1, :], tmp_b[:],
                                ALU.add)

        kn8 = small.tile([HEADS, CH], F32, tag="kn8")
        kn8b = small.tile([HEADS, CH], F32, tag="kn8x")
        nc.gpsimd.dma_start(
            kn8[:],
            bounce_out[0:C, :].rearrange("(h c) k -> h c k", c=CH)[:, :, 25])
        nc.gpsimd.dma_start(
            kn8b[:],
            bounce_out[C:2 * C, :].rearrange("(h c) k -> h c k",
                                             c=CH)[:, :, 25])
        nc.vector.tensor_tensor(kn8[:], kn8[:], kn8b[:], ALU.add)

        # rq = temp/sqrt(qn); rk = 1/sqrt(kn) as [8,24]
        rq_a = small.tile([128, 3], F32, tag="rqa")
        rq_b = small.tile([64, 3], F32, tag="rqb")
        for ti, (cmp, rq, nrow) in enumerate(((cmp_a, rq_a, 128),
                                              (cmp_b, rq_b, 64))):
            nc.scalar.activation(rq[:, 0:1], cmp[:, 24:25], ACTF.Sqrt)
            nc.vector.reciprocal(rq[:, 1:2], rq[:, 0:1])
            nc.vector.tensor_scalar(rq[:, 2:3], rq[:, 1:2],
                                    tmpq_t[0:nrow, ti:ti + 1], None, ALU.mult)
        rk8 = small.tile([HEADS, 2 * CH], F32, tag="rk8")
        nc.scalar.activation(rk8[:, 0:CH], kn8[:], ACTF.Sqrt)
        nc.vector.reciprocal(rk8[:, CH:2 * CH], rk8[:, 0:CH])
        rk8b = small.tile([HEADS, CH], BF16, tag="rk8b")
        nc.vector.tensor_copy(rk8b[:], rk8[:, CH:2 * CH])

        knb_a = small.tile([128, CH], F32, tag="knba")
        knb_b = small.tile([64, CH], F32, tag="knbb")
        knb_ps = pps.tile([128, CH], F32, tag="ppa", name="knb_ps", bufs=2)
        nc.tensor.matmul(knb_ps[:], em_t[:, 0:128], rk8b[:],
                         start=True, stop=True)
        nc.scalar.copy(knb_a[:], knb_ps[:])
        knb_ps2 = pps.tile([128, CH], F32, tag="ppa", name="knb_ps2", bufs=2)
        nc.tensor.matmul(knb_ps2[0:64, :], em_t[:, 128:192], rk8b[:],
                         start=True, stop=True)
        nc.scalar.copy(knb_b[:], knb_ps2[0:64, :])

        # s = A*rq*knb ; softmax over d (free dim)
        attn16 = small.tile([128, CH], BF16, tag="att16a")
        attn16b = small.tile([64, CH], BF16, tag="att16b")
        for cmp, rq, knb, a16, nrow in ((cmp_a, rq_a, knb_a, attn16, 128),
                                        (cmp_b, rq_b, knb_b, attn16b, 64)):
            at = small.tile([128, CH], F32, tag="atf")
            sm = small.tile([128, 4], F32, tag="sm")
            nc.vector.tensor_scalar(at[0:nrow, :], cmp[0:nrow, 0:CH], rq[:, 2:3],
                                    None, ALU.mult)
            nc.vector.tensor_tensor(at[0:nrow, :], at[0:nrow, :], knb[:],
                                    ALU.mult)
            nc.vector.tensor_reduce(sm[0:nrow, 0:1], at[0:nrow, :], AX.X, ALU.max)
            nc.vector.tensor_scalar(at[0:nrow, :], at[0:nrow, :], sm[0:nrow, 0:1],
                                    None, ALU.subtract)
            nc.scalar.activation(at[0:nrow, :], at[0:nrow, :], ACTF.Exp)
            nc.vector.tensor_reduce(sm[0:nrow, 1:2], at[0:nrow, :], AX.X, ALU.add)
            nc.vector.reciprocal(sm[0:nrow, 2:3], sm[0:nrow, 1:2])
            nc.vector.tensor_scalar(a16[0:nrow, :], at[0:nrow, :],
                                    sm[0:nrow, 2:3], None, ALU.mult)

        # block-diag attn^T via DRAM round-trip (transposing strided DMAs),
        # split across the HWDGE (sync) and SWDGE (gpsimd) queues
        attn_d = drm.tile([C, CH], BF16)
        nc.sync.dma_start(attn_d[0:128, :], attn16[:])
        nc.gpsimd.dma_start(attn_d[128:192, :], attn16b[:])
        bd1 = small.tile([120, 120], BF16, tag="bd1")      # heads 0-4 attn^T
        bd2 = small.tile([72, 72], BF16, tag="bd2")        # heads 5-7 attn^T
        nc.vector.memset(bd1[:], 0.0)
        nc.vector.memset(bd2[:], 0.0)
        for h in range(5):
            r0 = h * CH
            nc.sync.dma_start(
                bd1[r0:r0 + CH, r0:r0 + CH],
                attn_d[r0:r0 + CH, :].rearrange("c d -> d c"))
        for h in range(5, 8):
            r0 = (h - 5) * CH
            nc.gpsimd.dma_start(
                bd2[r0:r0 + CH, r0:r0 + CH],
                attn_d[h * CH:(h + 1) * CH, :].rearrange("c d -> d c"))
        # fold depthwise-v tap weights into attn^T: bd1_tap = bd1 * w_v[d,t]
        bd1t = small.tile([120, 9 * 120], BF16, tag="bd1t")
        for ti in range(9):
            nc.vector.tensor_scalar(bd1t[:, ti * 120:(ti + 1) * 120], bd1[:],
                                    dvw1_t[:, ti:ti + 1], None, ALU.mult)

        p2_attn(0)
        p2_attn(1)
        p2_attn(2)
        p2_attn(3)


# ======================================================================
def _prep_inputs(x, y, qkv_w, dw_w, proj_w, temperature):
    wq_t = np.ascontiguousarray(qkv_w[0:C].T)          # [in, out]
    wk_t = np.ascontiguousarray(qkv_w[C:2 * C].T)
    wv_t = np.ascontiguousarray(qkv_w[2 * C:3 * C].T)
    wp_t = np.ascontiguousarray(proj_w.T)

    def planes2(w):
        out = np.zeros((128, 2, C), np.float32)
        out[:, 0, :] = w[0:128]
        out[0:64, 1, :] = w[128:192]
        return out.reshape(128, 2 * C).astype(F8NP)

    wq8, wk8 = planes2(wq_t), planes2(wk_t)
    wv16 = wv_t.astype(BF16NP)
    wp1 = wp_t[0:120].astype(BF16NP)
    wp2 = wp_t[120:192].astype(BF16NP)

    dw = dw_w.reshape(3 * C, 9).astype(np.float32)
    dw_q, dw_k, dw_v = dw[0:C], dw[C:2 * C], dw[2 * C:3 * C]
    dqk = np.concatenate([dw_q[0:128], dw_q[128:192], dw_k[0:64],
                          dw_k[64:192]], axis=0)
    dqkd = np.zeros((3 * 128, 9 * 128), np.float32)
    for i in range(3):
        for t in range(9):
            blk = dqk[i * 128:(i + 1) * 128, t]
            np.fill_diagonal(
                dqkd[i * 128:(i + 1) * 128, t * 128:(t + 1) * 128], blk)
    tmpq = np.repeat(np.asarray(temperature, np.float32).reshape(HEADS),
                     CH).reshape(C, 1)
    em = np.zeros((HEADS, C), np.float32)
    for hh in range(HEADS):
        em[hh, hh * CH:(hh + 1) * CH] = 1.0

    in_maps = []
    for core in range(8):
        bi, half = core // 2, core % 2
        r0 = half * HOUT - 1
        xsl = np.zeros((C, HIN, W), np.float32)
        ysl = np.zeros((C, HIN, W), np.float32)
        lo, hi = max(r0, 0), min(r0 + HIN, 128)
        xsl[:, lo - r0:hi - r0] = x[bi, :, lo:hi]
        ysl[:, lo - r0:hi - r0] = y[bi, :, lo:hi]
        xsl = xsl.reshape(C, PXIN)
        ysl = ysl.reshape(C, PXIN)

        def planes_px(t):
            out = np.zeros((128, 2, PXIN), np.float32)
            out[:, 0, :] = t[0:128]
            out[0:64, 1, :] = t[128:192]
            return out.reshape(128, 2 * PXIN).astype(F8NP)

        in_maps.append({
            "xs": xsl.astype(BF16NP),
            "xs8": planes_px(xsl), "ys8": planes_px(ysl),
            "wq8": wq8, "wk8": wk8, "wv": wv16,
            "wp1": wp1, "wp2": wp2,
            "dqkd": dqkd.astype(F8NP),
            "dvw": dw_v.astype(np.float32),
            "tmpq": tmpq, "em": em.astype(BF16NP),
            "eye": np.eye(128, dtype=np.float32),
        })
    return in_maps


def kernel(x, y, qkv_w, dw_w, proj_w, temperature, _trace=False):
    x = np.asarray(x, np.float32)
    y = np.asarray(y, np.float32)
    if "nc" not in _CACHE:
        _CACHE["nc"] = build_program()
    nc = _CACHE["nc"]
    in_maps = _prep_inputs(x, y, np.asarray(qkv_w, np.float32),
                           np.asarray(dw_w, np.float32),
                           np.asarray(proj_w, np.float32),
                           np.asarray(temperature, np.float32))
    res = bass_utils.run_bass_kernel_spmd(nc, in_maps,
                                          core_ids=list(range(8)),
                                          trace=_trace)
    _CACHE["last_result"] = res
    out = np.empty((4, C, 128, W), np.float32)
    for core in range(8):
        bi, half = core // 2, core % 2
        out[bi, :, half * HOUT:(half + 1) * HOUT] = \
            res.results[core]["outp"].reshape(C, HOUT, W)
    return out


# revision 86
# speedup vs baseline: 1.6717x; 1.0629x over previous
"""Trainium2 Bass kernel for nn_Attention (channel attention, XCA-style).

Sharding: 8 cores = (batch b=core//2) x (image half = core%2, 64 rows + halo).
Cross-core: AllGather of tiny gram stats over core pairs + local add.

All heavy matmuls run in fp8e4m3 with DoubleRow (2 contraction planes per
instruction, 0.5 cycles/row): conv q,k,v (channel planes), transposed
depthwise (vertical tap pairs, 16-aligned via WS=144), gram (pixel-chunk
pairs), attn 9-tap folds (tap pairs), proj (attn-channel planes).
"""

import sys
import numpy as np

sys.path.insert(0, "/opt/trn_rl_repo")

import contextlib  # noqa: E402

import ml_dtypes  # noqa: E402

from concourse import bass, bacc, tile, mybir  # noqa: E402
from concourse import bass_utils  # noqa: E402

F32 = mybir.dt.float32
BF16 = mybir.dt.bfloat16
F8 = mybir.dt.float8e4
ALU = mybir.AluOpType
ACTF = mybir.ActivationFunctionType
AX = mybir.AxisListType
DR = mybir.MatmulPerfMode.DoubleRow
BF16NP = ml_dtypes.bfloat16
F8NP = ml_dtypes.float8_e4m3

C = 192
HEADS = 8
CH = 24
W = 128
HOUT = 64
HIN = HOUT + 2
WS = 144                  # padded row stride (16-aligned for DoubleRow)
PXIN = HIN * W            # 8448
PXOUT = HOUT * W          # 8192

RS = 16                   # stripe out-rows
NS = HOUT // RS
RIN = RS + 2
SPXI = RIN * W            # 2304
SPXO = RS * W             # 2048
LT = RIN * WS             # padded buffer length (2592)
MMCH = 512

TAPS = [(dy, dx) for dy in (0, 1, 2) for dx in (0, 1, 2)]
# DoubleRow tap pairs must have 16-aligned flat-offset delta: vertical
# pairs (same dx) have delta WS=144. Taps 6,7,8 run as single fp8 MMs.
TAP_PAIRS = [(0, 3), (1, 4), (2, 5)]
TAP_SINGLES = [6, 7, 8]

_CACHE = {}


def _chunks(total, step):
    out, s = [], 0
    while s < total:
        out.append((s, min(step, total - s)))
        s += step
    return out


def _tap_off(cix, ti):
    dy, dx = TAPS[ti]
    return (cix + dy) * WS + 1 + dx


def pair_view(flat, cix, ti, tj):
    """[P, 2, 128] view of two tap windows (plane stride = o1-o0)."""
    o0, o1 = _tap_off(cix, ti), _tap_off(cix, tj)
    d = o1 - o0
    v = flat[:, o0:o0 + 2 * d].rearrange("p (two d) -> p two d", d=d)
    return v[:, :, 0:128]


def wide_pair(wtile, ti, tj, blkw=128):
    """[P, 2, 128] view of two tap blocks in a [P, 9*blkw] weight tile."""
    o0, d = ti * blkw, (tj - ti) * blkw
    v = wtile[:, o0:o0 + 2 * d].rearrange("p (two d) -> p two d", d=d)
    return v[:, :, 0:128]


def build_program():
    nc = bacc.Bacc("TRN2", target_bir_lowering=False, debug=False,
                   enable_asserts=False, num_devices=8)
    io = {}
    io["xs8"] = nc.dram_tensor("xs8", [128, 2 * PXIN], F8,
                               kind="ExternalInput").ap()
    io["ys8"] = nc.dram_tensor("ys8", [128, 2 * PXIN], F8,
                               kind="ExternalInput").ap()
    io["wq8"] = nc.dram_tensor("wq8", [128, 2 * C], F8,
                               kind="ExternalInput").ap()
    io["wk8"] = nc.dram_tensor("wk8", [128, 2 * C], F8,
                               kind="ExternalInput").ap()
    io["xs"] = nc.dram_tensor("xs", [C, PXIN], BF16, kind="ExternalInput").ap()
    io["wv"] = nc.dram_tensor("wv", [C, C], BF16, kind="ExternalInput").ap()
    io["wp1"] = nc.dram_tensor("wp1", [120, C], BF16,
                               kind="ExternalInput").ap()
    io["wp2"] = nc.dram_tensor("wp2", [72, C], BF16,
                               kind="ExternalInput").ap()
    io["dqkd"] = nc.dram_tensor("dqkd", [3 * 128, 9 * 128], F8,
                                kind="ExternalInput").ap()
    io["dvw"] = nc.dram_tensor("dvw", [C, 9], F32, kind="ExternalInput").ap()
    io["tmpq"] = nc.dram_tensor("tmpq", [C, 1], F32, kind="ExternalInput").ap()
    io["em"] = nc.dram_tensor("em", [HEADS, C], BF16, kind="ExternalInput").ap()
    io["eye"] = nc.dram_tensor("eye", [128, 128], F32,
                               kind="ExternalInput").ap()
    io["outp"] = nc.dram_tensor("outp", [C, PXOUT], F32,
                                kind="ExternalOutput").ap()

    with tile.TileContext(nc) as tc, contextlib.ExitStack() as es:
        _emit(nc, tc, io, es)
    nc.compile()
    return nc


def _emit(nc, tc, io, es):
    # ---------------- persistent weights ------------------------------
    wpool = es.enter_context(tc.tile_pool(name="w", bufs=1))
    wq8_t = wpool.tile([128, 2 * C], F8, tag="wq8")
    wk8_t = wpool.tile([128, 2 * C], F8, tag="wk8")
    nc.sync.dma_start(wq8_t[:], io["wq8"][:])
    nc.sync.dma_start(wk8_t[:], io["wk8"][:])
    wv_a = wpool.tile([128, C], BF16, tag="wva")
    wv_b = wpool.tile([64, C], BF16, tag="wvb")
    wp1_t = wpool.tile([120, C], BF16, tag="wp1")
    wp2_t = wpool.tile([72, C], BF16, tag="wp2")
    with tc.tile_wait_until(0.010):
        nc.sync.dma_start(wv_a[:], io["wv"][0:128, :])
        nc.sync.dma_start(wv_b[:], io["wv"][128:192, :])
        nc.sync.dma_start(wp1_t[:], io["wp1"][:])
        nc.sync.dma_start(wp2_t[:], io["wp2"][:])
    dqkd_t = [wpool.tile([128, 9 * 128], F8, tag=f"dqkd{i}",
                         name=f"dqkd{i}") for i in range(3)]
    for i in range(3):
        nc.sync.dma_start(dqkd_t[i][:], io["dqkd"][i * 128:(i + 1) * 128, :])
    dvw1_t = wpool.tile([120, 9], F32, tag="dvw1")
    dvw2_t = wpool.tile([72, 9], F32, tag="dvw2")
    tmpq_t = wpool.tile([128, 2], F32, tag="tmpq")
    em_t = wpool.tile([HEADS, C], BF16, tag="em")
    eye_t = wpool.tile([128, 128], F32, tag="eye")
    with tc.tile_wait_until(0.010):
        nc.sync.dma_start(dvw1_t[:], io["dvw"][0:120, :])
        nc.sync.dma_start(dvw2_t[:], io["dvw"][120:192, :])
        nc.sync.dma_start(tmpq_t[:, 0:1], io["tmpq"][0:128, :])
        nc.sync.dma_start(tmpq_t[0:64, 1:2], io["tmpq"][128:192, :])
        nc.sync.dma_start(em_t[:], io["em"][:])
        nc.sync.dma_start(eye_t[:], io["eye"][:])

    # ---------------- pools -------------------------------------------
    inb = es.enter_context(tc.tile_pool(name="inb", bufs=1))
    tbuf = es.enter_context(tc.tile_pool(name="tbuf", bufs=1))
    dwo = es.enter_context(tc.tile_pool(name="dwo", bufs=1))
    stck = es.enter_context(tc.tile_pool(name="stck", bufs=1))
    small = es.enter_context(tc.tile_pool(name="small", bufs=1))
    outsb = es.enter_context(tc.tile_pool(name="outsb", bufs=2))
    drm = es.enter_context(tc.tile_pool(name="drm", bufs=1, space="DRAM"))

    # pre-zero only the pad columns of each padded buffer slot (cols 0:2
    # and 130:132 of every row; data region is overwritten each stripe;
    # cols 132:144 are never read)
    for b in range(3):
        for _sl in range(2):
            tz = tbuf.tile([128, LT], F8, tag=f"t{b}", name=f"tz{b}_{_sl}",
                           bufs=2)
            tzv = tz[:].rearrange("p (r w) -> p r w", w=WS)
            nc.vector.memset(tzv[:, :, 0:2], 0.0)
            nc.vector.memset(tzv[:, :, 130:132], 0.0)
    tv_tiles = {}
    for _s in range(NS):
        _tva = tbuf.tile([120, LT], BF16, tag="tv_{}".format(_s),
                         name=f"tvz_{_s}", bufs=1)
        _tvb = tbuf.tile([72, LT], BF16, tag="tvb_{}".format(_s),
                         name=f"tvbz_{_s}", bufs=1)
        for _t in (_tva, _tvb):
            _tvv = _t[:].rearrange("p (r w) -> p r w", w=WS)
            nc.vector.memset(_tvv[:, :, 0:2], 0.0)
            nc.vector.memset(_tvv[:, :, 130:132], 0.0)

    xs8v = io["xs8"].rearrange("p (two n) -> p two n", two=2)
    ys8v = io["ys8"].rearrange("p (two n) -> p two n", two=2)
    wq8v = wq8_t[:].rearrange("p (two c) -> p two c", two=2)
    wk8v = wk8_t[:].rearrange("p (two c) -> p two c", two=2)

    # ================= PASS 1 =========================================
    # gram PSUM layout:
    #   gA  = [selfg0 (128) | b1 self (128) | selfg2 (128)]  (diag -> norms)
    #   hAB = per-head q x k blocks at partition base 0:
    #         [h0..h4 (5x24) | h5-main 8 rows (24) | h6 (24) | h7 (24) |
    #          h5-aux 16 rows (24)]
    x_tiles = {}
    with tc.tile_pool(name="cps", bufs=1, space="PSUM") as cpsum, \
         tc.tile_pool(name="gps", bufs=1, space="PSUM") as gpsum:
        gA_ps = gpsum.tile([128, 384], F32, tag="gA")
        hAB_ps = gpsum.tile([24, 216], F32, tag="hAB")

        def _qpieces(h):          # (stk idx, lo, hi, row_off)
            q0 = 24 * h
            if q0 + 24 <= 128:
                return [(0, q0, q0 + 24, 0)]
            if q0 >= 128:
                return [(1, q0 - 128, q0 - 104, 0)]
            return [(0, q0, 128, 0), (1, 0, q0 - 104, 128 - q0)]

        def _kpieces(h):          # (stk idx, lo, hi, col_off)
            k0 = 24 * h
            if k0 + 24 <= 64:
                return [(1, 64 + k0, 88 + k0, 0)]
            if k0 >= 64:
                return [(2, k0 - 64, k0 - 40, 0)]
            return [(1, 64 + k0, 128, 0), (2, 0, k0 - 40, 64 - k0)]

        def _hout(h, roff, rs, coff, cs):
            if h <= 4:
                c0 = 24 * h
            elif h == 5:
                c0 = 120 if roff == 0 else 192
            else:
                c0 = 120 + 24 * (h - 5)
            return hAB_ps[0:rs, c0 + coff:c0 + coff + cs]

        for s in range(NS):
            i0 = s * RS * W
            t_blk = [tbuf.tile([128, LT], F8, tag=f"t{b}",
                               name=f"t{b}_{s}", bufs=2) for b in range(3)]

            xq8 = inb.tile([128, 2 * SPXI], F8, tag="xq8", bufs=2)
            yq8 = inb.tile([128, 2 * SPXI], F8, tag="yq8", bufs=2)
            xa16 = inb.tile([128, SPXI], BF16, tag=f"xa16_{s}", bufs=1)
            xb16 = inb.tile([64, SPXI], BF16, tag=f"xb16_{s}", bufs=1)
            x_tiles[s] = (xa16, xb16)
            nc.gpsimd.dma_start(
                yq8[:].rearrange("p (two n) -> p two n", two=2),
                ys8v[:, :, i0:i0 + SPXI])
            nc.gpsimd.dma_start(
                xq8[:].rearrange("p (two n) -> p two n", two=2),
                xs8v[:, :, i0:i0 + SPXI])
            nc.gpsimd.dma_start(xa16[:], io["xs"][0:128, i0:i0 + SPXI])
            nc.gpsimd.dma_start(xb16[:], io["xs"][128:192, i0:i0 + SPXI])
            xqv = xq8[:].rearrange("p (two n) -> p two n", two=2)
            yqv = yq8[:].rearrange("p (two n) -> p two n", two=2)

            # conv q,k (fp8 DR over the 192-channel contraction), v (bf16)
            # t blocks: [q 0:128] | [q 128:192 ; k 0:64] | [k 64:192]
            tv_a = tbuf.tile([120, LT], BF16, tag="tv_{}".format(s),
                             name=f"tva_{s}", bufs=1)
            tv_b = tbuf.tile([72, LT], BF16, tag="tvb_{}".format(s),
                             name=f"tvb_{s}", bufs=1)
            tv_tiles[s] = (tv_a, tv_b)
            for n0, n in _chunks(SPXI, MMCH):
                r0, nr = n0 // W, n // W
                ps0 = cpsum.tile([128, MMCH], F32, tag="cps0", bufs=2)
                ps1 = cpsum.tile([128, MMCH], F32, tag="cps1", bufs=2)
                psk = cpsum.tile([128, MMCH], F32, tag="cps0", bufs=2,
                                 name=f"psk_{s}_{n0}")
                ps2 = cpsum.tile([128, MMCH], F32, tag="cps1", bufs=2,
                                 name=f"ps2_{s}_{n0}")
                nc.tensor.matmul(ps0[:, 0:n], wq8v[:, :, 0:128],
                                 yqv[:, :, n0:n0 + n], start=True, stop=True,
                                 perf_mode=DR)
                nc.tensor.matmul(ps1[0:64, 0:n], wq8v[:, :, 128:192],
                                 yqv[:, :, n0:n0 + n], start=True, stop=True,
                                 perf_mode=DR)
                nc.tensor.matmul(psk[0:64, 0:n], wk8v[:, :, 0:64],
                                 xqv[:, :, n0:n0 + n], start=True, stop=True,
                                 perf_mode=DR)
                nc.tensor.matmul(ps2[:, 0:n], wk8v[:, :, 64:192],
                                 xqv[:, :, n0:n0 + n], start=True, stop=True,
                                 perf_mode=DR)
                t0d = t_blk[0][:].rearrange("p (r w) -> p r w", w=WS)
                t1d = t_blk[1][:].rearrange("p (r w) -> p r w", w=WS)
                t2d = t_blk[2][:].rearrange("p (r w) -> p r w", w=WS)

                def pw(ps, lo, hi):
                    return ps[lo:hi, 0:n].rearrange("p (r w) -> p r w", w=W)

                nc.scalar.copy(t0d[:, r0:r0 + nr, 2:130], pw(ps0, 0, 128))
                nc.vector.tensor_copy(t1d[0:64, r0:r0 + nr, 2:130],
                                      pw(ps1, 0, 64))
                nc.scalar.copy(t1d[64:128, r0:r0 + nr, 2:130],
                               pw(psk, 0, 64))
                nc.scalar.copy(t2d[:, r0:r0 + nr, 2:130], pw(ps2, 0, 128))
                if s < 2:
                    cv0 = cpsum.tile([128, MMCH], F32, tag="cps0", bufs=2,
                                     name=f"cv0_{s}_{n0}")
                    cv1 = cpsum.tile([128, MMCH], F32, tag="cps1", bufs=2,
                                     name=f"cv1_{s}_{n0}")
                    nc.tensor.matmul(cv0[0:120, 0:n], wv_a[:, 0:120],
                                     xa16[:, n0:n0 + n], start=True,
                                     stop=False)
                    nc.tensor.matmul(cv0[0:120, 0:n], wv_b[:, 0:120],
                                     xb16[:, n0:n0 + n], start=False,
                                     stop=True)
                    nc.tensor.matmul(cv1[0:72, 0:n], wv_a[:, 120:192],
                                     xa16[:, n0:n0 + n], start=True,
                                     stop=False)
                    nc.tensor.matmul(cv1[0:72, 0:n], wv_b[:, 120:192],
                                     xb16[:, n0:n0 + n], start=False,
                                     stop=True)
                    tvad = tv_a[:].rearrange("p (r w) -> p r w", w=WS)
                    tvbd = tv_b[:].rearrange("p (r w) -> p r w", w=WS)
                    nc.scalar.copy(tvad[0:120, r0:r0 + nr, 2:130],
                                   pw(cv0, 0, 120))
                    nc.vector.tensor_copy(tvbd[0:72, r0:r0 + nr, 2:130],
                                          pw(cv1, 0, 72))

            # transposed depthwise on PE (fp8 DR tap pairs) -> px-major
            # stacks, then gram accumulation (fp8 DR chunk pairs)
            nchunk = SPXO // 128
            for g in range(nchunk // 4):
                stk = [stck.tile([128, MMCH], F8, tag=f"stk{b}",
                                 name=f"stk{b}_{s}_{g}", bufs=2)
                       for b in range(3)]
                for b in range(3):
                    tp = cpsum.tile([128, MMCH], F32, tag="tps",
                                    name=f"tp{b}_{s}_{g}", bufs=2)
                    tflat = t_blk[b][:]
                    for ci in range(4):
                        cix = g * 4 + ci
                        q0 = ci * 128
                        for pi, (ti, tj) in enumerate(TAP_PAIRS):
                            nc.tensor.matmul(
                                tp[:, q0:q0 + 128],
                                pair_view(tflat, cix, ti, tj),
                                wide_pair(dqkd_t[b], ti, tj),
                                start=(pi == 0), stop=False, perf_mode=DR)
                        for si, ti in enumerate(TAP_SINGLES):
                            o8 = _tap_off(cix, ti)
                            nc.tensor.matmul(
                                tp[:, q0:q0 + 128], tflat[:, o8:o8 + 128],
                                dqkd_t[b][:, ti * 128:(ti + 1) * 128],
                                start=False,
                                stop=(si == len(TAP_SINGLES) - 1))
                    if b == 1:
                        nc.scalar.copy(stk[b][:], tp[:])
                    else:
                        nc.vector.tensor_copy(stk[b][:], tp[:])
                for p in range(2):
                    cix = g * 4 + 2 * p
                    first = (s == 0 and cix == 0)
                    last = (s == NS - 1 and cix == nchunk - 2)
                    c0 = (2 * p) * 128
                    sp = [stk[b][:, c0:c0 + 256].rearrange(
                        "p (two c) -> p two c", two=2) for b in range(3)]
                    nc.tensor.matmul(gA_ps[:, 0:128], sp[0], sp[0],
                                     start=first, stop=last, perf_mode=DR)
                    nc.tensor.matmul(gA_ps[:, 128:256], sp[1], sp[1],
                                     start=first, stop=last, perf_mode=DR)
                    nc.tensor.matmul(gA_ps[:, 256:384], sp[2], sp[2],
                                     start=first, stop=last, perf_mode=DR)
                    for h in range(HEADS):
                        for (lt, llo, lhi, roff) in _qpieces(h):
                            for (rt, rlo, rhi, coff) in _kpieces(h):
                                nc.tensor.matmul(
                                    _hout(h, roff, lhi - llo, coff,
                                          rhi - rlo),
                                    sp[lt][:, :, llo:lhi],
                                    sp[rt][:, :, rlo:rhi],
                                    start=first, stop=last, perf_mode=DR)

        # ---- norms from self-gram diagonals; per-head blocks -> SBUF
        g0m = small.tile([128, 128], F32, tag="g0m")
        nc.vector.tensor_tensor(g0m[:], gA_ps[:, 0:128], eye_t[:], ALU.mult)
        g1m = small.tile([128, 128], F32, tag="g1m")
        nc.vector.tensor_tensor(g1m[:], gA_ps[:, 128:256], eye_t[:],
                                ALU.mult)
        g2m = small.tile([128, 128], F32, tag="g2m")
        nc.vector.tensor_tensor(g2m[:], gA_ps[:, 256:384], eye_t[:],
                                ALU.mult)
        hAB_sb = small.tile([24, 216], F32, tag="hab")
        nc.scalar.copy(hAB_sb[:], hAB_ps[:])

    # ================= PASS 2 =========================================
    with tc.tile_pool(name="p2ps", bufs=1, space="PSUM") as pps:
        vb_tiles = {}

        def p2_vbdw(s):
            # v[120:192] depthwise on DVE (channel-major), PE does attn only
            tv_b = tv_tiles[s][1]
            tshb = tbuf.tile([72, LT], BF16, tag="tshb", name=f"tshb_{s}")
            nc.vector.tensor_copy(tshb[:, 0:LT - 2], tv_b[:, 1:LT - 1])
            vb = dwo.tile([72, SPXO], BF16, tag=f"vbdw{s % 2}",
                          name=f"vbdw_{s}", bufs=1)
            vb_tiles[s] = vb
            prodb = dwo.tile([72, SPXO], BF16, tag="prodb", name=f"prodb_{s}")
            vbv = vb[:].rearrange("p (r w) -> p r w", w=W)
            prodbv = prodb[:].rearrange("p (r w) -> p r w", w=W)
            for ti, (dy, dx) in enumerate(TAPS):
                sc = dvw2_t[:, ti:ti + 1]
                if dx == 1:
                    s3 = tv_b[:].rearrange("p (r w) -> p r w", w=WS)
                    view = s3[:, dy:dy + RS, 2:130]
                else:
                    s3 = tshb[:].rearrange("p (r w) -> p r w", w=WS)
                    view = s3[:, dy:dy + RS, dx:dx + 128]
                dstv = vbv if ti == 0 else prodbv
                nc.vector.tensor_scalar(dstv, view, sc, None, ALU.mult)
                if ti > 0:
                    nc.vector.tensor_tensor(vb[:], vb[:], prodb[:], ALU.add)

        def p2_attn(s):
            o0 = s * SPXO
            tv_a, tv_b = tv_tiles.pop(s)
            tva3 = tv_a[:].rearrange("p (r w) -> p r w", w=WS)
            vb = vb_tiles.pop(s)
            oa = outsb.tile([128, SPXO], F32, tag="oa", name=f"oa_{s}",
                            bufs=2)
            ob = outsb.tile([64, SPXO], F32, tag="ob", name=f"ob_{s}",
                            bufs=2)
            for n0, n in _chunks(SPXO, MMCH):
                r0, nr = n0 // W, n // W
                ops1 = pps.tile([120, MMCH], F32, tag="ops1", bufs=2)
                ops2 = pps.tile([72, MMCH], F32, tag="ops2", bufs=2)
                for ti, (dy, dx) in enumerate(TAPS):
                    nc.tensor.matmul(
                        ops1[:, 0:n], bd1t[:, ti * 120:(ti + 1) * 120],
                        tva3[0:120, r0 + dy:r0 + dy + nr, 1 + dx:129 + dx],
                        start=(ti == 0), stop=(ti == 8))
                nc.tensor.matmul(ops2[:, 0:n], bd2[:], vb[:, n0:n0 + n],
                                 start=True, stop=True)
                ao1 = dwo.tile([120, MMCH], BF16, tag="ao1", bufs=2)
                ao2 = dwo.tile([72, MMCH], BF16, tag="ao2", bufs=2)
                nc.scalar.copy(ao1[:, 0:n], ops1[:, 0:n])
                nc.scalar.copy(ao2[:, 0:n], ops2[:, 0:n])
                ppa = pps.tile([128, MMCH], F32, tag="ppa", bufs=2)
                ppb = pps.tile([64, MMCH], F32, tag="ppb", bufs=2)
                nc.tensor.matmul(ppa[:, 0:n], wp1_t[:, 0:128], ao1[:, 0:n],
                                 start=True, stop=False)
                nc.tensor.matmul(ppa[:, 0:n], wp2_t[:, 0:128], ao2[:, 0:n],
                                 start=False, stop=True)
                nc.tensor.matmul(ppb[:, 0:n], wp1_t[:, 128:192], ao1[:, 0:n],
                                 start=True, stop=False)
                nc.tensor.matmul(ppb[:, 0:n], wp2_t[:, 128:192], ao2[:, 0:n],
                                 start=False, stop=True)
                nc.scalar.copy(oa[:, n0:n0 + n], ppa[:, 0:n])
                nc.scalar.copy(ob[:, n0:n0 + n], ppb[:, 0:n])
                nc.gpsimd.dma_start(
                    io["outp"][0:128, o0 + n0:o0 + n0 + n], oa[:, n0:n0 + n])
                nc.gpsimd.dma_start(
                    io["outp"][128:192, o0 + n0:o0 + n0 + n],
                    ob[:, n0:n0 + n])

        # deferred conv-v for stripes 2,3: fills PE during the collective
        def p2_conv(s):
            tv_a, tv_b = tv_tiles[s]
            xa16, xb16 = x_tiles.pop(s)
            for n0, n in _chunks(SPXI, MMCH):
                r0, nr = n0 // W, n // W
                cv0 = pps.tile([120, MMCH], F32, tag="ops1", bufs=2,
                               name=f"dcv0_{s}_{n0}")
                cv1 = pps.tile([72, MMCH], F32, tag="ops2", bufs=2,
                               name=f"dcv1_{s}_{n0}")
                nc.tensor.matmul(cv0[:, 0:n], wv_a[:, 0:120],
                                 xa16[:, n0:n0 + n], start=True, stop=False)
                nc.tensor.matmul(cv0[:, 0:n], wv_b[:, 0:120],
                                 xb16[:, n0:n0 + n], start=False, stop=True)
                nc.tensor.matmul(cv1[:, 0:n], wv_a[:, 120:192],
                                 xa16[:, n0:n0 + n], start=True, stop=False)
                nc.tensor.matmul(cv1[:, 0:n], wv_b[:, 120:192],
                                 xb16[:, n0:n0 + n], start=False, stop=True)
                tvad = tv_a[:].rearrange("p (r w) -> p r w", w=WS)
                tvbd = tv_b[:].rearrange("p (r w) -> p r w", w=WS)
                nc.scalar.copy(
                    tvad[0:120, r0:r0 + nr, 2:130],
                    cv0[:, 0:n].rearrange("p (r w) -> p r w", w=W))
                nc.vector.tensor_copy(
                    tvbd[0:72, r0:r0 + nr, 2:130],
                    cv1[:, 0:n].rearrange("p (r w) -> p r w", w=W))

        qn_red = small.tile([128, 3], F32, tag="qnr")
        nc.vector.tensor_reduce(qn_red[:, 0:1], g0m[:], AX.X, ALU.add)
        nc.vector.tensor_reduce(qn_red[:, 1:2], g1m[:], AX.X, ALU.add)
        nc.vector.tensor_reduce(qn_red[:, 2:3], g2m[:], AX.X, ALU.add)
        # norm staging: na = [qn(q0:128) | kn(k0:128)], nb = rows 128:192
        na = small.tile([128, 2], F32, tag="na")
        nb = small.tile([64, 2], F32, tag="nb")
        nc.vector.tensor_copy(na[:, 0:1], qn_red[:, 0:1])
        nc.scalar.copy(na[0:64, 1:2], qn_red[64:128, 1:2])
        nc.scalar.copy(na[64:128, 1:2], qn_red[0:64, 2:3])
        nc.vector.tensor_copy(nb[:, 0:1], qn_red[0:64, 1:2])
        nc.scalar.copy(nb[:, 1:2], qn_red[64:128, 2:3])

        bounce_in = drm.tile([C, 26], F32)
        bounce_out = drm.tile([2 * C, 26], F32)
        nc.sync.dma_start(
            bounce_in[0:120, 0:CH].rearrange("(h c) k -> c h k", c=CH),
            hAB_sb[:, 0:120].rearrange("c (h k) -> c h k", h=5))
        nc.sync.dma_start(bounce_in[120:128, 0:CH], hAB_sb[0:8, 120:144])
        nc.sync.dma_start(bounce_in[128:144, 0:CH], hAB_sb[0:16, 192:216])
        nc.sync.dma_start(
            bounce_in[144:192, 0:CH].rearrange("(h c) k -> c h k", c=CH),
            hAB_sb[:, 144:192].rearrange("c (h k) -> c h k", h=2))
        nc.sync.dma_start(bounce_in[0:128, 24:26], na[:])
        nc.sync.dma_start(bounce_in[128:192, 24:26], nb[:])

        nc.gpsimd.collective_compute(
            "AllGather", ALU.bypass,
            replica_groups=[[0, 1], [2, 3], [4, 5], [6, 7]],
            ins=[bounce_in[:].opt()], outs=[bounce_out[:].opt()])

        with tc.tile_wait_until(0.090):
            p2_conv(2)
            p2_conv(3)
        p2_vbdw(0)
        p2_vbdw(1)
        p2_vbdw(2)
        p2_vbdw(3)

        # one DMA pulls all 384 gathered rows as [128, 3, 26]; the local
        # add then combines row j*128+p blocks (64-aligned cross-base)
        cmp3 = small.tile([128, 3 * 26], F32, tag="cmp3")
        nc.sync.dma_start(
            cmp3[:].rearrange("p (j k) -> p j k", j=3),
            bounce_out[:].rearrange("(j p) k -> p j k", j=3))
        c3v = cmp3[:].rearrange("p (j k) -> p j k", j=3)
        cmp_a = small.tile([128, 26], F32, tag="cmpa")
        cmp_b = small.tile([64, 26], F32, tag="cmpb")
        tmp_ab = small.tile([128, 26], F32, tag="cmptmp")
        nc.vector.tensor_copy(tmp_ab[0:64, :], c3v[64:128, 1, :])
        nc.vector.tensor_copy(tmp_ab[64:128, :], c3v[0:64, 2, :])
        nc.vector.tensor_tensor(cmp_a[:], c3v[:, 0, :], tmp_ab[:], ALU.add)
        tmp_b = small.tile([64, 26], F32, tag="cmptmpb")
        nc.scalar.copy(tmp_b[:], c3v[64:128, 2, :])
        nc.vector.tensor_tensor(cmp_b[:], c3v[0:64, 1, :], tmp_b[:],
                                ALU.add)

        kn8 = small.tile([HEADS, CH], F32, tag="kn8")
        kn8b = small.tile([HEADS, CH], F32, tag="kn8x")
        nc.gpsimd.dma_start(
            kn8[:],
            bounce_out[0:C, :].rearrange("(h c) k -> h c k", c=CH)[:, :, 25])
        nc.gpsimd.dma_start(
            kn8b[:],
            bounce_out[C:2 * C, :].rearrange("(h c) k -> h c k",
                                             c=CH)[:, :, 25])
        nc.vector.tensor_tensor(kn8[:], kn8[:], kn8b[:], ALU.add)

        # rq = temp/sqrt(qn); rk = 1/sqrt(kn) as [8,24]
        rq_a = small.tile([128, 3], F32, tag="rqa")
        rq_b = small.tile([64, 3], F32, tag="rqb")
        for ti, (cmp, rq, nrow) in enumerate(((cmp_a, rq_a, 128),
                                              (cmp_b, rq_b, 64))):
            nc.scalar.activation(rq[:, 0:1], cmp[:, 24:25], ACTF.Sqrt)
            nc.vector.reciprocal(rq[:, 1:2], rq[:, 0:1])
            nc.vector.tensor_scalar(rq[:, 2:3], rq[:, 1:2],
                                    tmpq_t[0:nrow, ti:ti + 1], None, ALU.mult)
        rk8 = small.tile([HEADS, 2 * CH], F32, tag="rk8")
        nc.scalar.activation(rk8[:, 0:CH], kn8[:], ACTF.Sqrt)
        nc.vector.reciprocal(rk8[:, CH:2 * CH], rk8[:, 0:CH])
        rk8b = small.tile([HEADS, CH], BF16, tag="rk8b")
        nc.vector.tensor_copy(rk8b[:], rk8[:, CH:2 * CH])

        knb_a = small.tile([128, CH], F32, tag="knba")
        knb_b = small.tile([64, CH], F32, tag="knbb")
        knb_ps = pps.tile([128, CH], F32, tag="ppa", name="knb_ps", bufs=2)
        nc.tensor.matmul(knb_ps[:], em_t[:, 0:128], rk8b[:],
                         start=True, stop=True)
        nc.scalar.copy(knb_a[:], knb_ps[:])
        knb_ps2 = pps.tile([128, CH], F32, tag="ppa", name="knb_ps2", bufs=2)
        nc.tensor.matmul(knb_ps2[0:64, :], em_t[:, 128:192], rk8b[:],
                         start=True, stop=True)
        nc.scalar.copy(knb_b[:], knb_ps2[0:64, :])

        # s = A*rq*knb ; softmax over d (free dim)
        attn16 = small.tile([128, CH], BF16, tag="att16a")
        attn16b = small.tile([64, CH], BF16, tag="att16b")
        for cmp, rq, knb, a16, nrow in ((cmp_a, rq_a, knb_a, attn16, 128),
                                        (cmp_b, rq_b, knb_b, attn16b, 64)):
            at = small.tile([128, CH], F32, tag="atf")
            sm = small.tile([128, 4], F32, tag="sm")
            nc.vector.tensor_scalar(at[0:nrow, :], cmp[0:nrow, 0:CH], rq[:, 2:3],
                                    None, ALU.mult)
            nc.vector.tensor_tensor(at[0:nrow, :], at[0:nrow, :], knb[:],
                                    ALU.mult)
            nc.scalar.activation(at[0:nrow, :], at[0:nrow, :], ACTF.Exp)
            nc.vector.tensor_reduce(sm[0:nrow, 1:2], at[0:nrow, :], AX.X, ALU.add)
            nc.vector.reciprocal(sm[0:nrow, 2:3], sm[0:nrow, 1:2])
            nc.vector.tensor_scalar(a16[0:nrow, :], at[0:nrow, :],
                                    sm[0:nrow, 2:3], None, ALU.mult)

        # block-diag attn^T via DRAM round-trip (transposing strided DMAs),
        # split across the HWDGE (sync) and SWDGE (gpsimd) queues
        attn_d = drm.tile([C, CH], BF16)
        nc.sync.dma_start(attn_d[0:128, :], attn16[:])
        nc.gpsimd.dma_start(attn_d[128:192, :], attn16b[:])
        bd1 = small.tile([120, 120], BF16, tag="bd1")      # heads 0-4 attn^T
        bd2 = small.tile([72, 72], BF16, tag="bd2")        # heads 5-7 attn^T
        nc.vector.memset(bd1[:], 0.0)
        nc.vector.memset(bd2[:], 0.0)
        for h in range(5):
            r0 = h * CH
            nc.sync.dma_start(
                bd1[r0:r0 + CH, r0:r0 + CH],
                attn_d[r0:r0 + CH, :].rearrange("c d -> d c"))
        for h in range(5, 8):
            r0 = (h - 5) * CH
            nc.gpsimd.dma_start(
                bd2[r0:r0 + CH, r0:r0 + CH],
                attn_d[h * CH:(h + 1) * CH, :].rearrange("c d -> d c"))
        # fold depthwise-v tap weights into attn^T: bd1_tap = bd1 * w_v[d,t]
        bd1t = small.tile([120, 9 * 120], BF16, tag="bd1t")
        for ti in range(9):
            nc.vector.tensor_scalar(bd1t[:, ti * 120:(ti + 1) * 120], bd1[:],
                                    dvw1_t[:, ti:ti + 1], None, ALU.mult)

        p2_attn(0)
        p2_attn(1)
        p2_attn(2)
        p2_attn(3)


# ======================================================================
def _prep_inputs(x, y, qkv_w, dw_w, proj_w, temperature):
    wq_t = np.ascontiguousarray(qkv_w[0:C].T)          # [in, out]
    wk_t = np.ascontiguousarray(qkv_w[C:2 * C].T)
    wv_t = np.ascontiguousarray(qkv_w[2 * C:3 * C].T)
    wp_t = np.ascontiguousarray(proj_w.T)

    def planes2(w):
        out = np.zeros((128, 2, C), np.float32)
        out[:, 0, :] = w[0:128]
        out[0:64, 1, :] = w[128:192]
        return out.reshape(128, 2 * C).astype(F8NP)

    wq8, wk8 = planes2(wq_t), planes2(wk_t)
    wv16 = wv_t.astype(BF16NP)
    wp1 = wp_t[0:120].astype(BF16NP)
    wp2 = wp_t[120:192].astype(BF16NP)

    dw = dw_w.reshape(3 * C, 9).astype(np.float32)
    dw_q, dw_k, dw_v = dw[0:C], dw[C:2 * C], dw[2 * C:3 * C]
    dqk = np.concatenate([dw_q[0:128], dw_q[128:192], dw_k[0:64],
                          dw_k[64:192]], axis=0)
    dqkd = np.zeros((3 * 128, 9 * 128), np.float32)
    for i in range(3):
        for t in range(9):
            blk = dqk[i * 128:(i + 1) * 128, t]
            np.fill_diagonal(
                dqkd[i * 128:(i + 1) * 128, t * 128:(t + 1) * 128], blk)
    tmpq = np.repeat(np.asarray(temperature, np.float32).reshape(HEADS),
                     CH).reshape(C, 1)
    em = np.zeros((HEADS, C), np.float32)
    for hh in range(HEADS):
        em[hh, hh * CH:(hh + 1) * CH] = 1.0

    in_maps = []
    for core in range(8):
        bi, half = core // 2, core % 2
        r0 = half * HOUT - 1
        xsl = np.zeros((C, HIN, W), np.float32)
        ysl = np.zeros((C, HIN, W), np.float32)
        lo, hi = max(r0, 0), min(r0 + HIN, 128)
        xsl[:, lo - r0:hi - r0] = x[bi, :, lo:hi]
        ysl[:, lo - r0:hi - r0] = y[bi, :, lo:hi]
        xsl = xsl.reshape(C, PXIN)
        ysl = ysl.reshape(C, PXIN)

        def planes_px(t):
            out = np.zeros((128, 2, PXIN), np.float32)
            out[:, 0, :] = t[0:128]
            out[0:64, 1, :] = t[128:192]
            return out.reshape(128, 2 * PXIN).astype(F8NP)

        in_maps.append({
            "xs": xsl.astype(BF16NP),
            "xs8": planes_px(xsl), "ys8": planes_px(ysl),
            "wq8": wq8, "wk8": wk8, "wv": wv16,
            "wp1": wp1, "wp2": wp2,
            "dqkd": dqkd.astype(F8NP),
            "dvw": dw_v.astype(np.float32),
            "tmpq": tmpq, "em": em.astype(BF16NP),
            "eye": np.eye(128, dtype=np.float32),
        })
    return in_maps


def kernel(x, y, qkv_w, dw_w, proj_w, temperature, _trace=False):
    x = np.asarray(x, np.float32)
    y = np.asarray(y, np.float32)
    if "nc" not in _CACHE:
        _CACHE["nc"] = build_program()
    nc = _CACHE["nc"]
    in_maps = _prep_inputs(x, y, np.asarray(qkv_w, np.float32),
                           np.asarray(dw_w, np.float32),
                           np.asarray(proj_w, np.float32),
                           np.asarray(temperature, np.float32))
    res = bass_utils.run_bass_kernel_spmd(nc, in_maps,
                                          core_ids=list(range(8)),
                                          trace=_trace)
    _CACHE["last_result"] = res
    out = np.empty((4, C, 128, W), np.float32)
    for core in range(8):
        bi, half = core // 2, core % 2
        out[bi, :, half * HOUT:(half + 1) * HOUT] = \
            res.results[core]["outp"].reshape(C, HOUT, W)
    return out


# revision 91
# speedup vs baseline: 1.6932x; 1.0128x over previous
"""Trainium2 Bass kernel for nn_Attention (channel attention, XCA-style).

Sharding: 8 cores = (batch b=core//2) x (image half = core%2, 64 rows + halo).
Cross-core: AllGather of tiny gram stats over core pairs + local add.

All heavy matmuls run in fp8e4m3 with DoubleRow (2 contraction planes per
instruction, 0.5 cycles/row): conv q,k,v (channel planes), transposed
depthwise (vertical tap pairs, 16-aligned via WS=144), gram (pixel-chunk
pairs), attn 9-tap folds (tap pairs), proj (attn-channel planes).
"""

import sys
import numpy as np

sys.path.insert(0, "/opt/trn_rl_repo")

import contextlib  # noqa: E402

import ml_dtypes  # noqa: E402

from concourse import bass, bacc, tile, mybir  # noqa: E402
from concourse import bass_utils  # noqa: E402

F32 = mybir.dt.float32
BF16 = mybir.dt.bfloat16
F8 = mybir.dt.float8e4
ALU = mybir.AluOpType
ACTF = mybir.ActivationFunctionType
AX = mybir.AxisListType
DR = mybir.MatmulPerfMode.DoubleRow
BF16NP = ml_dtypes.bfloat16
F8NP = ml_dtypes.float8_e4m3

C = 192
HEADS = 8
CH = 24
W = 128
HOUT = 64
HIN = HOUT + 2
WS = 144                  # padded row stride (16-aligned for DoubleRow)
PXIN = HIN * W            # 8448
PXOUT = HOUT * W          # 8192

RS = 16                   # stripe out-rows
NS = HOUT // RS
RIN = RS + 2
SPXI = RIN * W            # 2304
SPXO = RS * W             # 2048
LT = RIN * WS             # padded buffer length (2592)
MMCH = 512

TAPS = [(dy, dx) for dy in (0, 1, 2) for dx in (0, 1, 2)]
# DoubleRow tap pairs must have 16-aligned flat-offset delta: vertical
# pairs (same dx) have delta WS=144. Taps 6,7,8 run as single fp8 MMs.
TAP_PAIRS = [(0, 3), (1, 4), (2, 5)]
TAP_SINGLES = [6, 7, 8]

_CACHE = {}


def _chunks(total, step):
    out, s = [], 0
    while s < total:
        out.append((s, min(step, total - s)))
        s += step
    return out


def _tap_off(cix, ti):
    dy, dx = TAPS[ti]
    return (cix + dy) * WS + 1 + dx


def pair_view(flat, cix, ti, tj):
    """[P, 2, 128] view of two tap windows (plane stride = o1-o0)."""
    o0, o1 = _tap_off(cix, ti), _tap_off(cix, tj)
    d = o1 - o0
    v = flat[:, o0:o0 + 2 * d].rearrange("p (two d) -> p two d", d=d)
    return v[:, :, 0:128]


def wide_pair(wtile, ti, tj, blkw=128):
    """[P, 2, 128] view of two tap blocks in a [P, 9*blkw] weight tile."""
    o0, d = ti * blkw, (tj - ti) * blkw
    v = wtile[:, o0:o0 + 2 * d].rearrange("p (two d) -> p two d", d=d)
    return v[:, :, 0:128]


def build_program():
    nc = bacc.Bacc("TRN2", target_bir_lowering=False, debug=False,
                   enable_asserts=False, num_devices=8)
    io = {}
    io["xs8"] = nc.dram_tensor("xs8", [128, 2 * PXIN], F8,
                               kind="ExternalInput").ap()
    io["ys8"] = nc.dram_tensor("ys8", [128, 2 * PXIN], F8,
                               kind="ExternalInput").ap()
    io["wq8"] = nc.dram_tensor("wq8", [128, 2 * C], F8,
                               kind="ExternalInput").ap()
    io["wk8"] = nc.dram_tensor("wk8", [128, 2 * C], F8,
                               kind="ExternalInput").ap()
    io["xs"] = nc.dram_tensor("xs", [C, PXIN], BF16, kind="ExternalInput").ap()
    io["wv"] = nc.dram_tensor("wv", [C, C], BF16, kind="ExternalInput").ap()
    io["wp1"] = nc.dram_tensor("wp1", [120, C], BF16,
                               kind="ExternalInput").ap()
    io["wp2"] = nc.dram_tensor("wp2", [72, C], BF16,
                               kind="ExternalInput").ap()
    io["dqkd"] = nc.dram_tensor("dqkd", [3 * 128, 9 * 128], F8,
                                kind="ExternalInput").ap()
    io["dvw"] = nc.dram_tensor("dvw", [C, 9], F32, kind="ExternalInput").ap()
    io["tmpq"] = nc.dram_tensor("tmpq", [C, 1], F32, kind="ExternalInput").ap()
    io["em"] = nc.dram_tensor("em", [HEADS, C], F32, kind="ExternalInput").ap()
    io["eye"] = nc.dram_tensor("eye", [128, 128], F32,
                               kind="ExternalInput").ap()
    io["outp"] = nc.dram_tensor("outp", [C, PXOUT], F32,
                                kind="ExternalOutput").ap()

    with tile.TileContext(nc) as tc, contextlib.ExitStack() as es:
        _emit(nc, tc, io, es)
    nc.compile()
    return nc


def _emit(nc, tc, io, es):
    # ---------------- persistent weights ------------------------------
    wpool = es.enter_context(tc.tile_pool(name="w", bufs=1))
    wq8_t = wpool.tile([128, 2 * C], F8, tag="wq8")
    wk8_t = wpool.tile([128, 2 * C], F8, tag="wk8")
    nc.sync.dma_start(wq8_t[:], io["wq8"][:])
    nc.sync.dma_start(wk8_t[:], io["wk8"][:])
    wv_a = wpool.tile([128, C], BF16, tag="wva")
    wv_b = wpool.tile([64, C], BF16, tag="wvb")
    wp1_t = wpool.tile([120, C], BF16, tag="wp1")
    wp2_t = wpool.tile([72, C], BF16, tag="wp2")
    with tc.tile_wait_until(0.010):
        nc.sync.dma_start(wv_a[:], io["wv"][0:128, :])
        nc.sync.dma_start(wv_b[:], io["wv"][128:192, :])
        nc.sync.dma_start(wp1_t[:], io["wp1"][:])
        nc.sync.dma_start(wp2_t[:], io["wp2"][:])
    dqkd_t = [wpool.tile([128, 9 * 128], F8, tag=f"dqkd{i}",
                         name=f"dqkd{i}") for i in range(3)]
    for i in range(3):
        nc.sync.dma_start(dqkd_t[i][:], io["dqkd"][i * 128:(i + 1) * 128, :])
    dvw1_t = wpool.tile([120, 9], F32, tag="dvw1")
    dvw2_t = wpool.tile([72, 9], F32, tag="dvw2")
    tmpq_t = wpool.tile([128, 2], F32, tag="tmpq")
    em_t = wpool.tile([HEADS, C], F32, tag="em")
    eye_t = wpool.tile([128, 128], F32, tag="eye")
    with tc.tile_wait_until(0.010):
        nc.sync.dma_start(dvw1_t[:], io["dvw"][0:120, :])
        nc.sync.dma_start(dvw2_t[:], io["dvw"][120:192, :])
        nc.sync.dma_start(tmpq_t[:, 0:1], io["tmpq"][0:128, :])
        nc.sync.dma_start(tmpq_t[0:64, 1:2], io["tmpq"][128:192, :])
        nc.sync.dma_start(em_t[:], io["em"][:])
        nc.sync.dma_start(eye_t[:], io["eye"][:])

    # ---------------- pools -------------------------------------------
    inb = es.enter_context(tc.tile_pool(name="inb", bufs=1))
    tbuf = es.enter_context(tc.tile_pool(name="tbuf", bufs=1))
    dwo = es.enter_context(tc.tile_pool(name="dwo", bufs=1))
    stck = es.enter_context(tc.tile_pool(name="stck", bufs=1))
    small = es.enter_context(tc.tile_pool(name="small", bufs=1))
    outsb = es.enter_context(tc.tile_pool(name="outsb", bufs=2))
    drm = es.enter_context(tc.tile_pool(name="drm", bufs=1, space="DRAM"))

    # pre-zero only the pad columns of each padded buffer slot (cols 0:2
    # and 130:132 of every row; data region is overwritten each stripe;
    # cols 132:144 are never read)
    for b in range(3):
        for _sl in range(2):
            tz = tbuf.tile([128, LT], F8, tag=f"t{b}", name=f"tz{b}_{_sl}",
                           bufs=2)
            tzv = tz[:].rearrange("p (r w) -> p r w", w=WS)
            nc.vector.memset(tzv[:, :, 0:2], 0.0)
            nc.vector.memset(tzv[:, :, 130:132], 0.0)
    tv_tiles = {}
    for _s in range(NS):
        _tva = tbuf.tile([120, LT], BF16, tag="tv_{}".format(_s),
                         name=f"tvz_{_s}", bufs=1)
        _tvb = tbuf.tile([72, LT], BF16, tag="tvb_{}".format(_s),
                         name=f"tvbz_{_s}", bufs=1)
        for _t in (_tva, _tvb):
            _tvv = _t[:].rearrange("p (r w) -> p r w", w=WS)
            nc.vector.memset(_tvv[:, :, 0:2], 0.0)
            nc.vector.memset(_tvv[:, :, 130:132], 0.0)

    xs8v = io["xs8"].rearrange("p (two n) -> p two n", two=2)
    ys8v = io["ys8"].rearrange("p (two n) -> p two n", two=2)
    wq8v = wq8_t[:].rearrange("p (two c) -> p two c", two=2)
    wk8v = wk8_t[:].rearrange("p (two c) -> p two c", two=2)

    # ================= PASS 1 =========================================
    # gram PSUM layout:
    #   gA  = [selfg0 (128) | b1 self (128) | selfg2 (128)]  (diag -> norms)
    #   hAB = per-head q x k blocks at partition base 0:
    #         [h0..h4 (5x24) | h5-main 8 rows (24) | h6 (24) | h7 (24) |
    #          h5-aux 16 rows (24)]
    x_tiles = {}
    with tc.tile_pool(name="cps", bufs=1, space="PSUM") as cpsum, \
         tc.tile_pool(name="gps", bufs=1, space="PSUM") as gpsum:
        gA_ps = gpsum.tile([128, 384], F32, tag="gA")
        hAB_ps = gpsum.tile([24, 216], F32, tag="hAB")

        def _qpieces(h):          # (stk idx, lo, hi, row_off)
            q0 = 24 * h
            if q0 + 24 <= 128:
                return [(0, q0, q0 + 24, 0)]
            if q0 >= 128:
                return [(1, q0 - 128, q0 - 104, 0)]
            return [(0, q0, 128, 0), (1, 0, q0 - 104, 128 - q0)]

        def _kpieces(h):          # (stk idx, lo, hi, col_off)
            k0 = 24 * h
            if k0 + 24 <= 64:
                return [(1, 64 + k0, 88 + k0, 0)]
            if k0 >= 64:
                return [(2, k0 - 64, k0 - 40, 0)]
            return [(1, 64 + k0, 128, 0), (2, 0, k0 - 40, 64 - k0)]

        def _hout(h, roff, rs, coff, cs):
            if h <= 4:
                c0 = 24 * h
            elif h == 5:
                c0 = 120 if roff == 0 else 192
            else:
                c0 = 120 + 24 * (h - 5)
            return hAB_ps[0:rs, c0 + coff:c0 + coff + cs]

        for s in range(NS):
            i0 = s * RS * W
            t_blk = [tbuf.tile([128, LT], F8, tag=f"t{b}",
                               name=f"t{b}_{s}", bufs=2) for b in range(3)]

            xq8 = inb.tile([128, 2 * SPXI], F8, tag="xq8", bufs=2)
            yq8 = inb.tile([128, 2 * SPXI], F8, tag="yq8", bufs=2)
            xa16 = inb.tile([128, SPXI], BF16, tag=f"xa16_{s}", bufs=1)
            xb16 = inb.tile([64, SPXI], BF16, tag=f"xb16_{s}", bufs=1)
            x_tiles[s] = (xa16, xb16)
            nc.gpsimd.dma_start(
                yq8[:].rearrange("p (two n) -> p two n", two=2),
                ys8v[:, :, i0:i0 + SPXI])
            nc.gpsimd.dma_start(
                xq8[:].rearrange("p (two n) -> p two n", two=2),
                xs8v[:, :, i0:i0 + SPXI])
            nc.gpsimd.dma_start(xa16[:], io["xs"][0:128, i0:i0 + SPXI])
            nc.gpsimd.dma_start(xb16[:], io["xs"][128:192, i0:i0 + SPXI])
            xqv = xq8[:].rearrange("p (two n) -> p two n", two=2)
            yqv = yq8[:].rearrange("p (two n) -> p two n", two=2)

            # conv q,k (fp8 DR over the 192-channel contraction), v (bf16)
            # t blocks: [q 0:128] | [q 128:192 ; k 0:64] | [k 64:192]
            tv_a = tbuf.tile([120, LT], BF16, tag="tv_{}".format(s),
                             name=f"tva_{s}", bufs=1)
            tv_b = tbuf.tile([72, LT], BF16, tag="tvb_{}".format(s),
                             name=f"tvb_{s}", bufs=1)
            tv_tiles[s] = (tv_a, tv_b)
            for n0, n in _chunks(SPXI, MMCH):
                r0, nr = n0 // W, n // W
                ps0 = cpsum.tile([128, MMCH], F32, tag="cps0", bufs=2)
                ps1 = cpsum.tile([128, MMCH], F32, tag="cps1", bufs=2)
                psk = cpsum.tile([128, MMCH], F32, tag="cps0", bufs=2,
                                 name=f"psk_{s}_{n0}")
                ps2 = cpsum.tile([128, MMCH], F32, tag="cps1", bufs=2,
                                 name=f"ps2_{s}_{n0}")
                nc.tensor.matmul(ps0[:, 0:n], wq8v[:, :, 0:128],
                                 yqv[:, :, n0:n0 + n], start=True, stop=True,
                                 perf_mode=DR)
                nc.tensor.matmul(ps1[0:64, 0:n], wq8v[:, :, 128:192],
                                 yqv[:, :, n0:n0 + n], start=True, stop=True,
                                 perf_mode=DR)
                nc.tensor.matmul(psk[0:64, 0:n], wk8v[:, :, 0:64],
                                 xqv[:, :, n0:n0 + n], start=True, stop=True,
                                 perf_mode=DR)
                nc.tensor.matmul(ps2[:, 0:n], wk8v[:, :, 64:192],
                                 xqv[:, :, n0:n0 + n], start=True, stop=True,
                                 perf_mode=DR)
                t0d = t_blk[0][:].rearrange("p (r w) -> p r w", w=WS)
                t1d = t_blk[1][:].rearrange("p (r w) -> p r w", w=WS)
                t2d = t_blk[2][:].rearrange("p (r w) -> p r w", w=WS)

                def pw(ps, lo, hi):
                    return ps[lo:hi, 0:n].rearrange("p (r w) -> p r w", w=W)

                nc.scalar.copy(t0d[:, r0:r0 + nr, 2:130], pw(ps0, 0, 128))
                nc.vector.tensor_copy(t1d[0:64, r0:r0 + nr, 2:130],
                                      pw(ps1, 0, 64))
                nc.scalar.copy(t1d[64:128, r0:r0 + nr, 2:130],
                               pw(psk, 0, 64))
                nc.scalar.copy(t2d[:, r0:r0 + nr, 2:130], pw(ps2, 0, 128))
                if s < 2:
                    cv0 = cpsum.tile([128, MMCH], F32, tag="cps0", bufs=2,
                                     name=f"cv0_{s}_{n0}")
                    cv1 = cpsum.tile([128, MMCH], F32, tag="cps1", bufs=2,
                                     name=f"cv1_{s}_{n0}")
                    nc.tensor.matmul(cv0[0:120, 0:n], wv_a[:, 0:120],
                                     xa16[:, n0:n0 + n], start=True,
                                     stop=False)
                    nc.tensor.matmul(cv0[0:120, 0:n], wv_b[:, 0:120],
                                     xb16[:, n0:n0 + n], start=False,
                                     stop=True)
                    nc.tensor.matmul(cv1[0:72, 0:n], wv_a[:, 120:192],
                                     xa16[:, n0:n0 + n], start=True,
                                     stop=False)
                    nc.tensor.matmul(cv1[0:72, 0:n], wv_b[:, 120:192],
                                     xb16[:, n0:n0 + n], start=False,
                                     stop=True)
                    tvad = tv_a[:].rearrange("p (r w) -> p r w", w=WS)
                    tvbd = tv_b[:].rearrange("p (r w) -> p r w", w=WS)
                    nc.scalar.copy(tvad[0:120, r0:r0 + nr, 2:130],
                                   pw(cv0, 0, 120))
                    nc.vector.tensor_copy(tvbd[0:72, r0:r0 + nr, 2:130],
                                          pw(cv1, 0, 72))

            # transposed depthwise on PE (fp8 DR tap pairs) -> px-major
            # stacks, then gram accumulation (fp8 DR chunk pairs)
            nchunk = SPXO // 128
            for g in range(nchunk // 4):
                stk = [stck.tile([128, MMCH], F8, tag=f"stk{b}",
                                 name=f"stk{b}_{s}_{g}", bufs=2)
                       for b in range(3)]
                for b in range(3):
                    tp = cpsum.tile([128, MMCH], F32, tag="tps",
                                    name=f"tp{b}_{s}_{g}", bufs=2)
                    tflat = t_blk[b][:]
                    for ci in range(4):
                        cix = g * 4 + ci
                        q0 = ci * 128
                        for pi, (ti, tj) in enumerate(TAP_PAIRS):
                            nc.tensor.matmul(
                                tp[:, q0:q0 + 128],
                                pair_view(tflat, cix, ti, tj),
                                wide_pair(dqkd_t[b], ti, tj),
                                start=(pi == 0), stop=False, perf_mode=DR)
                        for si, ti in enumerate(TAP_SINGLES):
                            o8 = _tap_off(cix, ti)
                            nc.tensor.matmul(
                                tp[:, q0:q0 + 128], tflat[:, o8:o8 + 128],
                                dqkd_t[b][:, ti * 128:(ti + 1) * 128],
                                start=False,
                                stop=(si == len(TAP_SINGLES) - 1))
                    if b == 1:
                        nc.scalar.copy(stk[b][:], tp[:])
                    else:
                        nc.vector.tensor_copy(stk[b][:], tp[:])
                for p in range(2):
                    cix = g * 4 + 2 * p
                    first = (s == 0 and cix == 0)
                    last = (s == NS - 1 and cix == nchunk - 2)
                    c0 = (2 * p) * 128
                    sp = [stk[b][:, c0:c0 + 256].rearrange(
                        "p (two c) -> p two c", two=2) for b in range(3)]
                    nc.tensor.matmul(gA_ps[:, 0:128], sp[0], sp[0],
                                     start=first, stop=last, perf_mode=DR)
                    nc.tensor.matmul(gA_ps[:, 128:256], sp[1], sp[1],
                                     start=first, stop=last, perf_mode=DR)
                    nc.tensor.matmul(gA_ps[:, 256:384], sp[2], sp[2],
                                     start=first, stop=last, perf_mode=DR)
                    for h in range(HEADS):
                        for (lt, llo, lhi, roff) in _qpieces(h):
                            for (rt, rlo, rhi, coff) in _kpieces(h):
                                nc.tensor.matmul(
                                    _hout(h, roff, lhi - llo, coff,
                                          rhi - rlo),
                                    sp[lt][:, :, llo:lhi],
                                    sp[rt][:, :, rlo:rhi],
                                    start=first, stop=last, perf_mode=DR)

        # ---- norms from self-gram diagonals; per-head blocks -> SBUF
        g0m = small.tile([128, 128], F32, tag="g0m")
        nc.vector.tensor_tensor(g0m[:], gA_ps[:, 0:128], eye_t[:], ALU.mult)
        g1m = small.tile([128, 128], F32, tag="g1m")
        nc.vector.tensor_tensor(g1m[:], gA_ps[:, 128:256], eye_t[:],
                                ALU.mult)
        g2m = small.tile([128, 128], F32, tag="g2m")
        nc.vector.tensor_tensor(g2m[:], gA_ps[:, 256:384], eye_t[:],
                                ALU.mult)
        hAB_sb = small.tile([24, 216], F32, tag="hab")
        nc.scalar.copy(hAB_sb[:], hAB_ps[:])

    # ================= PASS 2 =========================================
    with tc.tile_pool(name="p2ps", bufs=1, space="PSUM") as pps:
        vb_tiles = {}

        def p2_vbdw(s):
            # v[120:192] depthwise on DVE (channel-major), PE does attn only
            tv_b = tv_tiles[s][1]
            tshb = tbuf.tile([72, LT], BF16, tag="tshb", name=f"tshb_{s}")
            nc.vector.tensor_copy(tshb[:, 0:LT - 2], tv_b[:, 1:LT - 1])
            vb = dwo.tile([72, SPXO], BF16, tag=f"vbdw{s % 2}",
                          name=f"vbdw_{s}", bufs=1)
            vb_tiles[s] = vb
            prodb = dwo.tile([72, SPXO], BF16, tag="prodb", name=f"prodb_{s}")
            vbv = vb[:].rearrange("p (r w) -> p r w", w=W)
            prodbv = prodb[:].rearrange("p (r w) -> p r w", w=W)
            for ti, (dy, dx) in enumerate(TAPS):
                sc = dvw2_t[:, ti:ti + 1]
                if dx == 1:
                    s3 = tv_b[:].rearrange("p (r w) -> p r w", w=WS)
                    view = s3[:, dy:dy + RS, 2:130]
                else:
                    s3 = tshb[:].rearrange("p (r w) -> p r w", w=WS)
                    view = s3[:, dy:dy + RS, dx:dx + 128]
                dstv = vbv if ti == 0 else prodbv
                nc.vector.tensor_scalar(dstv, view, sc, None, ALU.mult)
                if ti > 0:
                    nc.vector.tensor_tensor(vb[:], vb[:], prodb[:], ALU.add)

        def p2_attn(s):
            o0 = s * SPXO
            tv_a, tv_b = tv_tiles.pop(s)
            tva3 = tv_a[:].rearrange("p (r w) -> p r w", w=WS)
            vb = vb_tiles.pop(s)
            oa = outsb.tile([128, SPXO], F32, tag="oa", name=f"oa_{s}",
                            bufs=2)
            ob = outsb.tile([64, SPXO], F32, tag="ob", name=f"ob_{s}",
                            bufs=2)
            for n0, n in _chunks(SPXO, MMCH):
                r0, nr = n0 // W, n // W
                ops1 = pps.tile([120, MMCH], F32, tag="ops1", bufs=2)
                ops2 = pps.tile([72, MMCH], F32, tag="ops2", bufs=2)
                for ti, (dy, dx) in enumerate(TAPS):
                    nc.tensor.matmul(
                        ops1[:, 0:n], bd1t[:, ti * 120:(ti + 1) * 120],
                        tva3[0:120, r0 + dy:r0 + dy + nr, 1 + dx:129 + dx],
                        start=(ti == 0), stop=(ti == 8))
                nc.tensor.matmul(ops2[:, 0:n], bd2[:], vb[:, n0:n0 + n],
                                 start=True, stop=True)
                ao1 = dwo.tile([120, MMCH], BF16, tag="ao1", bufs=2)
                ao2 = dwo.tile([72, MMCH], BF16, tag="ao2", bufs=2)
                nc.scalar.copy(ao1[:, 0:n], ops1[:, 0:n])
                nc.scalar.copy(ao2[:, 0:n], ops2[:, 0:n])
                ppa = pps.tile([128, MMCH], F32, tag="ppa", bufs=2)
                ppb = pps.tile([64, MMCH], F32, tag="ppb", bufs=2)
                nc.tensor.matmul(ppa[:, 0:n], wp1_t[:, 0:128], ao1[:, 0:n],
                                 start=True, stop=False)
                nc.tensor.matmul(ppa[:, 0:n], wp2_t[:, 0:128], ao2[:, 0:n],
                                 start=False, stop=True)
                nc.tensor.matmul(ppb[:, 0:n], wp1_t[:, 128:192], ao1[:, 0:n],
                                 start=True, stop=False)
                nc.tensor.matmul(ppb[:, 0:n], wp2_t[:, 128:192], ao2[:, 0:n],
                                 start=False, stop=True)
                nc.scalar.copy(oa[:, n0:n0 + n], ppa[:, 0:n])
                nc.scalar.copy(ob[:, n0:n0 + n], ppb[:, 0:n])
                eng = nc.sync if (s == 3 and n0 + n == SPXO) else nc.gpsimd
                eng.dma_start(
                    io["outp"][0:128, o0 + n0:o0 + n0 + n], oa[:, n0:n0 + n])
                eng.dma_start(
                    io["outp"][128:192, o0 + n0:o0 + n0 + n],
                    ob[:, n0:n0 + n])

        # deferred conv-v for stripes 2,3: fills PE during the collective
        def p2_conv(s):
            tv_a, tv_b = tv_tiles[s]
            xa16, xb16 = x_tiles.pop(s)
            for n0, n in _chunks(SPXI, MMCH):
                r0, nr = n0 // W, n // W
                cv0 = pps.tile([120, MMCH], F32, tag="ops1", bufs=2,
                               name=f"dcv0_{s}_{n0}")
                cv1 = pps.tile([72, MMCH], F32, tag="ops2", bufs=2,
                               name=f"dcv1_{s}_{n0}")
                nc.tensor.matmul(cv0[:, 0:n], wv_a[:, 0:120],
                                 xa16[:, n0:n0 + n], start=True, stop=False)
                nc.tensor.matmul(cv0[:, 0:n], wv_b[:, 0:120],
                                 xb16[:, n0:n0 + n], start=False, stop=True)
                nc.tensor.matmul(cv1[:, 0:n], wv_a[:, 120:192],
                                 xa16[:, n0:n0 + n], start=True, stop=False)
                nc.tensor.matmul(cv1[:, 0:n], wv_b[:, 120:192],
                                 xb16[:, n0:n0 + n], start=False, stop=True)
                tvad = tv_a[:].rearrange("p (r w) -> p r w", w=WS)
                tvbd = tv_b[:].rearrange("p (r w) -> p r w", w=WS)
                nc.scalar.copy(
                    tvad[0:120, r0:r0 + nr, 2:130],
                    cv0[:, 0:n].rearrange("p (r w) -> p r w", w=W))
                nc.vector.tensor_copy(
                    tvbd[0:72, r0:r0 + nr, 2:130],
                    cv1[:, 0:n].rearrange("p (r w) -> p r w", w=W))

        qn_red = small.tile([128, 3], F32, tag="qnr")
        nc.vector.tensor_reduce(qn_red[:, 0:1], g0m[:], AX.X, ALU.add)
        nc.vector.tensor_reduce(qn_red[:, 1:2], g1m[:], AX.X, ALU.add)
        nc.vector.tensor_reduce(qn_red[:, 2:3], g2m[:], AX.X, ALU.add)
        # norm staging: na = [qn(q0:128) | kn(k0:128)], nb = rows 128:192
        na = small.tile([128, 2], F32, tag="na")
        nb = small.tile([64, 2], F32, tag="nb")
        nc.vector.tensor_copy(na[:, 0:1], qn_red[:, 0:1])
        nc.scalar.copy(na[0:64, 1:2], qn_red[64:128, 1:2])
        nc.scalar.copy(na[64:128, 1:2], qn_red[0:64, 2:3])
        nc.vector.tensor_copy(nb[:, 0:1], qn_red[0:64, 1:2])
        nc.scalar.copy(nb[:, 1:2], qn_red[64:128, 2:3])

        bounce_in = drm.tile([C, 26], F32)
        bounce_out = drm.tile([2 * C, 26], F32)
        nc.sync.dma_start(
            bounce_in[0:120, 0:CH].rearrange("(h c) k -> c h k", c=CH),
            hAB_sb[:, 0:120].rearrange("c (h k) -> c h k", h=5))
        nc.sync.dma_start(bounce_in[120:128, 0:CH], hAB_sb[0:8, 120:144])
        nc.sync.dma_start(bounce_in[128:144, 0:CH], hAB_sb[0:16, 192:216])
        nc.sync.dma_start(
            bounce_in[144:192, 0:CH].rearrange("(h c) k -> c h k", c=CH),
            hAB_sb[:, 144:192].rearrange("c (h k) -> c h k", h=2))
        nc.sync.dma_start(bounce_in[0:128, 24:26], na[:])
        nc.sync.dma_start(bounce_in[128:192, 24:26], nb[:])

        nc.gpsimd.collective_compute(
            "AllGather", ALU.bypass,
            replica_groups=[[0, 1], [2, 3], [4, 5], [6, 7]],
            ins=[bounce_in[:].opt()], outs=[bounce_out[:].opt()])

        with tc.tile_wait_until(0.090):
            p2_conv(2)
            p2_conv(3)
        p2_vbdw(0)
        p2_vbdw(1)
        p2_vbdw(2)
        p2_vbdw(3)

        # one DMA pulls all 384 gathered rows as [128, 3, 26]; the local
        # add then combines row j*128+p blocks (64-aligned cross-base)
        cmp3 = small.tile([128, 3 * 26], F32, tag="cmp3")
        nc.sync.dma_start(
            cmp3[:].rearrange("p (j k) -> p j k", j=3),
            bounce_out[:].rearrange("(j p) k -> p j k", j=3))
        c3v = cmp3[:].rearrange("p (j k) -> p j k", j=3)
        cmp_a = small.tile([128, 26], F32, tag="cmpa")
        cmp_b = small.tile([64, 26], F32, tag="cmpb")
        tmp_ab = small.tile([128, 26], F32, tag="cmptmp")
        nc.vector.tensor_copy(tmp_ab[0:64, :], c3v[64:128, 1, :])
        nc.vector.tensor_copy(tmp_ab[64:128, :], c3v[0:64, 2, :])
        nc.vector.tensor_tensor(cmp_a[:], c3v[:, 0, :], tmp_ab[:], ALU.add)
        tmp_b = small.tile([64, 26], F32, tag="cmptmpb")
        nc.scalar.copy(tmp_b[:], c3v[64:128, 2, :])
        nc.vector.tensor_tensor(cmp_b[:], c3v[0:64, 1, :], tmp_b[:],
                                ALU.add)

        kn8 = small.tile([HEADS, CH], F32, tag="kn8")
        kn8b = small.tile([HEADS, CH], F32, tag="kn8x")
        nc.gpsimd.dma_start(
            kn8[:],
            bounce_out[0:C, :].rearrange("(h c) k -> h c k", c=CH)[:, :, 25])
        nc.gpsimd.dma_start(
            kn8b[:],
            bounce_out[C:2 * C, :].rearrange("(h c) k -> h c k",
                                             c=CH)[:, :, 25])
        nc.vector.tensor_tensor(kn8[:], kn8[:], kn8b[:], ALU.add)

        # rq = temp/sqrt(qn); rk = 1/sqrt(kn) as [8,24]
        rq_a = small.tile([128, 3], F32, tag="rqa")
        rq_b = small.tile([64, 3], F32, tag="rqb")
        for ti, (cmp, rq, nrow) in enumerate(((cmp_a, rq_a, 128),
                                              (cmp_b, rq_b, 64))):
            nc.scalar.activation(rq[:, 0:1], cmp[:, 24:25], ACTF.Sqrt)
            nc.vector.reciprocal(rq[:, 1:2], rq[:, 0:1])
            nc.vector.tensor_scalar(rq[:, 2:3], rq[:, 1:2],
                                    tmpq_t[0:nrow, ti:ti + 1], None, ALU.mult)
        rk8 = small.tile([HEADS, 2 * CH], F32, tag="rk8")
        nc.scalar.activation(rk8[:, 0:CH], kn8[:], ACTF.Sqrt)
        nc.vector.reciprocal(rk8[:, CH:2 * CH], rk8[:, 0:CH])

        knb_a = pps.tile([128, CH], F32, tag="ppa", name="knb_ps", bufs=2)
        nc.tensor.matmul(knb_a[:], em_t[:, 0:128], rk8[:, CH:2 * CH],
                         start=True, stop=True)
        knb_b = pps.tile([128, CH], F32, tag="ppa", name="knb_ps2", bufs=2)
        nc.tensor.matmul(knb_b[0:64, :], em_t[:, 128:192],
                         rk8[:, CH:2 * CH], start=True, stop=True)

        # s = A*rq*knb ; softmax over d (free dim)
        attn16 = small.tile([128, CH], BF16, tag="att16a")
        attn16b = small.tile([64, CH], BF16, tag="att16b")
        for cmp, rq, knb, a16, nrow in (
                (cmp_a, rq_a, knb_a[:], attn16, 128),
                (cmp_b, rq_b, knb_b[0:64, :], attn16b, 64)):
            at = small.tile([128, CH], F32, tag="atf")
            sm = small.tile([128, 4], F32, tag="sm")
            nc.vector.tensor_scalar(at[0:nrow, :], cmp[0:nrow, 0:CH], rq[:, 2:3],
                                    None, ALU.mult)
            nc.vector.tensor_tensor(at[0:nrow, :], at[0:nrow, :], knb,
                                    ALU.mult)
            nc.scalar.activation(at[0:nrow, :], at[0:nrow, :], ACTF.Exp)
            nc.vector.tensor_reduce(sm[0:nrow, 1:2], at[0:nrow, :], AX.X, ALU.add)
            nc.vector.reciprocal(sm[0:nrow, 2:3], sm[0:nrow, 1:2])
            nc.vector.tensor_scalar(a16[0:nrow, :], at[0:nrow, :],
                                    sm[0:nrow, 2:3], None, ALU.mult)

        # block-diag attn^T via DRAM round-trip (transposing strided DMAs),
        # split across the HWDGE (sync) and SWDGE (gpsimd) queues
        attn_d = drm.tile([C, CH], BF16)
        nc.sync.dma_start(attn_d[0:128, :], attn16[:])
        nc.gpsimd.dma_start(attn_d[128:192, :], attn16b[:])
        bd1 = small.tile([120, 120], BF16, tag="bd1")      # heads 0-4 attn^T
        bd2 = small.tile([72, 72], BF16, tag="bd2")        # heads 5-7 attn^T
        nc.vector.memset(bd1[:], 0.0)
        nc.vector.memset(bd2[:], 0.0)
        for h in range(5):
            r0 = h * CH
            nc.sync.dma_start(
                bd1[r0:r0 + CH, r0:r0 + CH],
                attn_d[r0:r0 + CH, :].rearrange("c d -> d c"))
        for h in range(5, 8):
            r0 = (h - 5) * CH
            nc.gpsimd.dma_start(
                bd2[r0:r0 + CH, r0:r0 + CH],
                attn_d[h * CH:(h + 1) * CH, :].rearrange("c d -> d c"))
        # fold depthwise-v tap weights into attn^T: bd1_tap = bd1 * w_v[d,t]
        bd1t = small.tile([120, 9 * 120], BF16, tag="bd1t")
        for ti in range(9):
            nc.vector.tensor_scalar(bd1t[:, ti * 120:(ti + 1) * 120], bd1[:],
                                    dvw1_t[:, ti:ti + 1], None, ALU.mult)

        p2_attn(0)
        p2_attn(1)
        p2_attn(2)
        p2_attn(3)


# ======================================================================
def _prep_inputs(x, y, qkv_w, dw_w, proj_w, temperature):
    wq_t = np.ascontiguousarray(qkv_w[0:C].T)          # [in, out]
    wk_t = np.ascontiguousarray(qkv_w[C:2 * C].T)
    wv_t = np.ascontiguousarray(qkv_w[2 * C:3 * C].T)
    wp_t = np.ascontiguousarray(proj_w.T)

    def planes2(w):
        out = np.zeros((128, 2, C), np.float32)
        out[:, 0, :] = w[0:128]
        out[0:64, 1, :] = w[128:192]
        return out.reshape(128, 2 * C).astype(F8NP)

    wq8, wk8 = planes2(wq_t), planes2(wk_t)
    wv16 = wv_t.astype(BF16NP)
    wp1 = wp_t[0:120].astype(BF16NP)
    wp2 = wp_t[120:192].astype(BF16NP)

    dw = dw_w.reshape(3 * C, 9).astype(np.float32)
    dw_q, dw_k, dw_v = dw[0:C], dw[C:2 * C], dw[2 * C:3 * C]
    dqk = np.concatenate([dw_q[0:128], dw_q[128:192], dw_k[0:64],
                          dw_k[64:192]], axis=0)
    dqkd = np.zeros((3 * 128, 9 * 128), np.float32)
    for i in range(3):
        for t in range(9):
            blk = dqk[i * 128:(i + 1) * 128, t]
            np.fill_diagonal(
                dqkd[i * 128:(i + 1) * 128, t * 128:(t + 1) * 128], blk)
    tmpq = np.repeat(np.asarray(temperature, np.float32).reshape(HEADS),
                     CH).reshape(C, 1)
    em = np.zeros((HEADS, C), np.float32)
    for hh in range(HEADS):
        em[hh, hh * CH:(hh + 1) * CH] = 1.0

    in_maps = []
    for core in range(8):
        bi, half = core // 2, core % 2
        r0 = half * HOUT - 1
        xsl = np.zeros((C, HIN, W), np.float32)
        ysl = np.zeros((C, HIN, W), np.float32)
        lo, hi = max(r0, 0), min(r0 + HIN, 128)
        xsl[:, lo - r0:hi - r0] = x[bi, :, lo:hi]
        ysl[:, lo - r0:hi - r0] = y[bi, :, lo:hi]
        xsl = xsl.reshape(C, PXIN)
        ysl = ysl.reshape(C, PXIN)

        def planes_px(t):
            out = np.zeros((128, 2, PXIN), np.float32)
            out[:, 0, :] = t[0:128]
            out[0:64, 1, :] = t[128:192]
            return out.reshape(128, 2 * PXIN).astype(F8NP)

        in_maps.append({
            "xs": xsl.astype(BF16NP),
            "xs8": planes_px(xsl), "ys8": planes_px(ysl),
            "wq8": wq8, "wk8": wk8, "wv": wv16,
            "wp1": wp1, "wp2": wp2,
            "dqkd": dqkd.astype(F8NP),
            "dvw": dw_v.astype(np.float32),
            "tmpq": tmpq, "em": em,
            "eye": np.eye(128, dtype=np.float32),
        })
    return in_maps


def kernel(x, y, qkv_w, dw_w, proj_w, temperature, _trace=False):
    x = np.asarray(x, np.float32)
    y = np.asarray(y, np.float32)
    if "nc" not in _CACHE:
        _CACHE["nc"] = build_program()
    nc = _CACHE["nc"]
    in_maps = _prep_inputs(x, y, np.asarray(qkv_w, np.float32),
                           np.asarray(dw_w, np.float32),
                           np.asarray(proj_w, np.float32),
                           np.asarray(temperature, np.float32))
    res = bass_utils.run_bass_kernel_spmd(nc, in_maps,
                                          core_ids=list(range(8)),
                                          trace=_trace)
    _CACHE["last_result"] = res
    out = np.empty((4, C, 128, W), np.float32)
    for core in range(8):
        bi, half = core // 2, core % 2
        out[bi, :, half * HOUT:(half + 1) * HOUT] = \
            res.results[core]["outp"].reshape(C, HOUT, W)
    return out
